# revision 33
# baseline (speedup 1.0000x reference)
"""Trainium2 Bass kernel for AudioOnlyGNN (3-layer GCN + BatchNorm + mean-pool + MLP).

v3 — graph-partitioned static slot stream:

Nodes are assigned to cores by *graph* ownership (8 graphs per core,
balanced by node count), then degree-sorted within each core and laid out in
128-row tiles; tile t's slot budget k_t = max in-degree(+self) over that tile
across all cores, giving a static slot stream identical on every core.  For
each layer the host materialises the edge-source rows in slot order (a pure
gather) so the device reads large contiguous DMA blocks.

On device, a 128-slot block contributes to a [F, ncols] PSUM tile via one
matmul whose moving operand is a narrow "panel" (slot -> dst column weight
with the GCN normalisation baked in).  The aggregate is transformed
(W^T @ agg), bias/BN-shift added as rank-1 matmuls, ReLU'd, written back.
Layers 0/1 write h' = dinv*ReLU(...) so panels never depend on h.

Because every graph lives entirely on one core, the mean-pool and classifier
MLP complete locally inside the L2 launch (no cross-core reduction): launches
are [stats+pre-agg] [L0] [L1] [L2+pool+mlp].  Between launches the host only
reorders bytes (gather / transpose), never does arithmetic on activations.
"""

import sys

sys.path.insert(0, "/opt/trn_rl_repo")

import contextlib

import numpy as np
import ml_dtypes

import concourse.bacc as bacc
import concourse.bass as bass
import concourse.mybir as mybir
from concourse.tile import TileContext
from concourse.bass_utils import run_bass_kernel_spmd

BF16 = mybir.dt.bfloat16
F32 = mybir.dt.float32
FP8 = mybir.dt.float8e3  # e3m4

NPBF16 = ml_dtypes.bfloat16
NPFP8 = ml_dtypes.float8_e3m4

N_CORES = 8
BN_EPS = 1e-5
G = 64
G_PER = G // N_CORES   # graphs per core
TPRE = 0               # tiles of L0 pre-aggregated inside the stats launch

# dtype of the host-expanded per-slot source rows, per layer
DUP_DT = [FP8, FP8, FP8]
DUP_NP = [NPFP8, NPFP8, NPFP8]
OUT_DT = [FP8, FP8]
OUT_NP = [NPFP8, NPFP8]


def _chunk_list(n0, n1, lead, mid, tail=(4, 2, 1)):
    """Chunk [n0, n1) into sizes lead + [mid...] + tail (tapered ends)."""
    n = n1 - n0
    sizes = []
    for s in lead:
        if sum(sizes) + s > n:
            break
        sizes.append(s)
    tl = [s for s in tail if s < mid]
    while sum(sizes) + sum(tl) + mid <= n:
        sizes.append(mid)
    rem = n - sum(sizes) - sum(tl)
    while rem > 0:
        add = min(rem, mid)
        sizes.append(add)
        rem -= add
    sizes += tl
    sizes = [s for s in sizes if s > 0]
    # clip overflow
    while sum(sizes) > n:
        sizes[-1] -= sum(sizes) - n
        sizes = [s for s in sizes if s > 0]
    out = []
    t = n0
    for cs in sizes:
        out.append(list(range(t, t + cs)))
        t += cs
    assert t == n1, (sizes, n0, n1)
    return out


# ------------------------------------------------------------------ planning
def _plan(src, dst, batch, n_true):
    """Static (h-independent) structure: graph packing, renumbering, slots."""
    cnt_g = np.bincount(batch, minlength=G).astype(np.int64)
    g_order = np.argsort(-cnt_g, kind="stable")
    core_graphs = [[] for _ in range(N_CORES)]
    loads = np.zeros(N_CORES, np.int64)
    for g in g_order:
        cand = [i for i in range(N_CORES) if len(core_graphs[i]) < G_PER]
        i = min(cand, key=lambda i: loads[i])
        core_graphs[i].append(int(g))
        loads[i] += cnt_g[g]
    NT = max(49, int(-(-loads.max() // 128)))
    SHARD = NT * 128
    NPAD = N_CORES * SHARD

    graph_core = np.zeros(G, np.int64)
    graph_local = np.zeros(G, np.int64)
    for c in range(N_CORES):
        for lg, g in enumerate(core_graphs[c]):
            graph_core[g] = c
            graph_local[g] = lg

    degp_true = np.bincount(dst, minlength=n_true).astype(np.int64) + 1
    node_core = graph_core[batch]

    order = np.empty(NPAD, np.int64)
    virt = n_true
    for c in range(N_CORES):
        nodes_c = np.where(node_core == c)[0]
        nodes_c = nodes_c[np.argsort(degp_true[nodes_c], kind="stable")]
        npadc = SHARD - len(nodes_c)
        ids = np.concatenate([np.arange(virt, virt + npadc), nodes_c])
        virt += npadc
        idx = ((np.arange(NT) * N_CORES + c)[:, None] * 128
               + np.arange(128)[None, :])
        order[idx.ravel()] = ids
    assert virt == NPAD
    newpos = np.empty(NPAD, np.int64)
    newpos[order] = np.arange(NPAD)

    degp = np.zeros(NPAD, np.int64)
    degp[:n_true] = degp_true

    kt = np.zeros(NT, np.int64)
    for t in range(NT):
        kt[t] = degp[order[t * 1024:(t + 1) * 1024]].max()
    kt = np.maximum(kt, 1)

    blocks = []   # per tile: list of (lo, w)
    pan_cols = []  # per tile: list of panel col offsets
    wtot = 0
    for t in range(NT):
        k = int(kt[t])
        bl = []
        for b in range(k):
            lo = (128 * b) // k
            hi = (128 * (b + 1) - 1) // k
            bl.append((lo, hi - lo + 1))
        blocks.append(bl)
        offs = []
        for lo, w in bl:
            offs.append(wtot)
            wtot += w
        pan_cols.append(offs)

    nblk = int(kt.sum())
    tile_base = np.zeros(NT + 1, np.int64)
    tile_base[1:] = np.cumsum(128 * kt)
    meta = {"kt": kt, "blocks": blocks, "pan_cols": pan_cols,
            "wtot": wtot, "nblk": nblk, "order": order, "newpos": newpos,
            "n_true": n_true, "tile_base": tile_base,
            "total_slots": int(tile_base[-1]),
            "NT": NT, "SHARD": SHARD, "NPAD": NPAD,
            "core_graphs": core_graphs, "graph_core": graph_core,
            "graph_local": graph_local, "cnt_g": cnt_g}
    return meta


def _build_static(meta, src, dst, batch):
    """Per-core constant tables: slot->src map, per-layer panels, rows."""
    kt, blocks, pan_cols = meta["kt"], meta["blocks"], meta["pan_cols"]
    wtot, nblk, order, newpos = (meta["wtot"], meta["nblk"], meta["order"],
                                 meta["newpos"])
    n_true = meta["n_true"]
    NT, SHARD, NPAD = meta["NT"], meta["SHARD"], meta["NPAD"]
    graph_local, cnt_g = meta["graph_local"], meta["cnt_g"]

    deg = np.bincount(dst, minlength=NPAD).astype(np.float64) + 1.0
    dinv = (1.0 / np.sqrt(deg)).astype(np.float64)
    dinv_pad = dinv.copy()
    dinv_pad[n_true:] = 1.0

    dinv_new = dinv_pad[order]
    batch_pad = np.full(NPAD, 0, np.int64)
    batch_pad[:n_true] = batch
    batch_new = batch_pad[order]
    valid_new = (order < n_true)

    sneig = np.bincount(dst, weights=dinv[src], minlength=NPAD)
    d2 = dinv_pad * (sneig + dinv_pad)
    d2_new = d2[order]

    cntx = np.maximum(cnt_g.astype(np.float64), 1.0)   # [G]
    invc = 1.0 / cntx

    s_new = newpos[src]
    d_new = newpos[dst]
    g_tile = d_new // 128
    core_of = g_tile % N_CORES
    tloc = g_tile // N_CORES
    dloc = d_new % 128

    tile_base = meta["tile_base"]
    total_slots = meta["total_slots"]

    edge_w0 = dinv[src] * dinv_pad[dst] * dinv_pad[dst]

    cores = []
    for c in range(N_CORES):
        sel = core_of == c
        es, et, ed = s_new[sel], tloc[sel], dloc[sel]
        ew0 = edge_w0[sel]
        key = et * (128 * 64) + ed
        o = np.argsort(key, kind="stable")
        es, et, ed, ew0 = es[o], et[o], ed[o], ew0[o]
        k_of = kt[et]
        node_key = et * 128 + ed
        uniq, first_idx, counts = np.unique(node_key, return_index=True,
                                            return_counts=True)
        rank = np.arange(len(node_key)) - np.repeat(first_idx, counts)
        slot = tile_base[et] + ed * k_of + 1 + rank   # +1: self slot at 0

        tt = np.arange(NT).repeat(128)
        dd = np.tile(np.arange(128), NT)
        own_new = (tt * N_CORES + np.full(NT * 128, c)) * 128 + dd
        own_valid = valid_new[own_new]
        self_slot = tile_base[tt] + dd * kt[tt]

        slotsrc = np.full(total_slots, NPAD, np.int64)  # NPAD -> zero row
        slotsrc[slot] = es
        slotsrc[self_slot[own_valid]] = own_new[own_valid]

        dv_own = dinv_new[own_new]
        w_l0 = np.zeros(total_slots, np.float64)
        w_l0[slot] = ew0
        w_l0[self_slot[own_valid]] = (dv_own ** 3)[own_valid]
        col_dinv = np.repeat(dv_own, np.repeat(kt, 128))
        filled = np.zeros(total_slots, bool)
        filled[slot] = True
        filled[self_slot[own_valid]] = True
        w_l1 = np.where(filled, col_dinv ** 2, 0.0)
        w_l2 = np.where(filled, col_dinv, 0.0)

        pans = []
        for wv, psc in ((w_l0, 8.0), (w_l1, 8.0), (w_l2, 4.0)):
            pan = np.zeros((128, wtot), np.float64)
            for t in range(NT):
                k = int(kt[t])
                for b, (lo, w) in enumerate(blocks[t]):
                    co = pan_cols[t][b]
                    sl0 = tile_base[t] + b * 128
                    ss = np.arange(sl0, sl0 + 128)
                    cc = (ss - tile_base[t]) // k - lo
                    ok = (cc >= 0) & (cc < w)
                    pan[np.arange(128)[ok], co + cc[ok]] = wv[ss][ok]
            pans.append((pan * psc).astype(NPFP8))

        sig_row = np.zeros(SHARD, np.float64)
        sh_row = np.zeros(SHARD, np.float64)
        for t in range(NT):
            cols = slice(t * 128, (t + 1) * 128)
            nn = (t * N_CORES + c) * 128 + np.arange(128)
            sig_row[cols] = dinv_new[nn]
            sh_row[cols] = d2_new[nn] * dinv_new[nn]

        # pool panel [128, NT*G_PER]: 1.0 at (d, t*G_PER + local_graph)
        gpan = np.zeros((128, NT * G_PER), np.float64)
        for t in range(NT):
            nn = (t * N_CORES + c) * 128 + np.arange(128)
            gb = graph_local[batch_new[nn]]
            ok = valid_new[nn]
            gpan[np.arange(128)[ok], t * G_PER + gb[ok]] = 1.0

        cg = meta["core_graphs"][c]
        cores.append({
            "slotsrc": slotsrc,
            "pans": pans,
            "sig_row": sig_row,
            "sh_row": sh_row,
            "gpan": gpan.astype(NPBF16),
            "cntx": cntx[cg].astype(np.float32),     # [G_PER]
            "invc": invc[cg].astype(np.float32),     # [G_PER]
        })
    return cores


def _dup_layout(h_new, slotsrc, np_dt):
    """[NPAD(+1), F] new-indexed rows -> [128, NBLK*F] slot-stream layout."""
    rows = h_new[slotsrc]
    nblk = rows.shape[0] // 128
    F = rows.shape[1]
    return np.ascontiguousarray(
        rows.reshape(nblk, 128, F).transpose(1, 0, 2)
    ).reshape(128, nblk * F).astype(np_dt)


# ------------------------------------------------------------------ programs
def _build_stats_program(meta):
    """Per-core BN partial sums (Sum x, Sum x^2 over own nodes)."""
    F = 128
    NT = meta["NT"]
    nc = bacc.Bacc("TRN2", target_bir_lowering=False, debug=False,
                   num_devices=N_CORES)
    xs_d = nc.dram_tensor("x_sh", [128, NT * F], FP8,
                          kind="ExternalInput").ap()
    ident_d = nc.dram_tensor("ident", [128, 128], F32,
                             kind="ExternalInput").ap()
    out_d = nc.dram_tensor("stat_part", [128, 2], F32,
                           kind="ExternalOutput").ap()
    XS = [0, 10, 22, 35, NT]
    with TileContext(nc) as tc:
        with tc.tile_pool(name="w", bufs=1) as wp, \
             tc.tile_pool(name="ps", bufs=1, space="PSUM") as pp:
            xs = wp.tile([128, NT * F], FP8, tag="xs")
            ident_s = wp.tile([128, 128], F32, tag="id")
            nc.sync.dma_start(out=xs[:, :XS[1] * F], in_=xs_d[:, :XS[1] * F])
            nc.scalar.dma_start(out=ident_s[:], in_=ident_d[:])
            for q in range(1, 4):
                nc.sync.dma_start(out=xs[:, XS[q] * F:XS[q + 1] * F],
                                  in_=xs_d[:, XS[q] * F:XS[q + 1] * F])
            ones_s = wp.tile([128, 1], FP8, tag="ones")
            nc.vector.memset(ones_s[:], 1.0)
            xtx_ps = pp.tile([128, 128], F32, tag="xtx")
            sx_ps = pp.tile([128, 1], F32, tag="sx")
            for t in range(NT):
                sl = xs[:, t * F:(t + 1) * F]
                nc.tensor.matmul(xtx_ps[:], sl, sl, start=(t == 0),
                                 stop=(t == NT - 1), skip_group_check=True)
                nc.tensor.matmul(sx_ps[:], sl, ones_s[:],
                                 start=(t == 0), stop=(t == NT - 1),
                                 skip_group_check=True)
            dg = wp.tile([128, 128], F32, tag="dg")
            nc.vector.tensor_tensor(dg[:], xtx_ps[:], ident_s[:],
                                    mybir.AluOpType.mult)
            o = wp.tile([128, 2], F32, tag="o")
            nc.vector.tensor_reduce(o[:, 1:2], dg[:], mybir.AxisListType.X,
                                    mybir.AluOpType.add)
            nc.vector.tensor_copy(o[:, 0:1], sx_ps[:])
            nc.scalar.dma_start(out=out_d[:], in_=o[:])
    nc.compile()
    return nc


def _build_layer_program(meta, lay):
    kt, blocks, pan_cols, wtot, nblk, tile_base = (
        meta["kt"], meta["blocks"], meta["pan_cols"], meta["wtot"],
        meta["nblk"], meta["tile_base"])
    NT, SHARD = meta["NT"], meta["SHARD"]
    F = 128 if lay < 2 else 64
    H = 128
    H2 = 64
    H4 = 32
    C = 2
    Ho = H if lay < 2 else H2
    N_true = meta["n_true"]
    dt_in = DUP_DT[lay]
    dt_out = OUT_DT[lay] if lay < 2 else None

    nc = bacc.Bacc("TRN2", target_bir_lowering=False, debug=False,
                   num_devices=N_CORES)

    def din(name, shape, dt):
        return nc.dram_tensor(name, list(shape), dt, kind="ExternalInput").ap()

    dup_d = din("dup", [128, nblk * F], dt_in)
    if lay == 2:
        PW_EXTRA = NT * G_PER       # gpan (0/1: fp8-exact)
    else:
        PW_EXTRA = 0               # W1 in f32pack; W2|W3 in wpack
    pan_d = din("pan", [128, wtot + PW_EXTRA], FP8)
    if lay == 1:
        wp_d = din("wpack", [128, H + H2], BF16)
    # packed bf16 row constants
    if lay == 0:
        RP = 2 * SHARD + H        # sig | sh | b1
    elif lay == 1:
        RP = SHARD + H            # sig | b2
    else:
        RP = 1                    # b3 as a column
    rp_d = din("rowpack", [1, RP] if lay < 2 else [128, 65], BF16)
    if lay == 0:
        # sxp | exp | gamma | beta | W1(fp32)
        fp_d = din("f32pack", [128, 18 + H], F32)
        if TPRE:
            agp_d = din("aggT_pre", [128, TPRE * 128], BF16)
    if lay == 2:
        # mlp pack: Wc1 | Wc2 | bc1row | cntx | bc2 | invc  (f32)
        mp_d = din("mpack", [64, 80], F32)
        out_d = nc.dram_tensor("out", [G_PER, C], F32,
                               kind="ExternalOutput").ap()
    else:
        OW = 128 if lay == 0 else 64
        h_out = nc.dram_tensor("h_out", [OW, NT * 128], dt_out,
                               kind="ExternalOutput").ap()

    # process tiles high->low: degree sorting puts fat tiles at high
    # indices, so the tail (last chunk + final write) covers thin tiles.
    T0 = TPRE if lay == 0 else 0
    fwd = _chunk_list(T0, NT, [2, 2, 4], 8,
                      tail=(4, 2) if lay < 2 else (2,))
    chunk_tiles = []
    hi = NT
    for ch in fwd:
        chunk_tiles.append(list(range(hi - len(ch), hi)))
        hi -= len(ch)
    assert hi == T0
    PBASE = pan_cols[TPRE][0] if lay == 0 else 0

    with TileContext(nc) as tc:
        with contextlib.ExitStack() as ctx:
            cpool = ctx.enter_context(tc.tile_pool(name="const", bufs=1))
            dpool = ctx.enter_context(tc.tile_pool(name="dup", bufs=5))
            ppool = ctx.enter_context(tc.tile_pool(name="pan", bufs=2))

            def chunk_loads(tiles):
                ct0, ct1 = tiles[0], tiles[-1] + 1
                b0 = int(tile_base[ct0] // 128)
                b1 = int(tile_base[ct1] // 128)
                dup_sb = dpool.tile([128, (b1 - b0) * F], dt_in, tag="dup")
                nc.sync.dma_start(out=dup_sb[:], in_=dup_d[:, b0 * F:b1 * F])
                return dup_sb, b0

            pend = [chunk_loads(chunk_tiles[0])]
            pan_sb = ppool.tile([128, wtot - PBASE + PW_EXTRA], FP8,
                                tag="pan")
            if lay == 1:
                wpk_s = cpool.tile([128, H + H2], BF16, tag="c_wpk")
                nc.scalar.dma_start(out=wpk_s[:], in_=wp_d[:])
            fst = NT - 12
            PAN_OFF = PW_EXTRA
            PSPLIT = PAN_OFF + pan_cols[fst][0] - PBASE
            nc.sync.dma_start(out=pan_sb[:, PSPLIT:],
                              in_=pan_d[:, PBASE + PSPLIT:])
            if PW_EXTRA:
                nc.scalar.dma_start(out=pan_sb[:, :PW_EXTRA],
                                    in_=pan_d[:, PBASE:PBASE + PW_EXTRA])

            rp_s = cpool.tile([1, RP] if lay < 2 else [128, 65], BF16,
                              tag="c_rp")
            (nc.scalar if lay == 0 else nc.sync).dma_start(
                out=rp_s[:], in_=rp_d[:])
            if lay == 0:
                fp_s = cpool.tile([128, 18 + H], F32, tag="c_fp")
                nc.scalar.dma_start(out=fp_s[:], in_=fp_d[:])
                if TPRE:
                    agp_s = cpool.tile([128, TPRE * 128], BF16, tag="c_agp")
                    nc.scalar.dma_start(out=agp_s[:], in_=agp_d[:])
            if lay == 2:
                mp_s = cpool.tile([64, 80], F32, tag="c_mp")
                nc.scalar.dma_start(out=mp_s[:], in_=mp_d[:])
            nc.sync.dma_start(out=pan_sb[:, PAN_OFF:PSPLIT],
                              in_=pan_d[:, PBASE + PAN_OFF:PBASE + PSPLIT])
            if lay == 0:
                sig_s = rp_s[0:1, 0:SHARD]
                sh_s = rp_s[0:1, SHARD:2 * SHARD]
                b_s = rp_s[0:1, 2 * SHARD:2 * SHARD + H]
            elif lay == 1:
                sig_s = rp_s[0:1, 0:SHARD]
                b_s = rp_s[0:1, SHARD:SHARD + H]
            else:
                b_s = mp_s[0:H2, 78:79]   # [H2, 1] f32 column
            zr_s = cpool.tile([1, 512], BF16, tag="c_zr")
            nc.vector.memset(zr_s[:], 0.0)
            if lay == 0:
                w1f_s = fp_s[:, 18:18 + H]
                w_s = cpool.tile([128, H], BF16, tag="c_wt")
                rw_s = cpool.tile([1, H], BF16, tag="c_rw")
            elif lay == 1:
                w_s = wpk_s[:, 0:H]
                w3_s = wpk_s[:, H:H + H2]
            else:
                gpan_s = pan_sb[:, 0:NT * G_PER]
                id_s = rp_s[:, 0:64]
                wc1_s = mp_s[:, 0:H4]                  # [64, 32]
                wc2_s = mp_s[0:H4, H4:H4 + C]          # [32, 2]
                bc1_r = mp_s[0:1, 34:66]               # [1, 32]
                cntx_r = mp_s[0:1, 66:74]              # [1, 8]
                bc2_r = mp_s[0:1, 74:76]               # [1, 2]
                invc_c = mp_s[0:G_PER, 76:77]          # [8, 1]

            # ---- BN statistics (layer 0) -> W~1 and shift row rw
            if lay == 0:
                with tc.tile_pool(name="ps_st", bufs=1, space="PSUM") as pst, \
                     tc.tile_pool(name="st_w", bufs=2) as stw:
                    sxp_s = fp_s[:, 0:8]
                    exp_s = fp_s[:, 8:16]
                    gam_s = fp_s[:, 16:17]
                    bet_s = fp_s[:, 17:18]
                    ex2 = stw.tile([128, 1], F32, tag="v1")
                    nc.vector.tensor_reduce(ex2[:], exp_s,
                                            mybir.AxisListType.X,
                                            mybir.AluOpType.add)
                    sx = stw.tile([128, 1], F32, tag="v0")
                    nc.vector.tensor_reduce(sx[:], sxp_s,
                                            mybir.AxisListType.X,
                                            mybir.AluOpType.add)
                    mu = stw.tile([128, 1], F32, tag="v2")
                    nc.vector.tensor_scalar_mul(mu[:], sx[:], 1.0 / N_true)
                    var = stw.tile([128, 1], F32, tag="v3")
                    nc.vector.tensor_scalar_mul(var[:], ex2[:], 1.0 / N_true)
                    mu2 = stw.tile([128, 1], F32, tag="v4")
                    nc.vector.tensor_tensor(mu2[:], mu[:], mu[:],
                                            mybir.AluOpType.mult)
                    nc.vector.tensor_tensor(var[:], var[:], mu2[:],
                                            mybir.AluOpType.subtract)
                    nc.vector.tensor_scalar_add(var[:], var[:], BN_EPS)
                    rec = stw.tile([128, 1], F32, tag="v5")
                    nc.vector.reciprocal(rec[:], var[:])
                    isd = stw.tile([128, 1], F32, tag="v6")
                    nc.scalar.activation(isd[:], rec[:],
                                         mybir.ActivationFunctionType.Sqrt)
                    a_c = stw.tile([128, 1], F32, tag="v7")
                    nc.vector.tensor_tensor(a_c[:], gam_s, isd[:],
                                            mybir.AluOpType.mult)
                    a8 = stw.tile([128, 1], F32, tag="v9")
                    nc.vector.tensor_scalar_mul(a8[:], a_c[:], 0.125)
                    nc.vector.tensor_scalar_mul(w_s[:], w1f_s, a8[:])
                    ca = stw.tile([128, 1], F32, tag="v8")
                    nc.vector.tensor_tensor(ca[:], mu[:], a_c[:],
                                            mybir.AluOpType.mult)
                    nc.vector.tensor_tensor(ca[:], bet_s, ca[:],
                                            mybir.AluOpType.subtract)
                    rw_ps = pst.tile([1, H], F32, tag="rw")
                    nc.tensor.matmul(rw_ps[:], ca[:], w1f_s,
                                     start=True, stop=True)
                    nc.scalar.activation(rw_s[:], rw_ps[:],
                                         mybir.ActivationFunctionType.Copy)

            spool = ctx.enter_context(tc.tile_pool(name="stg", bufs=1))
            wpool = ctx.enter_context(tc.tile_pool(name="wk", bufs=4))
            ps_agg = ctx.enter_context(
                tc.tile_pool(name="ps_agg", bufs=3, space="PSUM"))
            if lay < 2:
                ps_out = ctx.enter_context(
                    tc.tile_pool(name="ps_out", bufs=3, space="PSUM"))
            if lay == 1:
                ps_t = ctx.enter_context(
                    tc.tile_pool(name="ps_t", bufs=2, space="PSUM"))
                tstage = spool.tile([64, NT * 128], dt_out, tag="tstg")
            if lay == 2:
                ps_tr = ctx.enter_context(
                    tc.tile_pool(name="ps_tr", bufs=2, space="PSUM"))
                ps_pl = ctx.enter_context(
                    tc.tile_pool(name="ps_pl", bufs=1, space="PSUM"))
                pool_ps = ps_pl.tile([H2, G_PER], F32, tag="pool")

            if lay == 0:
                stage = spool.tile([128, NT * 128], dt_out, tag="stg")
            elif lay == 1:
                stage = spool.tile([128, NT * 128], BF16, tag="stg")

            state = {"use_dve": False, "rot": 0}

            def flip():
                state["use_dve"] = not state["use_dve"]
                return state["use_dve"]

            def rot():
                state["rot"] = (state["rot"] + 1) % 3
                return state["rot"]

            def split_copy(dst, src_ps, w):
                """PSUM->SBUF copy split across Act | DVE halves."""
                h = (w // 2 + 63) & ~63 if w > 128 else w
                nc.scalar.activation(dst[:, 0:h], src_ps[:, 0:h],
                                     mybir.ActivationFunctionType.Copy)
                if h < w:
                    nc.vector.tensor_copy(dst[:, h:w], src_ps[:, h:w])

            def split_relu(dst, src_ps, w):
                h = (w // 2 + 63) & ~63 if w > 128 else w
                nc.scalar.activation(dst[:, 0:h], src_ps[:, 0:h],
                                     mybir.ActivationFunctionType.Relu)
                if h < w:
                    nc.vector.tensor_scalar_max(dst[:, h:w],
                                                src_ps[:, h:w], 0.0)

            def split_relu_bias(dst, src_ps, w, bias):
                h = (w // 2 + 63) & ~63 if w > 128 else w
                nc.scalar.activation(dst[:, 0:h], src_ps[:, 0:h],
                                     mybir.ActivationFunctionType.Relu,
                                     bias=bias)
                if h < w:
                    nc.vector.tensor_scalar(dst[:, h:w], src_ps[:, h:w],
                                            bias, 0.0,
                                            mybir.AluOpType.add,
                                            mybir.AluOpType.max)

            def phase1(pr, dup_sb, b0):
                """agg matmuls (+ L2: bias + relu straight from PSUM)."""
                if dup_sb is None:   # lay0 tiles pre-aggregated in stats
                    return pr, None, agp_s[:, pr[0] * 128:(pr[-1] + 1) * 128]
                pw = len(pr) * 128
                rows = H2 if lay == 2 else 128
                agg_ps = ps_agg.tile([rows, pw], F32, tag="agg")
                nc.tensor.matmul(agg_ps[:], zr_s[0:1, 0:rows],
                                 zr_s[0:1, 0:pw], start=True, stop=False,
                                 skip_group_check=True)
                nb_pair = sum(int(kt[t]) for t in pr)
                bi = 0
                for hi, t in enumerate(pr):
                    for b, (lo, w) in enumerate(blocks[t]):
                        gb = int(tile_base[t] // 128) + b
                        co = pan_cols[t][b]
                        bi += 1
                        nc.tensor.matmul(
                            agg_ps[:, hi * 128 + lo:hi * 128 + lo + w],
                            dup_sb[:, (gb - b0) * F:(gb - b0 + 1) * F],
                            pan_sb[:, PAN_OFF + co - PBASE:
                                   PAN_OFF + co - PBASE + w],
                            start=False, stop=(bi == nb_pair),
                            skip_group_check=True)
                if lay == 2:
                    hsT = wpool.tile([H2, pw], BF16, tag="hsT")
                    if flip():
                        nc.vector.tensor_scalar(
                            hsT[:], agg_ps[:], b_s, 0.0,
                            mybir.AluOpType.add, mybir.AluOpType.max)
                    else:
                        nc.scalar.activation(
                            hsT[:], agg_ps[:],
                            mybir.ActivationFunctionType.Relu,
                            bias=b_s)
                    return pr, agg_ps, hsT
                aggT = wpool.tile([128, pw], BF16, tag="aggT")
                if flip():
                    nc.vector.tensor_copy(aggT[:], agg_ps[:])
                else:
                    nc.scalar.activation(aggT[:], agg_ps[:],
                                         mybir.ActivationFunctionType.Copy)
                return pr, agg_ps, aggT

            def phase2(st1):
                pr, agg_ps, aggT = st1
                pw = len(pr) * 128
                if lay < 2:
                    h_ps = ps_out.tile([Ho, pw], F32, tag="hps")
                    c0 = pr[0] * 128
                    nc.tensor.matmul(h_ps[:], w_s[:] if lay == 0 else w_s,
                                     aggT[:], start=True, stop=False,
                                     skip_group_check=True)
                    nc.tensor.matmul(h_ps[:], b_s,
                                     sig_s[0:1, c0:c0 + pw],
                                     start=False, stop=(lay != 0),
                                     skip_group_check=True)
                    if lay == 0:
                        nc.tensor.matmul(h_ps[:], rw_s[:],
                                         sh_s[0:1, c0:c0 + pw],
                                         start=False, stop=True,
                                         skip_group_check=True)
                    so = pr[0] * 128
                    if flip():
                        nc.vector.tensor_scalar_max(
                            stage[:, so:so + pw], h_ps[:], 0.0)
                    else:
                        nc.scalar.activation(
                            stage[:, so:so + pw], h_ps[:],
                            mybir.ActivationFunctionType.Relu)
                    return st1
                # lay 2: transpose each tile's hsT: [64, 128] -> [128, 64]
                hsT = aggT
                tr_ps = ps_tr.tile([128, len(pr) * H2], BF16, tag="tr")
                for hi, t in enumerate(pr):
                    nc.tensor.transpose(tr_ps[:, hi * H2:(hi + 1) * H2],
                                        hsT[:, hi * 128:(hi + 1) * 128],
                                        id_s[0:64, 0:64])
                hs_sb = wpool.tile([128, len(pr) * H2], BF16, tag="hs")
                if flip():
                    nc.vector.tensor_copy(hs_sb[:], tr_ps[:])
                else:
                    nc.scalar.activation(
                        hs_sb[:], tr_ps[:],
                        mybir.ActivationFunctionType.Copy)
                return [(t, hs_sb, hi * H2) for hi, t in enumerate(pr)]

            def phase3(st2):
                if lay == 2:
                    flip()          # odd flips/group: engines alternate
                if lay == 1:
                    pr = st2[0]
                    pw = len(pr) * 128
                    so = pr[0] * 128
                    t_ps = ps_t.tile([H2, pw], F32, tag="tps")
                    nc.tensor.matmul(t_ps[:], w3_s, stage[:, so:so + pw],
                                     start=True, stop=True,
                                     skip_group_check=True)
                    if flip():
                        nc.vector.tensor_copy(tstage[:, so:so + pw], t_ps[:])
                    else:
                        nc.scalar.activation(
                            tstage[:, so:so + pw], t_ps[:],
                            mybir.ActivationFunctionType.Copy)
                elif lay == 2:
                    for t, hs_sb, off in st2:
                        state["npool"] = state.get("npool", 0) + 1
                        nc.tensor.matmul(pool_ps[:],
                                         hs_sb[:, off:off + H2],
                                         gpan_s[:, t * G_PER:(t + 1) * G_PER],
                                         start=(state["npool"] == 1),
                                         stop=(state["npool"] == NT),
                                         skip_group_check=True)

            all_pairs = []
            for ci, tiles in enumerate(chunk_tiles):
                dup_sb, b0 = pend.pop(0)
                if ci + 1 < len(chunk_tiles):
                    pend.append(chunk_loads(chunk_tiles[ci + 1]))
                GW = 4
                grps = [tiles[i:i + GW] for i in range(0, len(tiles), GW)]
                for g in reversed(grps):
                    all_pairs.append((g, dup_sb, b0))
            if lay == 0 and TPRE:
                # pre-aggregated thin tiles: compute-only, processed last
                pg = [list(range(i, min(i + 4, TPRE)))
                      for i in range(0, TPRE, 4)]
                for g in reversed(pg):
                    all_pairs.append((g, None, None))

            hastail = lay > 0
            q2, q3 = [], []
            out_stage = stage if lay == 0 else (tstage if lay == 1 else None)
            OW = 128 if lay == 0 else 64
            wb = [NT, 24, 12, 6, 2, 0]
            WRITES = [(wb[i + 1], wb[i]) for i in range(len(wb) - 1)]

            def maybe_write(done_min):
                if lay == 2:
                    return
                while WRITES and done_min <= WRITES[0][0]:
                    wt0, wt1 = WRITES.pop(0)
                    q = nc.sync if wt0 == 0 else nc.gpsimd
                    q.dma_start(
                        out=h_out[:, wt0 * 128:wt1 * 128],
                        in_=out_stage[:, wt0 * 128:wt1 * 128])

            def run3():
                st3 = q3.pop(0)
                phase3(st3)
                done = st3[0][0] if lay == 1 else st3[0][0]
                maybe_write(done)

            def run2():
                st2 = phase2(q2.pop(0))
                if hastail:
                    q3.append(st2)
                else:
                    maybe_write(st2[0][0])

            LAG2 = 1 if lay == 1 else 2
            LAG3 = 3 if lay == 2 else 2
            for item in all_pairs:
                st1 = phase1(*item)
                if len(q3) >= LAG3:
                    run3()
                if len(q2) >= LAG2:
                    run2()
                q2.append(st1)
            while q2 or q3:
                if q3:
                    run3()
                if q2:
                    run2()

            # ---- classifier MLP on this core's G_PER graphs (lay 2)
            if lay == 2:
                p01 = wpool.tile([H2, G_PER], F32, tag="p01")
                nc.vector.tensor_copy(p01[:], pool_ps[:])
                y_ps = ps_pl.tile([H4, G_PER], F32, tag="yps")
                nc.tensor.matmul(y_ps[:], wc1_s, p01[:],
                                 start=True, stop=False,
                                 skip_group_check=True)
                nc.tensor.matmul(y_ps[:], bc1_r, cntx_r,
                                 start=False, stop=True,
                                 skip_group_check=True)
                y_s = wpool.tile([H4, G_PER], F32, tag="ys")
                nc.vector.tensor_scalar_max(y_s[:], y_ps[:], 0.0)
                o_ps = ps_pl.tile([G_PER, C], F32, tag="ops")
                nc.tensor.matmul(o_ps[:], y_s[:], wc2_s,
                                 start=True, stop=False,
                                 skip_group_check=True)
                nc.tensor.matmul(o_ps[:], cntx_r, bc2_r,
                                 start=False, stop=True,
                                 skip_group_check=True)
                o_s = wpool.tile([G_PER, C], F32, tag="os")
                nc.scalar.activation(o_s[:], o_ps[:],
                                     mybir.ActivationFunctionType.Copy,
                                     scale=invc_c)
                nc.sync.dma_start(out=out_d[:], in_=o_s[:])

    nc.compile()
    return nc


# ------------------------------------------------------------------ driver
_CACHE = {}


def _get_programs(meta):
    key = (tuple(meta["kt"]), meta["n_true"], meta["NT"])
    if key not in _CACHE:
        progs = [_build_stats_program(meta)]
        progs += [_build_layer_program(meta, lay) for lay in range(3)]
        _CACHE[key] = progs
    return _CACHE[key]


def run_gnn(runner=None, **inputs):
    F, H, H2, H4, C = 128, 128, 64, 32, 2
    x = np.asarray(inputs["x"], np.float32)
    n_true = x.shape[0]
    src = np.asarray(inputs["edge_index"][0], np.int64)
    dst = np.asarray(inputs["edge_index"][1], np.int64)
    batch = np.asarray(inputs["batch"], np.int64)

    meta = _plan(src, dst, batch, n_true)
    NT, SHARD, NPAD = meta["NT"], meta["SHARD"], meta["NPAD"]
    cores = _build_static(meta, src, dst, batch)
    order = meta["order"]
    progs = _get_programs(meta)

    def run(nc, in_maps):
        if runner is not None:
            return runner(nc, in_maps)
        return run_bass_kernel_spmd(
            nc, in_maps, core_ids=list(range(N_CORES))).results

    x_new = np.zeros((NPAD + 1, F), np.float32)
    x_new[:NPAD][order < n_true] = x[order[order < n_true]]

    # ---- stats launch (BN partials + L0 pre-agg of tiles 0..TPRE-1)
    xb = x_new[:NPAD].astype(NPFP8)
    l0_dups = [_dup_layout(x_new, cores[c]["slotsrc"], DUP_NP[0])
               for c in range(N_CORES)]
    stats_maps = []
    for c in range(N_CORES):
        idx = ((np.arange(NT) * N_CORES + c)[:, None] * 128
               + np.arange(128)[None, :])
        slab = xb[idx]
        slab = np.ascontiguousarray(slab.transpose(1, 0, 2)).reshape(
            128, NT * F)
        stats_maps.append({
            "x_sh": slab, "ident": np.eye(128, dtype=np.float32)})
    res = run(progs[0], stats_maps)
    parts = np.stack([np.asarray(res[c]["stat_part"])
                      for c in range(N_CORES)], axis=2)
    sx_parts = np.ascontiguousarray(parts[:, 0, :], dtype=np.float32)
    ex2_parts = np.ascontiguousarray(parts[:, 1, :], dtype=np.float32)

    W = [np.asarray(inputs["W1"], np.float32),
         np.asarray(inputs["W2"], np.float32),
         np.asarray(inputs["W3"], np.float32)]
    brows = [np.asarray(inputs["b1"], np.float32).reshape(1, H),
             np.asarray(inputs["b2"], np.float32).reshape(1, H),
             np.asarray(inputs["b3"], np.float32).reshape(1, H2)]

    h_new = x_new
    core_out = None
    for lay in range(3):
        maps = []
        for c in range(N_CORES):
            st = cores[c]
            if lay == 0:
                rp = np.concatenate([st["sig_row"], st["sh_row"],
                                     brows[0].ravel()])
            elif lay == 1:
                rp = np.concatenate([st["sig_row"], brows[1].ravel()])
            else:
                rp = np.zeros((128, 65), np.float64)
                rp[0:64, 0:64] = np.eye(64)
            if lay == 1:
                pan = st["pans"][1]
            elif lay == 2:
                pan = np.concatenate([st["gpan"].astype(NPFP8),
                                      st["pans"][2]], axis=1)
            else:
                pan = st["pans"][0]
            m = {"dup": l0_dups[c] if lay == 0 else
                 _dup_layout(h_new, st["slotsrc"], DUP_NP[lay]),
                 "pan": np.ascontiguousarray(pan),
                 "rowpack": (rp.astype(NPBF16).reshape(1, -1) if lay < 2
                             else np.ascontiguousarray(rp.astype(NPBF16)))}
            if lay == 1:
                m["wpack"] = np.ascontiguousarray(np.concatenate(
                    [(W[1] / 8.0).astype(NPBF16), W[2].astype(NPBF16)],
                    axis=1))
            if lay == 0:
                fp = np.zeros((128, 18 + H), np.float32)
                fp[:, 0:8] = sx_parts
                fp[:, 8:16] = ex2_parts
                fp[:, 16] = np.asarray(inputs["bn_gamma"], np.float32)
                fp[:, 17] = np.asarray(inputs["bn_beta"], np.float32)
                fp[:, 18:] = W[0]
                m["f32pack"] = fp
            if lay == 2:
                mp = np.zeros((64, 80), np.float32)
                mp[:, 0:H4] = np.asarray(inputs["Wc1"], np.float32)
                mp[0:H4, H4:H4 + C] = np.asarray(inputs["Wc2"], np.float32)
                mp[0, 34:66] = np.asarray(inputs["bc1"], np.float32)
                mp[0, 66:74] = st["cntx"] * 4.0
                mp[0, 74:76] = np.asarray(inputs["bc2"], np.float32)
                mp[0:G_PER, 76] = st["invc"] / 4.0
                mp[0:64, 78] = np.asarray(inputs["b3"], np.float32) * 4.0
                m["mpack"] = mp
            maps.append(m)
        res = run(progs[1 + lay], maps)
        if lay < 2:
            OW = 128 if lay == 0 else 64
            h_new = np.zeros((NPAD + 1, OW), np.float32)
            for c in range(N_CORES):
                ho = np.asarray(res[c]["h_out"])
                hoT = ho.reshape(OW, NT, 128).transpose(1, 2, 0)
                idx = ((np.arange(NT) * N_CORES + c)[:, None] * 128
                       + np.arange(128)[None, :])
                h_new[idx] = hoT
        else:
            core_out = [np.asarray(res[c]["out"]) for c in range(N_CORES)]

    out = np.zeros((G, C), np.float32)
    for c in range(N_CORES):
        for lg, g in enumerate(meta["core_graphs"][c]):
            out[g] = core_out[c][lg]
    return out


def kernel(**inputs):
    return run_gnn(**inputs)


# revision 34
# speedup vs baseline: 1.0029x; 1.0029x over previous
"""Trainium2 Bass kernel for AudioOnlyGNN (3-layer GCN + BatchNorm + mean-pool + MLP).

v3 — graph-partitioned static slot stream:

Nodes are assigned to cores by *graph* ownership (8 graphs per core,
balanced by node count), then degree-sorted within each core and laid out in
128-row tiles; tile t's slot budget k_t = max in-degree(+self) over that tile
across all cores, giving a static slot stream identical on every core.  For
each layer the host materialises the edge-source rows in slot order (a pure
gather) so the device reads large contiguous DMA blocks.

On device, a 128-slot block contributes to a [F, ncols] PSUM tile via one
matmul whose moving operand is a narrow "panel" (slot -> dst column weight
with the GCN normalisation baked in).  The aggregate is transformed
(W^T @ agg), bias/BN-shift added as rank-1 matmuls, ReLU'd, written back.
Layers 0/1 write h' = dinv*ReLU(...) so panels never depend on h.

Because every graph lives entirely on one core, the mean-pool and classifier
MLP complete locally inside the L2 launch (no cross-core reduction): launches
are [stats+pre-agg] [L0] [L1] [L2+pool+mlp].  Between launches the host only
reorders bytes (gather / transpose), never does arithmetic on activations.
"""

import sys

sys.path.insert(0, "/opt/trn_rl_repo")

import contextlib

import numpy as np
import ml_dtypes

import concourse.bacc as bacc
import concourse.bass as bass
import concourse.mybir as mybir
from concourse.tile import TileContext
from concourse.bass_utils import run_bass_kernel_spmd

BF16 = mybir.dt.bfloat16
F32 = mybir.dt.float32
FP8 = mybir.dt.float8e3  # e3m4

NPBF16 = ml_dtypes.bfloat16
NPFP8 = ml_dtypes.float8_e3m4

N_CORES = 8
BN_EPS = 1e-5
G = 64
G_PER = G // N_CORES   # graphs per core
TPRE = 0               # tiles of L0 pre-aggregated inside the stats launch

# dtype of the host-expanded per-slot source rows, per layer
DUP_DT = [FP8, FP8, FP8]
DUP_NP = [NPFP8, NPFP8, NPFP8]
OUT_DT = [FP8, FP8]
OUT_NP = [NPFP8, NPFP8]


def _chunk_list(n0, n1, lead, mid, tail=(4, 2, 1)):
    """Chunk [n0, n1) into sizes lead + [mid...] + tail (tapered ends)."""
    n = n1 - n0
    sizes = []
    for s in lead:
        if sum(sizes) + s > n:
            break
        sizes.append(s)
    tl = [s for s in tail if s < mid]
    while sum(sizes) + sum(tl) + mid <= n:
        sizes.append(mid)
    rem = n - sum(sizes) - sum(tl)
    while rem > 0:
        add = min(rem, mid)
        sizes.append(add)
        rem -= add
    sizes += tl
    sizes = [s for s in sizes if s > 0]
    # clip overflow
    while sum(sizes) > n:
        sizes[-1] -= sum(sizes) - n
        sizes = [s for s in sizes if s > 0]
    out = []
    t = n0
    for cs in sizes:
        out.append(list(range(t, t + cs)))
        t += cs
    assert t == n1, (sizes, n0, n1)
    return out


# ------------------------------------------------------------------ planning
def _plan(src, dst, batch, n_true):
    """Static (h-independent) structure: graph packing, renumbering, slots."""
    cnt_g = np.bincount(batch, minlength=G).astype(np.int64)
    g_order = np.argsort(-cnt_g, kind="stable")
    core_graphs = [[] for _ in range(N_CORES)]
    loads = np.zeros(N_CORES, np.int64)
    for g in g_order:
        cand = [i for i in range(N_CORES) if len(core_graphs[i]) < G_PER]
        i = min(cand, key=lambda i: loads[i])
        core_graphs[i].append(int(g))
        loads[i] += cnt_g[g]
    NT = max(49, int(-(-loads.max() // 128)))
    SHARD = NT * 128
    NPAD = N_CORES * SHARD

    graph_core = np.zeros(G, np.int64)
    graph_local = np.zeros(G, np.int64)
    for c in range(N_CORES):
        for lg, g in enumerate(core_graphs[c]):
            graph_core[g] = c
            graph_local[g] = lg

    degp_true = np.bincount(dst, minlength=n_true).astype(np.int64) + 1
    node_core = graph_core[batch]

    order = np.empty(NPAD, np.int64)
    virt = n_true
    for c in range(N_CORES):
        nodes_c = np.where(node_core == c)[0]
        nodes_c = nodes_c[np.argsort(degp_true[nodes_c], kind="stable")]
        npadc = SHARD - len(nodes_c)
        ids = np.concatenate([np.arange(virt, virt + npadc), nodes_c])
        virt += npadc
        idx = ((np.arange(NT) * N_CORES + c)[:, None] * 128
               + np.arange(128)[None, :])
        order[idx.ravel()] = ids
    assert virt == NPAD
    newpos = np.empty(NPAD, np.int64)
    newpos[order] = np.arange(NPAD)

    degp = np.zeros(NPAD, np.int64)
    degp[:n_true] = degp_true

    kt = np.zeros(NT, np.int64)
    for t in range(NT):
        kt[t] = degp[order[t * 1024:(t + 1) * 1024]].max()
    kt = np.maximum(kt, 1)

    blocks = []   # per tile: list of (lo, w)
    pan_cols = []  # per tile: list of panel col offsets
    wtot = 0
    for t in range(NT):
        k = int(kt[t])
        bl = []
        for b in range(k):
            lo = (128 * b) // k
            hi = (128 * (b + 1) - 1) // k
            bl.append((lo, hi - lo + 1))
        blocks.append(bl)
        offs = []
        for lo, w in bl:
            offs.append(wtot)
            wtot += w
        pan_cols.append(offs)

    nblk = int(kt.sum())
    tile_base = np.zeros(NT + 1, np.int64)
    tile_base[1:] = np.cumsum(128 * kt)
    meta = {"kt": kt, "blocks": blocks, "pan_cols": pan_cols,
            "wtot": wtot, "nblk": nblk, "order": order, "newpos": newpos,
            "n_true": n_true, "tile_base": tile_base,
            "total_slots": int(tile_base[-1]),
            "NT": NT, "SHARD": SHARD, "NPAD": NPAD,
            "core_graphs": core_graphs, "graph_core": graph_core,
            "graph_local": graph_local, "cnt_g": cnt_g}
    return meta


def _build_static(meta, src, dst, batch):
    """Per-core constant tables: slot->src map, per-layer panels, rows."""
    kt, blocks, pan_cols = meta["kt"], meta["blocks"], meta["pan_cols"]
    wtot, nblk, order, newpos = (meta["wtot"], meta["nblk"], meta["order"],
                                 meta["newpos"])
    n_true = meta["n_true"]
    NT, SHARD, NPAD = meta["NT"], meta["SHARD"], meta["NPAD"]
    graph_local, cnt_g = meta["graph_local"], meta["cnt_g"]

    deg = np.bincount(dst, minlength=NPAD).astype(np.float64) + 1.0
    dinv = (1.0 / np.sqrt(deg)).astype(np.float64)
    dinv_pad = dinv.copy()
    dinv_pad[n_true:] = 1.0

    dinv_new = dinv_pad[order]
    batch_pad = np.full(NPAD, 0, np.int64)
    batch_pad[:n_true] = batch
    batch_new = batch_pad[order]
    valid_new = (order < n_true)

    sneig = np.bincount(dst, weights=dinv[src], minlength=NPAD)
    d2 = dinv_pad * (sneig + dinv_pad)
    d2_new = d2[order]

    cntx = np.maximum(cnt_g.astype(np.float64), 1.0)   # [G]
    invc = 1.0 / cntx

    s_new = newpos[src]
    d_new = newpos[dst]
    g_tile = d_new // 128
    core_of = g_tile % N_CORES
    tloc = g_tile // N_CORES
    dloc = d_new % 128

    tile_base = meta["tile_base"]
    total_slots = meta["total_slots"]

    edge_w0 = dinv[src] * dinv_pad[dst] * dinv_pad[dst]

    cores = []
    for c in range(N_CORES):
        sel = core_of == c
        es, et, ed = s_new[sel], tloc[sel], dloc[sel]
        ew0 = edge_w0[sel]
        key = et * (128 * 64) + ed
        o = np.argsort(key, kind="stable")
        es, et, ed, ew0 = es[o], et[o], ed[o], ew0[o]
        k_of = kt[et]
        node_key = et * 128 + ed
        uniq, first_idx, counts = np.unique(node_key, return_index=True,
                                            return_counts=True)
        rank = np.arange(len(node_key)) - np.repeat(first_idx, counts)
        slot = tile_base[et] + ed * k_of + 1 + rank   # +1: self slot at 0

        tt = np.arange(NT).repeat(128)
        dd = np.tile(np.arange(128), NT)
        own_new = (tt * N_CORES + np.full(NT * 128, c)) * 128 + dd
        own_valid = valid_new[own_new]
        self_slot = tile_base[tt] + dd * kt[tt]

        slotsrc = np.full(total_slots, NPAD, np.int64)  # NPAD -> zero row
        slotsrc[slot] = es
        slotsrc[self_slot[own_valid]] = own_new[own_valid]

        dv_own = dinv_new[own_new]
        w_l0 = np.zeros(total_slots, np.float64)
        w_l0[slot] = ew0
        w_l0[self_slot[own_valid]] = (dv_own ** 3)[own_valid]
        col_dinv = np.repeat(dv_own, np.repeat(kt, 128))
        filled = np.zeros(total_slots, bool)
        filled[slot] = True
        filled[self_slot[own_valid]] = True
        w_l1 = np.where(filled, col_dinv ** 2, 0.0)
        w_l2 = np.where(filled, col_dinv, 0.0)

        pans = []
        for wv, psc in ((w_l0, 8.0), (w_l1, 8.0), (w_l2, 4.0)):
            pan = np.zeros((128, wtot), np.float64)
            for t in range(NT):
                k = int(kt[t])
                for b, (lo, w) in enumerate(blocks[t]):
                    co = pan_cols[t][b]
                    sl0 = tile_base[t] + b * 128
                    ss = np.arange(sl0, sl0 + 128)
                    cc = (ss - tile_base[t]) // k - lo
                    ok = (cc >= 0) & (cc < w)
                    pan[np.arange(128)[ok], co + cc[ok]] = wv[ss][ok]
            pans.append((pan * psc).astype(NPFP8))

        sig_row = np.zeros(SHARD, np.float64)
        sh_row = np.zeros(SHARD, np.float64)
        for t in range(NT):
            cols = slice(t * 128, (t + 1) * 128)
            nn = (t * N_CORES + c) * 128 + np.arange(128)
            sig_row[cols] = dinv_new[nn]
            sh_row[cols] = d2_new[nn] * dinv_new[nn]

        # pool panel [128, NT*G_PER]: 1.0 at (d, t*G_PER + local_graph)
        gpan = np.zeros((128, NT * G_PER), np.float64)
        for t in range(NT):
            nn = (t * N_CORES + c) * 128 + np.arange(128)
            gb = graph_local[batch_new[nn]]
            ok = valid_new[nn]
            gpan[np.arange(128)[ok], t * G_PER + gb[ok]] = 1.0

        cg = meta["core_graphs"][c]
        cores.append({
            "slotsrc": slotsrc,
            "pans": pans,
            "sig_row": sig_row,
            "sh_row": sh_row,
            "gpan": gpan.astype(NPBF16),
            "cntx": cntx[cg].astype(np.float32),     # [G_PER]
            "invc": invc[cg].astype(np.float32),     # [G_PER]
        })
    return cores


def _dup_layout(h_new, slotsrc, np_dt):
    """[NPAD(+1), F] new-indexed rows -> [128, NBLK*F] slot-stream layout."""
    rows = h_new[slotsrc]
    nblk = rows.shape[0] // 128
    F = rows.shape[1]
    return np.ascontiguousarray(
        rows.reshape(nblk, 128, F).transpose(1, 0, 2)
    ).reshape(128, nblk * F).astype(np_dt)


# ------------------------------------------------------------------ programs
def _build_stats_program(meta):
    """Per-core BN partial sums (Sum x, Sum x^2 over own nodes)."""
    F = 128
    NT = meta["NT"]
    nc = bacc.Bacc("TRN2", target_bir_lowering=False, debug=False,
                   num_devices=N_CORES)
    xs_d = nc.dram_tensor("x_sh", [128, NT * F], FP8,
                          kind="ExternalInput").ap()
    ident_d = nc.dram_tensor("ident", [128, 128], F32,
                             kind="ExternalInput").ap()
    out_d = nc.dram_tensor("stat_part", [128, 2], F32,
                           kind="ExternalOutput").ap()
    XS = [0, 10, 22, 35, NT]
    with TileContext(nc) as tc:
        with tc.tile_pool(name="w", bufs=1) as wp, \
             tc.tile_pool(name="ps", bufs=1, space="PSUM") as pp:
            xs = wp.tile([128, NT * F], FP8, tag="xs")
            ident_s = wp.tile([128, 128], F32, tag="id")
            nc.sync.dma_start(out=xs[:, :XS[1] * F], in_=xs_d[:, :XS[1] * F])
            nc.scalar.dma_start(out=ident_s[:], in_=ident_d[:])
            for q in range(1, 4):
                nc.sync.dma_start(out=xs[:, XS[q] * F:XS[q + 1] * F],
                                  in_=xs_d[:, XS[q] * F:XS[q + 1] * F])
            ones_s = wp.tile([128, 1], FP8, tag="ones")
            nc.vector.memset(ones_s[:], 1.0)
            xtx_ps = pp.tile([128, 128], F32, tag="xtx")
            sx_ps = pp.tile([128, 1], F32, tag="sx")
            for t in range(NT):
                sl = xs[:, t * F:(t + 1) * F]
                nc.tensor.matmul(xtx_ps[:], sl, sl, start=(t == 0),
                                 stop=(t == NT - 1), skip_group_check=True)
                nc.tensor.matmul(sx_ps[:], sl, ones_s[:],
                                 start=(t == 0), stop=(t == NT - 1),
                                 skip_group_check=True)
            dg = wp.tile([128, 128], F32, tag="dg")
            nc.vector.tensor_tensor(dg[:], xtx_ps[:], ident_s[:],
                                    mybir.AluOpType.mult)
            o = wp.tile([128, 2], F32, tag="o")
            nc.vector.tensor_reduce(o[:, 1:2], dg[:], mybir.AxisListType.X,
                                    mybir.AluOpType.add)
            nc.vector.tensor_copy(o[:, 0:1], sx_ps[:])
            nc.scalar.dma_start(out=out_d[:], in_=o[:])
    nc.compile()
    return nc


def _build_layer_program(meta, lay):
    kt, blocks, pan_cols, wtot, nblk, tile_base = (
        meta["kt"], meta["blocks"], meta["pan_cols"], meta["wtot"],
        meta["nblk"], meta["tile_base"])
    NT, SHARD = meta["NT"], meta["SHARD"]
    F = 128 if lay < 2 else 64
    H = 128
    H2 = 64
    H4 = 32
    C = 2
    Ho = H if lay < 2 else H2
    N_true = meta["n_true"]
    dt_in = DUP_DT[lay]
    dt_out = OUT_DT[lay] if lay < 2 else None

    nc = bacc.Bacc("TRN2", target_bir_lowering=False, debug=False,
                   num_devices=N_CORES)

    def din(name, shape, dt):
        return nc.dram_tensor(name, list(shape), dt, kind="ExternalInput").ap()

    dup_d = din("dup", [128, nblk * F], dt_in)
    if lay == 2:
        PW_EXTRA = NT * G_PER       # gpan (0/1: fp8-exact)
    else:
        PW_EXTRA = 0               # W1 in f32pack; W2|W3 in wpack
    pan_d = din("pan", [128, wtot + PW_EXTRA], FP8)
    if lay == 1:
        wp_d = din("wpack", [128, H + H2], BF16)
    # packed bf16 row constants
    if lay == 0:
        RP = 2 * SHARD + H        # sig | sh | b1
    elif lay == 1:
        RP = SHARD + H            # sig | b2
    else:
        RP = 1                    # b3 as a column
    rp_d = din("rowpack", [1, RP] if lay < 2 else [128, 65], BF16)
    if lay == 0:
        # sxp | exp | gamma | beta | W1(fp32)
        fp_d = din("f32pack", [128, 18 + H], F32)
        if TPRE:
            agp_d = din("aggT_pre", [128, TPRE * 128], BF16)
    if lay == 2:
        # mlp pack: Wc1 | Wc2 | bc1row | cntx | bc2 | invc  (f32)
        mp_d = din("mpack", [64, 80], F32)
        out_d = nc.dram_tensor("out", [G_PER, C], F32,
                               kind="ExternalOutput").ap()
    else:
        OW = 128 if lay == 0 else 64
        h_out = nc.dram_tensor("h_out", [OW, NT * 128], dt_out,
                               kind="ExternalOutput").ap()

    # process tiles high->low: degree sorting puts fat tiles at high
    # indices, so the tail (last chunk + final write) covers thin tiles.
    T0 = TPRE if lay == 0 else 0
    fwd = _chunk_list(T0, NT, [2, 2, 4], 8, tail=(4, 2))
    chunk_tiles = []
    hi = NT
    for ch in fwd:
        chunk_tiles.append(list(range(hi - len(ch), hi)))
        hi -= len(ch)
    assert hi == T0
    PBASE = pan_cols[TPRE][0] if lay == 0 else 0

    with TileContext(nc) as tc:
        with contextlib.ExitStack() as ctx:
            cpool = ctx.enter_context(tc.tile_pool(name="const", bufs=1))
            dpool = ctx.enter_context(tc.tile_pool(name="dup", bufs=5))
            ppool = ctx.enter_context(tc.tile_pool(name="pan", bufs=2))

            def chunk_loads(tiles):
                ct0, ct1 = tiles[0], tiles[-1] + 1
                b0 = int(tile_base[ct0] // 128)
                b1 = int(tile_base[ct1] // 128)
                dup_sb = dpool.tile([128, (b1 - b0) * F], dt_in, tag="dup")
                nc.sync.dma_start(out=dup_sb[:], in_=dup_d[:, b0 * F:b1 * F])
                return dup_sb, b0

            pend = [chunk_loads(chunk_tiles[0])]
            pan_sb = ppool.tile([128, wtot - PBASE + PW_EXTRA], FP8,
                                tag="pan")
            if lay == 1:
                wpk_s = cpool.tile([128, H + H2], BF16, tag="c_wpk")
                nc.scalar.dma_start(out=wpk_s[:], in_=wp_d[:])
            fst = NT - 12
            PAN_OFF = PW_EXTRA
            PSPLIT = PAN_OFF + pan_cols[fst][0] - PBASE
            nc.sync.dma_start(out=pan_sb[:, PSPLIT:],
                              in_=pan_d[:, PBASE + PSPLIT:])
            if PW_EXTRA:
                nc.scalar.dma_start(out=pan_sb[:, :PW_EXTRA],
                                    in_=pan_d[:, PBASE:PBASE + PW_EXTRA])

            rp_s = cpool.tile([1, RP] if lay < 2 else [128, 65], BF16,
                              tag="c_rp")
            (nc.scalar if lay == 0 else nc.sync).dma_start(
                out=rp_s[:], in_=rp_d[:])
            if lay == 0:
                fp_s = cpool.tile([128, 18 + H], F32, tag="c_fp")
                nc.scalar.dma_start(out=fp_s[:], in_=fp_d[:])
                if TPRE:
                    agp_s = cpool.tile([128, TPRE * 128], BF16, tag="c_agp")
                    nc.scalar.dma_start(out=agp_s[:], in_=agp_d[:])
            if lay == 2:
                mp_s = cpool.tile([64, 80], F32, tag="c_mp")
                nc.scalar.dma_start(out=mp_s[:], in_=mp_d[:])
            nc.sync.dma_start(out=pan_sb[:, PAN_OFF:PSPLIT],
                              in_=pan_d[:, PBASE + PAN_OFF:PBASE + PSPLIT])
            if lay == 0:
                sig_s = rp_s[0:1, 0:SHARD]
                sh_s = rp_s[0:1, SHARD:2 * SHARD]
                b_s = rp_s[0:1, 2 * SHARD:2 * SHARD + H]
            elif lay == 1:
                sig_s = rp_s[0:1, 0:SHARD]
                b_s = rp_s[0:1, SHARD:SHARD + H]
            else:
                b_s = mp_s[0:H2, 78:79]   # [H2, 1] f32 column
            zr_s = cpool.tile([1, 512], BF16, tag="c_zr")
            nc.vector.memset(zr_s[:], 0.0)
            if lay == 0:
                w1f_s = fp_s[:, 18:18 + H]
                w_s = cpool.tile([128, H], BF16, tag="c_wt")
                rw_s = cpool.tile([1, H], BF16, tag="c_rw")
            elif lay == 1:
                w_s = wpk_s[:, 0:H]
                w3_s = wpk_s[:, H:H + H2]
            else:
                gpan_s = pan_sb[:, 0:NT * G_PER]
                id_s = rp_s[:, 0:64]
                wc1_s = mp_s[:, 0:H4]                  # [64, 32]
                wc2_s = mp_s[0:H4, H4:H4 + C]          # [32, 2]
                bc1_r = mp_s[0:1, 34:66]               # [1, 32]
                cntx_r = mp_s[0:1, 66:74]              # [1, 8]
                bc2_r = mp_s[0:1, 74:76]               # [1, 2]
                invc_c = mp_s[0:G_PER, 76:77]          # [8, 1]

            # ---- BN statistics (layer 0) -> W~1 and shift row rw
            if lay == 0:
                with tc.tile_pool(name="ps_st", bufs=1, space="PSUM") as pst, \
                     tc.tile_pool(name="st_w", bufs=2) as stw:
                    sxp_s = fp_s[:, 0:8]
                    exp_s = fp_s[:, 8:16]
                    gam_s = fp_s[:, 16:17]
                    bet_s = fp_s[:, 17:18]
                    ex2 = stw.tile([128, 1], F32, tag="v1")
                    nc.vector.tensor_reduce(ex2[:], exp_s,
                                            mybir.AxisListType.X,
                                            mybir.AluOpType.add)
                    sx = stw.tile([128, 1], F32, tag="v0")
                    nc.vector.tensor_reduce(sx[:], sxp_s,
                                            mybir.AxisListType.X,
                                            mybir.AluOpType.add)
                    mu = stw.tile([128, 1], F32, tag="v2")
                    nc.vector.tensor_scalar_mul(mu[:], sx[:], 1.0 / N_true)
                    var = stw.tile([128, 1], F32, tag="v3")
                    nc.vector.tensor_scalar_mul(var[:], ex2[:], 1.0 / N_true)
                    mu2 = stw.tile([128, 1], F32, tag="v4")
                    nc.vector.tensor_tensor(mu2[:], mu[:], mu[:],
                                            mybir.AluOpType.mult)
                    nc.vector.tensor_tensor(var[:], var[:], mu2[:],
                                            mybir.AluOpType.subtract)
                    nc.vector.tensor_scalar_add(var[:], var[:], BN_EPS)
                    rec = stw.tile([128, 1], F32, tag="v5")
                    nc.vector.reciprocal(rec[:], var[:])
                    isd = stw.tile([128, 1], F32, tag="v6")
                    nc.scalar.activation(isd[:], rec[:],
                                         mybir.ActivationFunctionType.Sqrt)
                    a_c = stw.tile([128, 1], F32, tag="v7")
                    nc.vector.tensor_tensor(a_c[:], gam_s, isd[:],
                                            mybir.AluOpType.mult)
                    a8 = stw.tile([128, 1], F32, tag="v9")
                    nc.vector.tensor_scalar_mul(a8[:], a_c[:], 0.125)
                    nc.vector.tensor_scalar_mul(w_s[:], w1f_s, a8[:])
                    ca = stw.tile([128, 1], F32, tag="v8")
                    nc.vector.tensor_tensor(ca[:], mu[:], a_c[:],
                                            mybir.AluOpType.mult)
                    nc.vector.tensor_tensor(ca[:], bet_s, ca[:],
                                            mybir.AluOpType.subtract)
                    rw_ps = pst.tile([1, H], F32, tag="rw")
                    nc.tensor.matmul(rw_ps[:], ca[:], w1f_s,
                                     start=True, stop=True)
                    nc.scalar.activation(rw_s[:], rw_ps[:],
                                         mybir.ActivationFunctionType.Copy)

            spool = ctx.enter_context(tc.tile_pool(name="stg", bufs=1))
            wpool = ctx.enter_context(tc.tile_pool(name="wk", bufs=4))
            ps_agg = ctx.enter_context(
                tc.tile_pool(name="ps_agg", bufs=3, space="PSUM"))
            if lay < 2:
                ps_out = ctx.enter_context(
                    tc.tile_pool(name="ps_out", bufs=3, space="PSUM"))
            if lay == 1:
                ps_t = ctx.enter_context(
                    tc.tile_pool(name="ps_t", bufs=2, space="PSUM"))
                tstage = spool.tile([64, NT * 128], dt_out, tag="tstg")
            if lay == 2:
                ps_tr = ctx.enter_context(
                    tc.tile_pool(name="ps_tr", bufs=2, space="PSUM"))
                ps_pl = ctx.enter_context(
                    tc.tile_pool(name="ps_pl", bufs=1, space="PSUM"))
                pool_ps = ps_pl.tile([H2, G_PER], F32, tag="pool")

            if lay == 0:
                stage = spool.tile([128, NT * 128], dt_out, tag="stg")
            elif lay == 1:
                stage = spool.tile([128, NT * 128], BF16, tag="stg")

            state = {"use_dve": False, "rot": 0}

            def flip():
                state["use_dve"] = not state["use_dve"]
                return state["use_dve"]

            def rot():
                state["rot"] = (state["rot"] + 1) % 3
                return state["rot"]

            def split_copy(dst, src_ps, w):
                """PSUM->SBUF copy split across Act | DVE halves."""
                h = (w // 2 + 63) & ~63 if w > 128 else w
                nc.scalar.activation(dst[:, 0:h], src_ps[:, 0:h],
                                     mybir.ActivationFunctionType.Copy)
                if h < w:
                    nc.vector.tensor_copy(dst[:, h:w], src_ps[:, h:w])

            def split_relu(dst, src_ps, w):
                h = (w // 2 + 63) & ~63 if w > 128 else w
                nc.scalar.activation(dst[:, 0:h], src_ps[:, 0:h],
                                     mybir.ActivationFunctionType.Relu)
                if h < w:
                    nc.vector.tensor_scalar_max(dst[:, h:w],
                                                src_ps[:, h:w], 0.0)

            def split_relu_bias(dst, src_ps, w, bias):
                h = (w // 2 + 63) & ~63 if w > 128 else w
                nc.scalar.activation(dst[:, 0:h], src_ps[:, 0:h],
                                     mybir.ActivationFunctionType.Relu,
                                     bias=bias)
                if h < w:
                    nc.vector.tensor_scalar(dst[:, h:w], src_ps[:, h:w],
                                            bias, 0.0,
                                            mybir.AluOpType.add,
                                            mybir.AluOpType.max)

            def phase1(pr, dup_sb, b0):
                """agg matmuls (+ L2: bias + relu straight from PSUM)."""
                if dup_sb is None:   # lay0 tiles pre-aggregated in stats
                    return pr, None, agp_s[:, pr[0] * 128:(pr[-1] + 1) * 128]
                pw = len(pr) * 128
                rows = H2 if lay == 2 else 128
                agg_ps = ps_agg.tile([rows, pw], F32, tag="agg")
                nc.tensor.matmul(agg_ps[:], zr_s[0:1, 0:rows],
                                 zr_s[0:1, 0:pw], start=True, stop=False,
                                 skip_group_check=True)
                nb_pair = sum(int(kt[t]) for t in pr)
                bi = 0
                for hi, t in enumerate(pr):
                    for b, (lo, w) in enumerate(blocks[t]):
                        gb = int(tile_base[t] // 128) + b
                        co = pan_cols[t][b]
                        bi += 1
                        nc.tensor.matmul(
                            agg_ps[:, hi * 128 + lo:hi * 128 + lo + w],
                            dup_sb[:, (gb - b0) * F:(gb - b0 + 1) * F],
                            pan_sb[:, PAN_OFF + co - PBASE:
                                   PAN_OFF + co - PBASE + w],
                            start=False, stop=(bi == nb_pair),
                            skip_group_check=True)
                if lay == 2:
                    hsT = wpool.tile([H2, pw], BF16, tag="hsT")
                    if flip():
                        nc.vector.tensor_scalar(
                            hsT[:], agg_ps[:], b_s, 0.0,
                            mybir.AluOpType.add, mybir.AluOpType.max)
                    else:
                        nc.scalar.activation(
                            hsT[:], agg_ps[:],
                            mybir.ActivationFunctionType.Relu,
                            bias=b_s)
                    return pr, agg_ps, hsT
                aggT = wpool.tile([128, pw], BF16, tag="aggT")
                if flip():
                    nc.vector.tensor_copy(aggT[:], agg_ps[:])
                else:
                    nc.scalar.activation(aggT[:], agg_ps[:],
                                         mybir.ActivationFunctionType.Copy)
                return pr, agg_ps, aggT

            def phase2(st1):
                pr, agg_ps, aggT = st1
                pw = len(pr) * 128
                if lay < 2:
                    h_ps = ps_out.tile([Ho, pw], F32, tag="hps")
                    c0 = pr[0] * 128
                    nc.tensor.matmul(h_ps[:], w_s[:] if lay == 0 else w_s,
                                     aggT[:], start=True, stop=False,
                                     skip_group_check=True)
                    nc.tensor.matmul(h_ps[:], b_s,
                                     sig_s[0:1, c0:c0 + pw],
                                     start=False, stop=(lay != 0),
                                     skip_group_check=True)
                    if lay == 0:
                        nc.tensor.matmul(h_ps[:], rw_s[:],
                                         sh_s[0:1, c0:c0 + pw],
                                         start=False, stop=True,
                                         skip_group_check=True)
                    so = pr[0] * 128
                    if flip():
                        nc.vector.tensor_scalar_max(
                            stage[:, so:so + pw], h_ps[:], 0.0)
                    else:
                        nc.scalar.activation(
                            stage[:, so:so + pw], h_ps[:],
                            mybir.ActivationFunctionType.Relu)
                    return st1
                # lay 2: transpose each tile's hsT: [64, 128] -> [128, 64]
                hsT = aggT
                tr_ps = ps_tr.tile([128, len(pr) * H2], BF16, tag="tr")
                for hi, t in enumerate(pr):
                    nc.tensor.transpose(tr_ps[:, hi * H2:(hi + 1) * H2],
                                        hsT[:, hi * 128:(hi + 1) * 128],
                                        id_s[0:64, 0:64])
                hs_sb = wpool.tile([128, len(pr) * H2], BF16, tag="hs")
                if flip():
                    nc.vector.tensor_copy(hs_sb[:], tr_ps[:])
                else:
                    nc.scalar.activation(
                        hs_sb[:], tr_ps[:],
                        mybir.ActivationFunctionType.Copy)
                return [(t, hs_sb, hi * H2) for hi, t in enumerate(pr)]

            def phase3(st2):
                if lay == 2:
                    flip()          # odd flips/group: engines alternate
                if lay == 1:
                    pr = st2[0]
                    pw = len(pr) * 128
                    so = pr[0] * 128
                    t_ps = ps_t.tile([H2, pw], F32, tag="tps")
                    nc.tensor.matmul(t_ps[:], w3_s, stage[:, so:so + pw],
                                     start=True, stop=True,
                                     skip_group_check=True)
                    if flip():
                        nc.vector.tensor_copy(tstage[:, so:so + pw], t_ps[:])
                    else:
                        nc.scalar.activation(
                            tstage[:, so:so + pw], t_ps[:],
                            mybir.ActivationFunctionType.Copy)
                elif lay == 2:
                    for t, hs_sb, off in st2:
                        state["npool"] = state.get("npool", 0) + 1
                        nc.tensor.matmul(pool_ps[:],
                                         hs_sb[:, off:off + H2],
                                         gpan_s[:, t * G_PER:(t + 1) * G_PER],
                                         start=(state["npool"] == 1),
                                         stop=(state["npool"] == NT),
                                         skip_group_check=True)

            all_pairs = []
            for ci, tiles in enumerate(chunk_tiles):
                dup_sb, b0 = pend.pop(0)
                if ci + 1 < len(chunk_tiles):
                    pend.append(chunk_loads(chunk_tiles[ci + 1]))
                GW = 4
                grps = [tiles[i:i + GW] for i in range(0, len(tiles), GW)]
                for g in reversed(grps):
                    all_pairs.append((g, dup_sb, b0))
            if lay == 0 and TPRE:
                # pre-aggregated thin tiles: compute-only, processed last
                pg = [list(range(i, min(i + 4, TPRE)))
                      for i in range(0, TPRE, 4)]
                for g in reversed(pg):
                    all_pairs.append((g, None, None))

            hastail = lay > 0
            q2, q3 = [], []
            out_stage = stage if lay == 0 else (tstage if lay == 1 else None)
            OW = 128 if lay == 0 else 64
            wb = [NT, 24, 12, 6, 2, 0]
            WRITES = [(wb[i + 1], wb[i]) for i in range(len(wb) - 1)]

            def maybe_write(done_min):
                if lay == 2:
                    return
                while WRITES and done_min <= WRITES[0][0]:
                    wt0, wt1 = WRITES.pop(0)
                    q = nc.sync if wt0 == 0 else nc.gpsimd
                    q.dma_start(
                        out=h_out[:, wt0 * 128:wt1 * 128],
                        in_=out_stage[:, wt0 * 128:wt1 * 128])

            def run3():
                st3 = q3.pop(0)
                phase3(st3)
                done = st3[0][0] if lay == 1 else st3[0][0]
                maybe_write(done)

            def run2():
                st2 = phase2(q2.pop(0))
                if hastail:
                    q3.append(st2)
                else:
                    maybe_write(st2[0][0])

            LAG2 = 1 if lay == 1 else 2
            LAG3 = 3 if lay == 2 else 2
            for item in all_pairs:
                st1 = phase1(*item)
                if len(q3) >= LAG3:
                    run3()
                if len(q2) >= LAG2:
                    run2()
                q2.append(st1)
            while q2 or q3:
                if q3:
                    run3()
                if q2:
                    run2()

            # ---- classifier MLP on this core's G_PER graphs (lay 2)
            if lay == 2:
                p01 = wpool.tile([H2, G_PER], F32, tag="p01")
                nc.vector.tensor_copy(p01[:], pool_ps[:])
                y_ps = ps_pl.tile([H4, G_PER], F32, tag="yps")
                nc.tensor.matmul(y_ps[:], wc1_s, p01[:],
                                 start=True, stop=False,
                                 skip_group_check=True)
                nc.tensor.matmul(y_ps[:], bc1_r, cntx_r,
                                 start=False, stop=True,
                                 skip_group_check=True)
                y_s = wpool.tile([H4, G_PER], F32, tag="ys")
                nc.vector.tensor_scalar_max(y_s[:], y_ps[:], 0.0)
                o_ps = ps_pl.tile([G_PER, C], F32, tag="ops")
                nc.tensor.matmul(o_ps[:], y_s[:], wc2_s,
                                 start=True, stop=False,
                                 skip_group_check=True)
                nc.tensor.matmul(o_ps[:], cntx_r, bc2_r,
                                 start=False, stop=True,
                                 skip_group_check=True)
                o_s = wpool.tile([G_PER, C], F32, tag="os")
                nc.scalar.activation(o_s[:], o_ps[:],
                                     mybir.ActivationFunctionType.Copy,
                                     scale=invc_c)
                nc.sync.dma_start(out=out_d[:], in_=o_s[:])

    nc.compile()
    return nc


# ------------------------------------------------------------------ driver
_CACHE = {}


def _get_programs(meta):
    key = (tuple(meta["kt"]), meta["n_true"], meta["NT"])
    if key not in _CACHE:
        progs = [_build_stats_program(meta)]
        progs += [_build_layer_program(meta, lay) for lay in range(3)]
        _CACHE[key] = progs
    return _CACHE[key]


def run_gnn(runner=None, **inputs):
    F, H, H2, H4, C = 128, 128, 64, 32, 2
    x = np.asarray(inputs["x"], np.float32)
    n_true = x.shape[0]
    src = np.asarray(inputs["edge_index"][0], np.int64)
    dst = np.asarray(inputs["edge_index"][1], np.int64)
    batch = np.asarray(inputs["batch"], np.int64)

    meta = _plan(src, dst, batch, n_true)
    NT, SHARD, NPAD = meta["NT"], meta["SHARD"], meta["NPAD"]
    cores = _build_static(meta, src, dst, batch)
    order = meta["order"]
    progs = _get_programs(meta)

    def run(nc, in_maps):
        if runner is not None:
            return runner(nc, in_maps)
        return run_bass_kernel_spmd(
            nc, in_maps, core_ids=list(range(N_CORES))).results

    x_new = np.zeros((NPAD + 1, F), np.float32)
    x_new[:NPAD][order < n_true] = x[order[order < n_true]]

    # ---- stats launch (BN partials + L0 pre-agg of tiles 0..TPRE-1)
    xb = x_new[:NPAD].astype(NPFP8)
    l0_dups = [_dup_layout(x_new, cores[c]["slotsrc"], DUP_NP[0])
               for c in range(N_CORES)]
    stats_maps = []
    for c in range(N_CORES):
        idx = ((np.arange(NT) * N_CORES + c)[:, None] * 128
               + np.arange(128)[None, :])
        slab = xb[idx]
        slab = np.ascontiguousarray(slab.transpose(1, 0, 2)).reshape(
            128, NT * F)
        stats_maps.append({
            "x_sh": slab, "ident": np.eye(128, dtype=np.float32)})
    res = run(progs[0], stats_maps)
    parts = np.stack([np.asarray(res[c]["stat_part"])
                      for c in range(N_CORES)], axis=2)
    sx_parts = np.ascontiguousarray(parts[:, 0, :], dtype=np.float32)
    ex2_parts = np.ascontiguousarray(parts[:, 1, :], dtype=np.float32)

    W = [np.asarray(inputs["W1"], np.float32),
         np.asarray(inputs["W2"], np.float32),
         np.asarray(inputs["W3"], np.float32)]
    brows = [np.asarray(inputs["b1"], np.float32).reshape(1, H),
             np.asarray(inputs["b2"], np.float32).reshape(1, H),
             np.asarray(inputs["b3"], np.float32).reshape(1, H2)]

    h_new = x_new
    core_out = None
    for lay in range(3):
        maps = []
        for c in range(N_CORES):
            st = cores[c]
            if lay == 0:
                rp = np.concatenate([st["sig_row"], st["sh_row"],
                                     brows[0].ravel()])
            elif lay == 1:
                rp = np.concatenate([st["sig_row"], brows[1].ravel()])
            else:
                rp = np.zeros((128, 65), np.float64)
                rp[0:64, 0:64] = np.eye(64)
            if lay == 1:
                pan = st["pans"][1]
            elif lay == 2:
                pan = np.concatenate([st["gpan"].astype(NPFP8),
                                      st["pans"][2]], axis=1)
            else:
                pan = st["pans"][0]
            m = {"dup": l0_dups[c] if lay == 0 else
                 _dup_layout(h_new, st["slotsrc"], DUP_NP[lay]),
                 "pan": np.ascontiguousarray(pan),
                 "rowpack": (rp.astype(NPBF16).reshape(1, -1) if lay < 2
                             else np.ascontiguousarray(rp.astype(NPBF16)))}
            if lay == 1:
                m["wpack"] = np.ascontiguousarray(np.concatenate(
                    [(W[1] / 8.0).astype(NPBF16), W[2].astype(NPBF16)],
                    axis=1))
            if lay == 0:
                fp = np.zeros((128, 18 + H), np.float32)
                fp[:, 0:8] = sx_parts
                fp[:, 8:16] = ex2_parts
                fp[:, 16] = np.asarray(inputs["bn_gamma"], np.float32)
                fp[:, 17] = np.asarray(inputs["bn_beta"], np.float32)
                fp[:, 18:] = W[0]
                m["f32pack"] = fp
            if lay == 2:
                mp = np.zeros((64, 80), np.float32)
                mp[:, 0:H4] = np.asarray(inputs["Wc1"], np.float32)
                mp[0:H4, H4:H4 + C] = np.asarray(inputs["Wc2"], np.float32)
                mp[0, 34:66] = np.asarray(inputs["bc1"], np.float32)
                mp[0, 66:74] = st["cntx"] * 4.0
                mp[0, 74:76] = np.asarray(inputs["bc2"], np.float32)
                mp[0:G_PER, 76] = st["invc"] / 4.0
                mp[0:64, 78] = np.asarray(inputs["b3"], np.float32) * 4.0
                m["mpack"] = mp
            maps.append(m)
        res = run(progs[1 + lay], maps)
        if lay < 2:
            OW = 128 if lay == 0 else 64
            h_new = np.zeros((NPAD + 1, OW), np.float32)
            for c in range(N_CORES):
                ho = np.asarray(res[c]["h_out"])
                hoT = ho.reshape(OW, NT, 128).transpose(1, 2, 0)
                idx = ((np.arange(NT) * N_CORES + c)[:, None] * 128
                       + np.arange(128)[None, :])
                h_new[idx] = hoT
        else:
            core_out = [np.asarray(res[c]["out"]) for c in range(N_CORES)]

    out = np.zeros((G, C), np.float32)
    for c in range(N_CORES):
        for lg, g in enumerate(meta["core_graphs"][c]):
            out[g] = core_out[c][lg]
    return out


def kernel(**inputs):
    return run_gnn(**inputs)


# revision 35
# speedup vs baseline: 1.0075x; 1.0045x over previous
"""Trainium2 Bass kernel for AudioOnlyGNN (3-layer GCN + BatchNorm + mean-pool + MLP).

v3 — graph-partitioned static slot stream:

Nodes are assigned to cores by *graph* ownership (8 graphs per core,
balanced by node count), then degree-sorted within each core and laid out in
128-row tiles; tile t's slot budget k_t = max in-degree(+self) over that tile
across all cores, giving a static slot stream identical on every core.  For
each layer the host materialises the edge-source rows in slot order (a pure
gather) so the device reads large contiguous DMA blocks.

On device, a 128-slot block contributes to a [F, ncols] PSUM tile via one
matmul whose moving operand is a narrow "panel" (slot -> dst column weight
with the GCN normalisation baked in).  The aggregate is transformed
(W^T @ agg), bias/BN-shift added as rank-1 matmuls, ReLU'd, written back.
Layers 0/1 write h' = dinv*ReLU(...) so panels never depend on h.

Because every graph lives entirely on one core, the mean-pool and classifier
MLP complete locally inside the L2 launch (no cross-core reduction): launches
are [stats+pre-agg] [L0] [L1] [L2+pool+mlp].  Between launches the host only
reorders bytes (gather / transpose), never does arithmetic on activations.
"""

import sys

sys.path.insert(0, "/opt/trn_rl_repo")

import contextlib

import numpy as np
import ml_dtypes

import concourse.bacc as bacc
import concourse.bass as bass
import concourse.mybir as mybir
from concourse.tile import TileContext
from concourse.bass_utils import run_bass_kernel_spmd

BF16 = mybir.dt.bfloat16
F32 = mybir.dt.float32
FP8 = mybir.dt.float8e3  # e3m4

NPBF16 = ml_dtypes.bfloat16
NPFP8 = ml_dtypes.float8_e3m4

N_CORES = 8
BN_EPS = 1e-5
G = 64
G_PER = G // N_CORES   # graphs per core
TPRE = 0               # tiles of L0 pre-aggregated inside the stats launch

# dtype of the host-expanded per-slot source rows, per layer
DUP_DT = [FP8, FP8, FP8]
DUP_NP = [NPFP8, NPFP8, NPFP8]
OUT_DT = [FP8, FP8]
OUT_NP = [NPFP8, NPFP8]


def _chunk_list(n0, n1, lead, mid, tail=(4, 2, 1)):
    """Chunk [n0, n1) into sizes lead + [mid...] + tail (tapered ends)."""
    n = n1 - n0
    sizes = []
    for s in lead:
        if sum(sizes) + s > n:
            break
        sizes.append(s)
    tl = [s for s in tail if s < mid]
    while sum(sizes) + sum(tl) + mid <= n:
        sizes.append(mid)
    rem = n - sum(sizes) - sum(tl)
    while rem > 0:
        add = min(rem, mid)
        sizes.append(add)
        rem -= add
    sizes += tl
    sizes = [s for s in sizes if s > 0]
    # clip overflow
    while sum(sizes) > n:
        sizes[-1] -= sum(sizes) - n
        sizes = [s for s in sizes if s > 0]
    out = []
    t = n0
    for cs in sizes:
        out.append(list(range(t, t + cs)))
        t += cs
    assert t == n1, (sizes, n0, n1)
    return out


# ------------------------------------------------------------------ planning
def _plan(src, dst, batch, n_true):
    """Static (h-independent) structure: graph packing, renumbering, slots."""
    cnt_g = np.bincount(batch, minlength=G).astype(np.int64)
    g_order = np.argsort(-cnt_g, kind="stable")
    core_graphs = [[] for _ in range(N_CORES)]
    loads = np.zeros(N_CORES, np.int64)
    for g in g_order:
        cand = [i for i in range(N_CORES) if len(core_graphs[i]) < G_PER]
        i = min(cand, key=lambda i: loads[i])
        core_graphs[i].append(int(g))
        loads[i] += cnt_g[g]
    NT = max(49, int(-(-loads.max() // 128)))
    SHARD = NT * 128
    NPAD = N_CORES * SHARD

    graph_core = np.zeros(G, np.int64)
    graph_local = np.zeros(G, np.int64)
    for c in range(N_CORES):
        for lg, g in enumerate(core_graphs[c]):
            graph_core[g] = c
            graph_local[g] = lg

    degp_true = np.bincount(dst, minlength=n_true).astype(np.int64) + 1
    node_core = graph_core[batch]

    order = np.empty(NPAD, np.int64)
    virt = n_true
    for c in range(N_CORES):
        nodes_c = np.where(node_core == c)[0]
        nodes_c = nodes_c[np.argsort(degp_true[nodes_c], kind="stable")]
        npadc = SHARD - len(nodes_c)
        ids = np.concatenate([np.arange(virt, virt + npadc), nodes_c])
        virt += npadc
        idx = ((np.arange(NT) * N_CORES + c)[:, None] * 128
               + np.arange(128)[None, :])
        order[idx.ravel()] = ids
    assert virt == NPAD
    newpos = np.empty(NPAD, np.int64)
    newpos[order] = np.arange(NPAD)

    degp = np.zeros(NPAD, np.int64)
    degp[:n_true] = degp_true

    kt = np.zeros(NT, np.int64)
    for t in range(NT):
        kt[t] = degp[order[t * 1024:(t + 1) * 1024]].max()
    kt = np.maximum(kt, 1)

    blocks = []   # per tile: list of (lo, w)
    pan_cols = []  # per tile: list of panel col offsets
    wtot = 0
    for t in range(NT):
        k = int(kt[t])
        bl = []
        for b in range(k):
            lo = (128 * b) // k
            hi = (128 * (b + 1) - 1) // k
            bl.append((lo, hi - lo + 1))
        blocks.append(bl)
        offs = []
        for lo, w in bl:
            offs.append(wtot)
            wtot += w
        pan_cols.append(offs)

    nblk = int(kt.sum())
    tile_base = np.zeros(NT + 1, np.int64)
    tile_base[1:] = np.cumsum(128 * kt)
    meta = {"kt": kt, "blocks": blocks, "pan_cols": pan_cols,
            "wtot": wtot, "nblk": nblk, "order": order, "newpos": newpos,
            "n_true": n_true, "tile_base": tile_base,
            "total_slots": int(tile_base[-1]),
            "NT": NT, "SHARD": SHARD, "NPAD": NPAD,
            "core_graphs": core_graphs, "graph_core": graph_core,
            "graph_local": graph_local, "cnt_g": cnt_g}
    return meta


def _build_static(meta, src, dst, batch):
    """Per-core constant tables: slot->src map, per-layer panels, rows."""
    kt, blocks, pan_cols = meta["kt"], meta["blocks"], meta["pan_cols"]
    wtot, nblk, order, newpos = (meta["wtot"], meta["nblk"], meta["order"],
                                 meta["newpos"])
    n_true = meta["n_true"]
    NT, SHARD, NPAD = meta["NT"], meta["SHARD"], meta["NPAD"]
    graph_local, cnt_g = meta["graph_local"], meta["cnt_g"]

    deg = np.bincount(dst, minlength=NPAD).astype(np.float64) + 1.0
    dinv = (1.0 / np.sqrt(deg)).astype(np.float64)
    dinv_pad = dinv.copy()
    dinv_pad[n_true:] = 1.0

    dinv_new = dinv_pad[order]
    batch_pad = np.full(NPAD, 0, np.int64)
    batch_pad[:n_true] = batch
    batch_new = batch_pad[order]
    valid_new = (order < n_true)

    sneig = np.bincount(dst, weights=dinv[src], minlength=NPAD)
    d2 = dinv_pad * (sneig + dinv_pad)
    d2_new = d2[order]

    cntx = np.maximum(cnt_g.astype(np.float64), 1.0)   # [G]
    invc = 1.0 / cntx

    s_new = newpos[src]
    d_new = newpos[dst]
    g_tile = d_new // 128
    core_of = g_tile % N_CORES
    tloc = g_tile // N_CORES
    dloc = d_new % 128

    tile_base = meta["tile_base"]
    total_slots = meta["total_slots"]

    edge_w0 = dinv[src] * dinv_pad[dst] * dinv_pad[dst]

    cores = []
    for c in range(N_CORES):
        sel = core_of == c
        es, et, ed = s_new[sel], tloc[sel], dloc[sel]
        ew0 = edge_w0[sel]
        key = et * (128 * 64) + ed
        o = np.argsort(key, kind="stable")
        es, et, ed, ew0 = es[o], et[o], ed[o], ew0[o]
        k_of = kt[et]
        node_key = et * 128 + ed
        uniq, first_idx, counts = np.unique(node_key, return_index=True,
                                            return_counts=True)
        rank = np.arange(len(node_key)) - np.repeat(first_idx, counts)
        slot = tile_base[et] + ed * k_of + 1 + rank   # +1: self slot at 0

        tt = np.arange(NT).repeat(128)
        dd = np.tile(np.arange(128), NT)
        own_new = (tt * N_CORES + np.full(NT * 128, c)) * 128 + dd
        own_valid = valid_new[own_new]
        self_slot = tile_base[tt] + dd * kt[tt]

        slotsrc = np.full(total_slots, NPAD, np.int64)  # NPAD -> zero row
        slotsrc[slot] = es
        slotsrc[self_slot[own_valid]] = own_new[own_valid]

        dv_own = dinv_new[own_new]
        w_l0 = np.zeros(total_slots, np.float64)
        w_l0[slot] = ew0
        w_l0[self_slot[own_valid]] = (dv_own ** 3)[own_valid]
        col_dinv = np.repeat(dv_own, np.repeat(kt, 128))
        filled = np.zeros(total_slots, bool)
        filled[slot] = True
        filled[self_slot[own_valid]] = True
        w_l1 = np.where(filled, col_dinv ** 2, 0.0)
        w_l2 = np.where(filled, col_dinv, 0.0)

        pans = []
        for wv, psc in ((w_l0, 8.0), (w_l1, 8.0), (w_l2, 4.0)):
            pan = np.zeros((128, wtot), np.float64)
            for t in range(NT):
                k = int(kt[t])
                for b, (lo, w) in enumerate(blocks[t]):
                    co = pan_cols[t][b]
                    sl0 = tile_base[t] + b * 128
                    ss = np.arange(sl0, sl0 + 128)
                    cc = (ss - tile_base[t]) // k - lo
                    ok = (cc >= 0) & (cc < w)
                    pan[np.arange(128)[ok], co + cc[ok]] = wv[ss][ok]
            pans.append((pan * psc).astype(NPFP8))

        sig_row = np.zeros(SHARD, np.float64)
        sh_row = np.zeros(SHARD, np.float64)
        for t in range(NT):
            cols = slice(t * 128, (t + 1) * 128)
            nn = (t * N_CORES + c) * 128 + np.arange(128)
            sig_row[cols] = dinv_new[nn]
            sh_row[cols] = d2_new[nn] * dinv_new[nn]

        # pool panel [128, NT*G_PER]: 1.0 at (d, t*G_PER + local_graph)
        gpan = np.zeros((128, NT * G_PER), np.float64)
        for t in range(NT):
            nn = (t * N_CORES + c) * 128 + np.arange(128)
            gb = graph_local[batch_new[nn]]
            ok = valid_new[nn]
            gpan[np.arange(128)[ok], t * G_PER + gb[ok]] = 1.0

        cg = meta["core_graphs"][c]
        cores.append({
            "slotsrc": slotsrc,
            "pans": pans,
            "sig_row": sig_row,
            "sh_row": sh_row,
            "gpan": gpan.astype(NPBF16),
            "cntx": cntx[cg].astype(np.float32),     # [G_PER]
            "invc": invc[cg].astype(np.float32),     # [G_PER]
        })
    return cores


def _dup_layout(h_new, slotsrc, np_dt):
    """[NPAD(+1), F] new-indexed rows -> [128, NBLK*F] slot-stream layout."""
    rows = h_new[slotsrc]
    nblk = rows.shape[0] // 128
    F = rows.shape[1]
    return np.ascontiguousarray(
        rows.reshape(nblk, 128, F).transpose(1, 0, 2)
    ).reshape(128, nblk * F).astype(np_dt)


# ------------------------------------------------------------------ programs
def _build_stats_program(meta):
    """Per-core BN partial sums (Sum x, Sum x^2 over own nodes)."""
    F = 128
    NT = meta["NT"]
    nc = bacc.Bacc("TRN2", target_bir_lowering=False, debug=False,
                   num_devices=N_CORES)
    xs_d = nc.dram_tensor("x_sh", [128, NT * F], FP8,
                          kind="ExternalInput").ap()
    ident_d = nc.dram_tensor("ident", [128, 128], F32,
                             kind="ExternalInput").ap()
    out_d = nc.dram_tensor("stat_part", [128, 2], F32,
                           kind="ExternalOutput").ap()
    XS = [0, 10, 22, 35, NT]
    with TileContext(nc) as tc:
        with tc.tile_pool(name="w", bufs=1) as wp, \
             tc.tile_pool(name="ps", bufs=1, space="PSUM") as pp:
            xs = wp.tile([128, NT * F], FP8, tag="xs")
            ident_s = wp.tile([128, 128], F32, tag="id")
            nc.sync.dma_start(out=xs[:, :XS[1] * F], in_=xs_d[:, :XS[1] * F])
            nc.scalar.dma_start(out=ident_s[:], in_=ident_d[:])
            for q in range(1, 4):
                nc.sync.dma_start(out=xs[:, XS[q] * F:XS[q + 1] * F],
                                  in_=xs_d[:, XS[q] * F:XS[q + 1] * F])
            ones_s = wp.tile([128, 1], FP8, tag="ones")
            nc.vector.memset(ones_s[:], 1.0)
            xtx_ps = pp.tile([128, 128], F32, tag="xtx")
            sx_ps = pp.tile([128, 1], F32, tag="sx")
            for t in range(NT):
                sl = xs[:, t * F:(t + 1) * F]
                nc.tensor.matmul(xtx_ps[:], sl, sl, start=(t == 0),
                                 stop=(t == NT - 1), skip_group_check=True)
                nc.tensor.matmul(sx_ps[:], sl, ones_s[:],
                                 start=(t == 0), stop=(t == NT - 1),
                                 skip_group_check=True)
            dg = wp.tile([128, 128], F32, tag="dg")
            nc.vector.tensor_tensor(dg[:], xtx_ps[:], ident_s[:],
                                    mybir.AluOpType.mult)
            o = wp.tile([128, 2], F32, tag="o")
            nc.vector.tensor_reduce(o[:, 1:2], dg[:], mybir.AxisListType.X,
                                    mybir.AluOpType.add)
            nc.vector.tensor_copy(o[:, 0:1], sx_ps[:])
            nc.scalar.dma_start(out=out_d[:], in_=o[:])
    nc.compile()
    return nc


def _build_layer_program(meta, lay):
    kt, blocks, pan_cols, wtot, nblk, tile_base = (
        meta["kt"], meta["blocks"], meta["pan_cols"], meta["wtot"],
        meta["nblk"], meta["tile_base"])
    NT, SHARD = meta["NT"], meta["SHARD"]
    F = 128 if lay < 2 else 64
    H = 128
    H2 = 64
    H4 = 32
    C = 2
    Ho = H if lay < 2 else H2
    N_true = meta["n_true"]
    dt_in = DUP_DT[lay]
    dt_out = OUT_DT[lay] if lay < 2 else None

    nc = bacc.Bacc("TRN2", target_bir_lowering=False, debug=False,
                   num_devices=N_CORES)

    def din(name, shape, dt):
        return nc.dram_tensor(name, list(shape), dt, kind="ExternalInput").ap()

    dup_d = din("dup", [128, nblk * F], dt_in)
    if lay == 2:
        PW_EXTRA = NT * G_PER       # gpan (0/1: fp8-exact)
    else:
        PW_EXTRA = 0               # W1 in f32pack; W2|W3 in wpack
    pan_d = din("pan", [128, wtot + PW_EXTRA], FP8)
    if lay == 1:
        wp_d = din("wpack", [128, H + H2], BF16)
    # packed bf16 row constants
    if lay == 0:
        RP = 2 * SHARD + H        # sig | sh | b1
    elif lay == 1:
        RP = SHARD + H            # sig | b2
    else:
        RP = 1                    # b3 as a column
    rp_d = din("rowpack", [1, RP] if lay < 2 else [128, 65], BF16)
    if lay == 0:
        # sxp | exp | gamma | beta | W1(fp32)
        fp_d = din("f32pack", [128, 18 + H], F32)
        if TPRE:
            agp_d = din("aggT_pre", [128, TPRE * 128], BF16)
    if lay == 2:
        # mlp pack: Wc1 | Wc2 | bc1row | cntx | bc2 | invc  (f32)
        mp_d = din("mpack", [64, 80], F32)
        out_d = nc.dram_tensor("out", [G_PER, C], F32,
                               kind="ExternalOutput").ap()
    else:
        OW = 128 if lay == 0 else 64
        h_out = nc.dram_tensor("h_out", [OW, NT * 128], dt_out,
                               kind="ExternalOutput").ap()

    # process tiles high->low: degree sorting puts fat tiles at high
    # indices, so the tail (last chunk + final write) covers thin tiles.
    T0 = TPRE if lay == 0 else 0
    fwd = _chunk_list(T0, NT, [2, 2, 4], 8, tail=(4, 2))
    chunk_tiles = []
    hi = NT
    for ch in fwd:
        chunk_tiles.append(list(range(hi - len(ch), hi)))
        hi -= len(ch)
    assert hi == T0
    PBASE = pan_cols[TPRE][0] if lay == 0 else 0

    with TileContext(nc) as tc:
        with contextlib.ExitStack() as ctx:
            cpool = ctx.enter_context(tc.tile_pool(name="const", bufs=1))
            dpool = ctx.enter_context(tc.tile_pool(name="dup", bufs=5))
            ppool = ctx.enter_context(tc.tile_pool(name="pan", bufs=2))

            def chunk_loads(tiles):
                ct0, ct1 = tiles[0], tiles[-1] + 1
                b0 = int(tile_base[ct0] // 128)
                b1 = int(tile_base[ct1] // 128)
                dup_sb = dpool.tile([128, (b1 - b0) * F], dt_in, tag="dup")
                nc.sync.dma_start(out=dup_sb[:], in_=dup_d[:, b0 * F:b1 * F])
                return dup_sb, b0

            pend = [chunk_loads(chunk_tiles[0])]
            pan_sb = ppool.tile([128, wtot - PBASE + PW_EXTRA], FP8,
                                tag="pan")
            if lay == 1:
                wpk_s = cpool.tile([128, H + H2], BF16, tag="c_wpk")
                nc.scalar.dma_start(out=wpk_s[:], in_=wp_d[:])
            fst = NT - 12
            PAN_OFF = PW_EXTRA
            PSPLIT = PAN_OFF + pan_cols[fst][0] - PBASE
            nc.sync.dma_start(out=pan_sb[:, PSPLIT:],
                              in_=pan_d[:, PBASE + PSPLIT:])
            if PW_EXTRA:
                nc.scalar.dma_start(out=pan_sb[:, :PW_EXTRA],
                                    in_=pan_d[:, PBASE:PBASE + PW_EXTRA])

            rp_s = cpool.tile([1, RP] if lay < 2 else [128, 65], BF16,
                              tag="c_rp")
            (nc.scalar if lay == 0 else nc.sync).dma_start(
                out=rp_s[:], in_=rp_d[:])
            if lay == 0:
                fp_s = cpool.tile([128, 18 + H], F32, tag="c_fp")
                nc.scalar.dma_start(out=fp_s[:], in_=fp_d[:])
                if TPRE:
                    agp_s = cpool.tile([128, TPRE * 128], BF16, tag="c_agp")
                    nc.scalar.dma_start(out=agp_s[:], in_=agp_d[:])
            if lay == 2:
                mp_s = cpool.tile([64, 80], F32, tag="c_mp")
                nc.scalar.dma_start(out=mp_s[:], in_=mp_d[:])
            nc.sync.dma_start(out=pan_sb[:, PAN_OFF:PSPLIT],
                              in_=pan_d[:, PBASE + PAN_OFF:PBASE + PSPLIT])
            if lay == 0:
                sig_s = rp_s[0:1, 0:SHARD]
                sh_s = rp_s[0:1, SHARD:2 * SHARD]
                b_s = rp_s[0:1, 2 * SHARD:2 * SHARD + H]
            elif lay == 1:
                sig_s = rp_s[0:1, 0:SHARD]
                b_s = rp_s[0:1, SHARD:SHARD + H]
            else:
                b_s = mp_s[0:H2, 78:79]   # [H2, 1] f32 column
            zr_s = cpool.tile([1, 512], BF16, tag="c_zr")
            nc.vector.memset(zr_s[:], 0.0)
            if lay == 0:
                w1f_s = fp_s[:, 18:18 + H]
                w_s = cpool.tile([128, H], BF16, tag="c_wt")
                rw_s = cpool.tile([1, H], BF16, tag="c_rw")
            elif lay == 1:
                w_s = wpk_s[:, 0:H]
                w3_s = wpk_s[:, H:H + H2]
            else:
                gpan_s = pan_sb[:, 0:NT * G_PER]
                id_s = rp_s[:, 0:64]
                wc1_s = mp_s[:, 0:H4]                  # [64, 32]
                wc2_s = mp_s[0:H4, H4:H4 + C]          # [32, 2]
                bc1_r = mp_s[0:1, 34:66]               # [1, 32]
                cntx_r = mp_s[0:1, 66:74]              # [1, 8]
                bc2_r = mp_s[0:1, 74:76]               # [1, 2]
                invc_c = mp_s[0:G_PER, 76:77]          # [8, 1]

            # ---- BN statistics (layer 0) -> W~1 and shift row rw
            if lay == 0:
                with tc.tile_pool(name="ps_st", bufs=1, space="PSUM") as pst, \
                     tc.tile_pool(name="st_w", bufs=2) as stw:
                    sxp_s = fp_s[:, 0:8]
                    exp_s = fp_s[:, 8:16]
                    gam_s = fp_s[:, 16:17]
                    bet_s = fp_s[:, 17:18]
                    ex2 = stw.tile([128, 1], F32, tag="v1")
                    nc.vector.tensor_reduce(ex2[:], exp_s,
                                            mybir.AxisListType.X,
                                            mybir.AluOpType.add)
                    sx = stw.tile([128, 1], F32, tag="v0")
                    nc.vector.tensor_reduce(sx[:], sxp_s,
                                            mybir.AxisListType.X,
                                            mybir.AluOpType.add)
                    mu = stw.tile([128, 1], F32, tag="v2")
                    nc.vector.tensor_scalar_mul(mu[:], sx[:], 1.0 / N_true)
                    var = stw.tile([128, 1], F32, tag="v3")
                    nc.vector.tensor_scalar_mul(var[:], ex2[:], 1.0 / N_true)
                    mu2 = stw.tile([128, 1], F32, tag="v4")
                    nc.vector.tensor_tensor(mu2[:], mu[:], mu[:],
                                            mybir.AluOpType.mult)
                    nc.vector.tensor_tensor(var[:], var[:], mu2[:],
                                            mybir.AluOpType.subtract)
                    nc.vector.tensor_scalar_add(var[:], var[:], BN_EPS)
                    rec = stw.tile([128, 1], F32, tag="v5")
                    nc.vector.reciprocal(rec[:], var[:])
                    isd = stw.tile([128, 1], F32, tag="v6")
                    nc.scalar.activation(isd[:], rec[:],
                                         mybir.ActivationFunctionType.Sqrt)
                    a_c = stw.tile([128, 1], F32, tag="v7")
                    nc.vector.tensor_tensor(a_c[:], gam_s, isd[:],
                                            mybir.AluOpType.mult)
                    a8 = stw.tile([128, 1], F32, tag="v9")
                    nc.vector.tensor_scalar_mul(a8[:], a_c[:], 0.125)
                    nc.vector.tensor_scalar_mul(w_s[:], w1f_s, a8[:])
                    ca = stw.tile([128, 1], F32, tag="v8")
                    nc.vector.tensor_tensor(ca[:], mu[:], a_c[:],
                                            mybir.AluOpType.mult)
                    nc.vector.tensor_tensor(ca[:], bet_s, ca[:],
                                            mybir.AluOpType.subtract)
                    rw_ps = pst.tile([1, H], F32, tag="rw")
                    nc.tensor.matmul(rw_ps[:], ca[:], w1f_s,
                                     start=True, stop=True)
                    nc.scalar.activation(rw_s[:], rw_ps[:],
                                         mybir.ActivationFunctionType.Copy)

            spool = ctx.enter_context(tc.tile_pool(name="stg", bufs=1))
            wpool = ctx.enter_context(tc.tile_pool(name="wk", bufs=4))
            ps_agg = ctx.enter_context(
                tc.tile_pool(name="ps_agg", bufs=3, space="PSUM"))
            if lay < 2:
                ps_out = ctx.enter_context(
                    tc.tile_pool(name="ps_out", bufs=3, space="PSUM"))
            if lay == 1:
                ps_t = ctx.enter_context(
                    tc.tile_pool(name="ps_t", bufs=2, space="PSUM"))
                tstage = spool.tile([64, NT * 128], dt_out, tag="tstg")
            if lay == 2:
                ps_tr = ctx.enter_context(
                    tc.tile_pool(name="ps_tr", bufs=2, space="PSUM"))
                ps_pl = ctx.enter_context(
                    tc.tile_pool(name="ps_pl", bufs=1, space="PSUM"))
                pool_ps = ps_pl.tile([H2, G_PER], F32, tag="pool")

            if lay == 0:
                stage = spool.tile([128, NT * 128], dt_out, tag="stg")
            elif lay == 1:
                stage = spool.tile([128, NT * 128], BF16, tag="stg")

            state = {"use_dve": False, "rot": 0}

            def flip():
                state["use_dve"] = not state["use_dve"]
                return state["use_dve"]

            def rot():
                state["rot"] = (state["rot"] + 1) % 3
                return state["rot"]

            def split_copy(dst, src_ps, w):
                """PSUM->SBUF copy split across Act | DVE halves."""
                h = (w // 2 + 63) & ~63 if w > 128 else w
                nc.scalar.activation(dst[:, 0:h], src_ps[:, 0:h],
                                     mybir.ActivationFunctionType.Copy)
                if h < w:
                    nc.vector.tensor_copy(dst[:, h:w], src_ps[:, h:w])

            def split_relu(dst, src_ps, w):
                h = (w // 2 + 63) & ~63 if w > 128 else w
                nc.scalar.activation(dst[:, 0:h], src_ps[:, 0:h],
                                     mybir.ActivationFunctionType.Relu)
                if h < w:
                    nc.vector.tensor_scalar_max(dst[:, h:w],
                                                src_ps[:, h:w], 0.0)

            def split_relu_bias(dst, src_ps, w, bias):
                h = (w // 2 + 63) & ~63 if w > 128 else w
                nc.scalar.activation(dst[:, 0:h], src_ps[:, 0:h],
                                     mybir.ActivationFunctionType.Relu,
                                     bias=bias)
                if h < w:
                    nc.vector.tensor_scalar(dst[:, h:w], src_ps[:, h:w],
                                            bias, 0.0,
                                            mybir.AluOpType.add,
                                            mybir.AluOpType.max)

            def phase1(pr, dup_sb, b0):
                """agg matmuls (+ L2: bias + relu straight from PSUM)."""
                if dup_sb is None:   # lay0 tiles pre-aggregated in stats
                    return pr, None, agp_s[:, pr[0] * 128:(pr[-1] + 1) * 128]
                pw = len(pr) * 128
                rows = H2 if lay == 2 else 128
                agg_ps = ps_agg.tile([rows, pw], F32, tag="agg")
                nc.tensor.matmul(agg_ps[:], zr_s[0:1, 0:rows],
                                 zr_s[0:1, 0:pw], start=True, stop=False,
                                 skip_group_check=True)
                nb_pair = sum(int(kt[t]) for t in pr)
                bi = 0
                for hi, t in enumerate(pr):
                    for b, (lo, w) in enumerate(blocks[t]):
                        gb = int(tile_base[t] // 128) + b
                        co = pan_cols[t][b]
                        bi += 1
                        nc.tensor.matmul(
                            agg_ps[:, hi * 128 + lo:hi * 128 + lo + w],
                            dup_sb[:, (gb - b0) * F:(gb - b0 + 1) * F],
                            pan_sb[:, PAN_OFF + co - PBASE:
                                   PAN_OFF + co - PBASE + w],
                            start=False, stop=(bi == nb_pair),
                            skip_group_check=True)
                if lay == 2:
                    hsT = wpool.tile([H2, pw], BF16, tag="hsT")
                    if flip():
                        nc.vector.tensor_scalar(
                            hsT[:], agg_ps[:], b_s, 0.0,
                            mybir.AluOpType.add, mybir.AluOpType.max)
                    else:
                        nc.scalar.activation(
                            hsT[:], agg_ps[:],
                            mybir.ActivationFunctionType.Relu,
                            bias=b_s)
                    return pr, agg_ps, hsT
                aggT = wpool.tile([128, pw], BF16, tag="aggT")
                if flip():
                    nc.vector.tensor_copy(aggT[:], agg_ps[:])
                else:
                    nc.scalar.activation(aggT[:], agg_ps[:],
                                         mybir.ActivationFunctionType.Copy)
                return pr, agg_ps, aggT

            def phase2(st1):
                pr, agg_ps, aggT = st1
                pw = len(pr) * 128
                if lay < 2:
                    h_ps = ps_out.tile([Ho, pw], F32, tag="hps")
                    c0 = pr[0] * 128
                    nc.tensor.matmul(h_ps[:], w_s[:] if lay == 0 else w_s,
                                     aggT[:], start=True, stop=False,
                                     skip_group_check=True)
                    nc.tensor.matmul(h_ps[:], b_s,
                                     sig_s[0:1, c0:c0 + pw],
                                     start=False, stop=(lay != 0),
                                     skip_group_check=True)
                    if lay == 0:
                        nc.tensor.matmul(h_ps[:], rw_s[:],
                                         sh_s[0:1, c0:c0 + pw],
                                         start=False, stop=True,
                                         skip_group_check=True)
                    so = pr[0] * 128
                    if flip():
                        nc.vector.tensor_scalar_max(
                            stage[:, so:so + pw], h_ps[:], 0.0)
                    else:
                        nc.scalar.activation(
                            stage[:, so:so + pw], h_ps[:],
                            mybir.ActivationFunctionType.Relu)
                    return st1
                # lay 2: transpose each tile's hsT: [64, 128] -> [128, 64]
                hsT = aggT
                tr_ps = ps_tr.tile([128, len(pr) * H2], BF16, tag="tr")
                for hi, t in enumerate(pr):
                    nc.tensor.transpose(tr_ps[:, hi * H2:(hi + 1) * H2],
                                        hsT[:, hi * 128:(hi + 1) * 128],
                                        id_s[0:64, 0:64])
                hs_sb = wpool.tile([128, len(pr) * H2], BF16, tag="hs")
                if flip():
                    nc.vector.tensor_copy(hs_sb[:], tr_ps[:])
                else:
                    nc.scalar.activation(
                        hs_sb[:], tr_ps[:],
                        mybir.ActivationFunctionType.Copy)
                return [(t, hs_sb, hi * H2) for hi, t in enumerate(pr)]

            def phase3(st2):
                if lay == 2:
                    flip()          # odd flips/group: engines alternate
                if lay == 1:
                    pr = st2[0]
                    pw = len(pr) * 128
                    so = pr[0] * 128
                    t_ps = ps_t.tile([H2, pw], F32, tag="tps")
                    nc.tensor.matmul(t_ps[:], w3_s, stage[:, so:so + pw],
                                     start=True, stop=True,
                                     skip_group_check=True)
                    if flip():
                        nc.vector.tensor_copy(tstage[:, so:so + pw], t_ps[:])
                    else:
                        nc.scalar.activation(
                            tstage[:, so:so + pw], t_ps[:],
                            mybir.ActivationFunctionType.Copy)
                elif lay == 2:
                    for t, hs_sb, off in st2:
                        state["npool"] = state.get("npool", 0) + 1
                        nc.tensor.matmul(pool_ps[:],
                                         hs_sb[:, off:off + H2],
                                         gpan_s[:, t * G_PER:(t + 1) * G_PER],
                                         start=(state["npool"] == 1),
                                         stop=(state["npool"] == NT),
                                         skip_group_check=True)

            all_pairs = []
            for ci, tiles in enumerate(chunk_tiles):
                dup_sb, b0 = pend.pop(0)
                if ci + 1 < len(chunk_tiles):
                    pend.append(chunk_loads(chunk_tiles[ci + 1]))
                GW = 4
                grps = [tiles[i:i + GW] for i in range(0, len(tiles), GW)]
                for g in reversed(grps):
                    all_pairs.append((g, dup_sb, b0))
            if lay == 0 and TPRE:
                # pre-aggregated thin tiles: compute-only, processed last
                pg = [list(range(i, min(i + 4, TPRE)))
                      for i in range(0, TPRE, 4)]
                for g in reversed(pg):
                    all_pairs.append((g, None, None))

            hastail = lay > 0
            q2, q3 = [], []
            out_stage = stage if lay == 0 else (tstage if lay == 1 else None)
            OW = 128 if lay == 0 else 64
            wb = [NT, 24, 8, 2, 0]
            WRITES = [(wb[i + 1], wb[i]) for i in range(len(wb) - 1)]

            def maybe_write(done_min):
                if lay == 2:
                    return
                while WRITES and done_min <= WRITES[0][0]:
                    wt0, wt1 = WRITES.pop(0)
                    q = nc.sync if wt0 == 0 else nc.gpsimd
                    q.dma_start(
                        out=h_out[:, wt0 * 128:wt1 * 128],
                        in_=out_stage[:, wt0 * 128:wt1 * 128])

            def run3():
                st3 = q3.pop(0)
                phase3(st3)
                done = st3[0][0] if lay == 1 else st3[0][0]
                maybe_write(done)

            def run2():
                st2 = phase2(q2.pop(0))
                if hastail:
                    q3.append(st2)
                else:
                    maybe_write(st2[0][0])

            LAG2 = 1 if lay == 1 else 2
            LAG3 = 1
            for item in all_pairs:
                st1 = phase1(*item)
                if len(q3) >= LAG3:
                    run3()
                if len(q2) >= LAG2:
                    run2()
                q2.append(st1)
            while q2 or q3:
                if q3:
                    run3()
                if q2:
                    run2()

            # ---- classifier MLP on this core's G_PER graphs (lay 2)
            if lay == 2:
                p01 = wpool.tile([H2, G_PER], F32, tag="p01")
                nc.vector.tensor_copy(p01[:], pool_ps[:])
                y_ps = ps_pl.tile([H4, G_PER], F32, tag="yps")
                nc.tensor.matmul(y_ps[:], wc1_s, p01[:],
                                 start=True, stop=False,
                                 skip_group_check=True)
                nc.tensor.matmul(y_ps[:], bc1_r, cntx_r,
                                 start=False, stop=True,
                                 skip_group_check=True)
                y_s = wpool.tile([H4, G_PER], F32, tag="ys")
                nc.vector.tensor_scalar_max(y_s[:], y_ps[:], 0.0)
                o_ps = ps_pl.tile([G_PER, C], F32, tag="ops")
                nc.tensor.matmul(o_ps[:], y_s[:], wc2_s,
                                 start=True, stop=False,
                                 skip_group_check=True)
                nc.tensor.matmul(o_ps[:], cntx_r, bc2_r,
                                 start=False, stop=True,
                                 skip_group_check=True)
                o_s = wpool.tile([G_PER, C], F32, tag="os")
                nc.scalar.activation(o_s[:], o_ps[:],
                                     mybir.ActivationFunctionType.Copy,
                                     scale=invc_c)
                nc.sync.dma_start(out=out_d[:], in_=o_s[:])

    nc.compile()
    return nc


# ------------------------------------------------------------------ driver
_CACHE = {}


def _get_programs(meta):
    key = (tuple(meta["kt"]), meta["n_true"], meta["NT"])
    if key not in _CACHE:
        progs = [_build_stats_program(meta)]
        progs += [_build_layer_program(meta, lay) for lay in range(3)]
        _CACHE[key] = progs
    return _CACHE[key]


def run_gnn(runner=None, **inputs):
    F, H, H2, H4, C = 128, 128, 64, 32, 2
    x = np.asarray(inputs["x"], np.float32)
    n_true = x.shape[0]
    src = np.asarray(inputs["edge_index"][0], np.int64)
    dst = np.asarray(inputs["edge_index"][1], np.int64)
    batch = np.asarray(inputs["batch"], np.int64)

    meta = _plan(src, dst, batch, n_true)
    NT, SHARD, NPAD = meta["NT"], meta["SHARD"], meta["NPAD"]
    cores = _build_static(meta, src, dst, batch)
    order = meta["order"]
    progs = _get_programs(meta)

    def run(nc, in_maps):
        if runner is not None:
            return runner(nc, in_maps)
        return run_bass_kernel_spmd(
            nc, in_maps, core_ids=list(range(N_CORES))).results

    x_new = np.zeros((NPAD + 1, F), np.float32)
    x_new[:NPAD][order < n_true] = x[order[order < n_true]]

    # ---- stats launch (BN partials + L0 pre-agg of tiles 0..TPRE-1)
    xb = x_new[:NPAD].astype(NPFP8)
    l0_dups = [_dup_layout(x_new, cores[c]["slotsrc"], DUP_NP[0])
               for c in range(N_CORES)]
    stats_maps = []
    for c in range(N_CORES):
        idx = ((np.arange(NT) * N_CORES + c)[:, None] * 128
               + np.arange(128)[None, :])
        slab = xb[idx]
        slab = np.ascontiguousarray(slab.transpose(1, 0, 2)).reshape(
            128, NT * F)
        stats_maps.append({
            "x_sh": slab, "ident": np.eye(128, dtype=np.float32)})
    res = run(progs[0], stats_maps)
    parts = np.stack([np.asarray(res[c]["stat_part"])
                      for c in range(N_CORES)], axis=2)
    sx_parts = np.ascontiguousarray(parts[:, 0, :], dtype=np.float32)
    ex2_parts = np.ascontiguousarray(parts[:, 1, :], dtype=np.float32)

    W = [np.asarray(inputs["W1"], np.float32),
         np.asarray(inputs["W2"], np.float32),
         np.asarray(inputs["W3"], np.float32)]
    brows = [np.asarray(inputs["b1"], np.float32).reshape(1, H),
             np.asarray(inputs["b2"], np.float32).reshape(1, H),
             np.asarray(inputs["b3"], np.float32).reshape(1, H2)]

    h_new = x_new
    core_out = None
    for lay in range(3):
        maps = []
        for c in range(N_CORES):
            st = cores[c]
            if lay == 0:
                rp = np.concatenate([st["sig_row"], st["sh_row"],
                                     brows[0].ravel()])
            elif lay == 1:
                rp = np.concatenate([st["sig_row"], brows[1].ravel()])
            else:
                rp = np.zeros((128, 65), np.float64)
                rp[0:64, 0:64] = np.eye(64)
            if lay == 1:
                pan = st["pans"][1]
            elif lay == 2:
                pan = np.concatenate([st["gpan"].astype(NPFP8),
                                      st["pans"][2]], axis=1)
            else:
                pan = st["pans"][0]
            m = {"dup": l0_dups[c] if lay == 0 else
                 _dup_layout(h_new, st["slotsrc"], DUP_NP[lay]),
                 "pan": np.ascontiguousarray(pan),
                 "rowpack": (rp.astype(NPBF16).reshape(1, -1) if lay < 2
                             else np.ascontiguousarray(rp.astype(NPBF16)))}
            if lay == 1:
                m["wpack"] = np.ascontiguousarray(np.concatenate(
                    [(W[1] / 8.0).astype(NPBF16), W[2].astype(NPBF16)],
                    axis=1))
            if lay == 0:
                fp = np.zeros((128, 18 + H), np.float32)
                fp[:, 0:8] = sx_parts
                fp[:, 8:16] = ex2_parts
                fp[:, 16] = np.asarray(inputs["bn_gamma"], np.float32)
                fp[:, 17] = np.asarray(inputs["bn_beta"], np.float32)
                fp[:, 18:] = W[0]
                m["f32pack"] = fp
            if lay == 2:
                mp = np.zeros((64, 80), np.float32)
                mp[:, 0:H4] = np.asarray(inputs["Wc1"], np.float32)
                mp[0:H4, H4:H4 + C] = np.asarray(inputs["Wc2"], np.float32)
                mp[0, 34:66] = np.asarray(inputs["bc1"], np.float32)
                mp[0, 66:74] = st["cntx"] * 4.0
                mp[0, 74:76] = np.asarray(inputs["bc2"], np.float32)
                mp[0:G_PER, 76] = st["invc"] / 4.0
                mp[0:64, 78] = np.asarray(inputs["b3"], np.float32) * 4.0
                m["mpack"] = mp
            maps.append(m)
        res = run(progs[1 + lay], maps)
        if lay < 2:
            OW = 128 if lay == 0 else 64
            h_new = np.zeros((NPAD + 1, OW), np.float32)
            for c in range(N_CORES):
                ho = np.asarray(res[c]["h_out"])
                hoT = ho.reshape(OW, NT, 128).transpose(1, 2, 0)
                idx = ((np.arange(NT) * N_CORES + c)[:, None] * 128
                       + np.arange(128)[None, :])
                h_new[idx] = hoT
        else:
            core_out = [np.asarray(res[c]["out"]) for c in range(N_CORES)]

    out = np.zeros((G, C), np.float32)
    for c in range(N_CORES):
        for lg, g in enumerate(meta["core_graphs"][c]):
            out[g] = core_out[c][lg]
    return out


def kernel(**inputs):
    return run_gnn(**inputs)


# revision 36
# speedup vs baseline: 1.0075x; 1.0000x over previous
"""Trainium2 Bass kernel for AudioOnlyGNN (3-layer GCN + BatchNorm + mean-pool + MLP).

v3 — graph-partitioned static slot stream:

Nodes are assigned to cores by *graph* ownership (8 graphs per core,
balanced by node count), then degree-sorted within each core and laid out in
128-row tiles; tile t's slot budget k_t = max in-degree(+self) over that tile
across all cores, giving a static slot stream identical on every core.  For
each layer the host materialises the edge-source rows in slot order (a pure
gather) so the device reads large contiguous DMA blocks.

On device, a 128-slot block contributes to a [F, ncols] PSUM tile via one
matmul whose moving operand is a narrow "panel" (slot -> dst column weight
with the GCN normalisation baked in).  The aggregate is transformed
(W^T @ agg), bias/BN-shift added as rank-1 matmuls, ReLU'd, written back.
Layers 0/1 write h' = dinv*ReLU(...) so panels never depend on h.

Because every graph lives entirely on one core, the mean-pool and classifier
MLP complete locally inside the L2 launch (no cross-core reduction): launches
are [stats+pre-agg] [L0] [L1] [L2+pool+mlp].  Between launches the host only
reorders bytes (gather / transpose), never does arithmetic on activations.
"""

import sys

sys.path.insert(0, "/opt/trn_rl_repo")

import contextlib

import numpy as np
import ml_dtypes

import concourse.bacc as bacc
import concourse.bass as bass
import concourse.mybir as mybir
from concourse.tile import TileContext
from concourse.bass_utils import run_bass_kernel_spmd

BF16 = mybir.dt.bfloat16
F32 = mybir.dt.float32
FP8 = mybir.dt.float8e3  # e3m4

NPBF16 = ml_dtypes.bfloat16
NPFP8 = ml_dtypes.float8_e3m4

N_CORES = 8
BN_EPS = 1e-5
G = 64
G_PER = G // N_CORES   # graphs per core
TPRE = 0               # tiles of L0 pre-aggregated inside the stats launch

# dtype of the host-expanded per-slot source rows, per layer
DUP_DT = [FP8, FP8, FP8]
DUP_NP = [NPFP8, NPFP8, NPFP8]
OUT_DT = [FP8, FP8]
OUT_NP = [NPFP8, NPFP8]


def _chunk_list(n0, n1, lead, mid, tail=(4, 2, 1)):
    """Chunk [n0, n1) into sizes lead + [mid...] + tail (tapered ends)."""
    n = n1 - n0
    sizes = []
    for s in lead:
        if sum(sizes) + s > n:
            break
        sizes.append(s)
    tl = [s for s in tail if s < mid]
    while sum(sizes) + sum(tl) + mid <= n:
        sizes.append(mid)
    rem = n - sum(sizes) - sum(tl)
    while rem > 0:
        add = min(rem, mid)
        sizes.append(add)
        rem -= add
    sizes += tl
    sizes = [s for s in sizes if s > 0]
    # clip overflow
    while sum(sizes) > n:
        sizes[-1] -= sum(sizes) - n
        sizes = [s for s in sizes if s > 0]
    out = []
    t = n0
    for cs in sizes:
        out.append(list(range(t, t + cs)))
        t += cs
    assert t == n1, (sizes, n0, n1)
    return out


# ------------------------------------------------------------------ planning
def _plan(src, dst, batch, n_true):
    """Static (h-independent) structure: graph packing, renumbering, slots."""
    cnt_g = np.bincount(batch, minlength=G).astype(np.int64)
    g_order = np.argsort(-cnt_g, kind="stable")
    core_graphs = [[] for _ in range(N_CORES)]
    loads = np.zeros(N_CORES, np.int64)
    for g in g_order:
        cand = [i for i in range(N_CORES) if len(core_graphs[i]) < G_PER]
        i = min(cand, key=lambda i: loads[i])
        core_graphs[i].append(int(g))
        loads[i] += cnt_g[g]
    NT = max(49, int(-(-loads.max() // 128)))
    SHARD = NT * 128
    NPAD = N_CORES * SHARD

    graph_core = np.zeros(G, np.int64)
    graph_local = np.zeros(G, np.int64)
    for c in range(N_CORES):
        for lg, g in enumerate(core_graphs[c]):
            graph_core[g] = c
            graph_local[g] = lg

    degp_true = np.bincount(dst, minlength=n_true).astype(np.int64) + 1
    node_core = graph_core[batch]

    order = np.empty(NPAD, np.int64)
    virt = n_true
    for c in range(N_CORES):
        nodes_c = np.where(node_core == c)[0]
        nodes_c = nodes_c[np.argsort(degp_true[nodes_c], kind="stable")]
        npadc = SHARD - len(nodes_c)
        ids = np.concatenate([np.arange(virt, virt + npadc), nodes_c])
        virt += npadc
        idx = ((np.arange(NT) * N_CORES + c)[:, None] * 128
               + np.arange(128)[None, :])
        order[idx.ravel()] = ids
    assert virt == NPAD
    newpos = np.empty(NPAD, np.int64)
    newpos[order] = np.arange(NPAD)

    degp = np.zeros(NPAD, np.int64)
    degp[:n_true] = degp_true

    kt = np.zeros(NT, np.int64)
    for t in range(NT):
        kt[t] = degp[order[t * 1024:(t + 1) * 1024]].max()
    kt = np.maximum(kt, 1)

    blocks = []   # per tile: list of (lo, w)
    pan_cols = []  # per tile: list of panel col offsets
    wtot = 0
    for t in range(NT):
        k = int(kt[t])
        bl = []
        for b in range(k):
            lo = (128 * b) // k
            hi = (128 * (b + 1) - 1) // k
            bl.append((lo, hi - lo + 1))
        blocks.append(bl)
        offs = []
        for lo, w in bl:
            offs.append(wtot)
            wtot += w
        pan_cols.append(offs)

    nblk = int(kt.sum())
    tile_base = np.zeros(NT + 1, np.int64)
    tile_base[1:] = np.cumsum(128 * kt)
    meta = {"kt": kt, "blocks": blocks, "pan_cols": pan_cols,
            "wtot": wtot, "nblk": nblk, "order": order, "newpos": newpos,
            "n_true": n_true, "tile_base": tile_base,
            "total_slots": int(tile_base[-1]),
            "NT": NT, "SHARD": SHARD, "NPAD": NPAD,
            "core_graphs": core_graphs, "graph_core": graph_core,
            "graph_local": graph_local, "cnt_g": cnt_g}
    return meta


def _build_static(meta, src, dst, batch):
    """Per-core constant tables: slot->src map, per-layer panels, rows."""
    kt, blocks, pan_cols = meta["kt"], meta["blocks"], meta["pan_cols"]
    wtot, nblk, order, newpos = (meta["wtot"], meta["nblk"], meta["order"],
                                 meta["newpos"])
    n_true = meta["n_true"]
    NT, SHARD, NPAD = meta["NT"], meta["SHARD"], meta["NPAD"]
    graph_local, cnt_g = meta["graph_local"], meta["cnt_g"]

    deg = np.bincount(dst, minlength=NPAD).astype(np.float64) + 1.0
    dinv = (1.0 / np.sqrt(deg)).astype(np.float64)
    dinv_pad = dinv.copy()
    dinv_pad[n_true:] = 1.0

    dinv_new = dinv_pad[order]
    batch_pad = np.full(NPAD, 0, np.int64)
    batch_pad[:n_true] = batch
    batch_new = batch_pad[order]
    valid_new = (order < n_true)

    sneig = np.bincount(dst, weights=dinv[src], minlength=NPAD)
    d2 = dinv_pad * (sneig + dinv_pad)
    d2_new = d2[order]

    cntx = np.maximum(cnt_g.astype(np.float64), 1.0)   # [G]
    invc = 1.0 / cntx

    s_new = newpos[src]
    d_new = newpos[dst]
    g_tile = d_new // 128
    core_of = g_tile % N_CORES
    tloc = g_tile // N_CORES
    dloc = d_new % 128

    tile_base = meta["tile_base"]
    total_slots = meta["total_slots"]

    edge_w0 = dinv[src] * dinv_pad[dst] * dinv_pad[dst]

    cores = []
    for c in range(N_CORES):
        sel = core_of == c
        es, et, ed = s_new[sel], tloc[sel], dloc[sel]
        ew0 = edge_w0[sel]
        key = et * (128 * 64) + ed
        o = np.argsort(key, kind="stable")
        es, et, ed, ew0 = es[o], et[o], ed[o], ew0[o]
        k_of = kt[et]
        node_key = et * 128 + ed
        uniq, first_idx, counts = np.unique(node_key, return_index=True,
                                            return_counts=True)
        rank = np.arange(len(node_key)) - np.repeat(first_idx, counts)
        slot = tile_base[et] + ed * k_of + 1 + rank   # +1: self slot at 0

        tt = np.arange(NT).repeat(128)
        dd = np.tile(np.arange(128), NT)
        own_new = (tt * N_CORES + np.full(NT * 128, c)) * 128 + dd
        own_valid = valid_new[own_new]
        self_slot = tile_base[tt] + dd * kt[tt]

        slotsrc = np.full(total_slots, NPAD, np.int64)  # NPAD -> zero row
        slotsrc[slot] = es
        slotsrc[self_slot[own_valid]] = own_new[own_valid]

        dv_own = dinv_new[own_new]
        w_l0 = np.zeros(total_slots, np.float64)
        w_l0[slot] = ew0
        w_l0[self_slot[own_valid]] = (dv_own ** 3)[own_valid]
        col_dinv = np.repeat(dv_own, np.repeat(kt, 128))
        filled = np.zeros(total_slots, bool)
        filled[slot] = True
        filled[self_slot[own_valid]] = True
        w_l1 = np.where(filled, col_dinv ** 2, 0.0)
        w_l2 = np.where(filled, col_dinv, 0.0)

        pans = []
        for wv, psc in ((w_l0, 8.0), (w_l1, 8.0), (w_l2, 4.0)):
            pan = np.zeros((128, wtot), np.float64)
            for t in range(NT):
                k = int(kt[t])
                for b, (lo, w) in enumerate(blocks[t]):
                    co = pan_cols[t][b]
                    sl0 = tile_base[t] + b * 128
                    ss = np.arange(sl0, sl0 + 128)
                    cc = (ss - tile_base[t]) // k - lo
                    ok = (cc >= 0) & (cc < w)
                    pan[np.arange(128)[ok], co + cc[ok]] = wv[ss][ok]
            pans.append((pan * psc).astype(NPFP8))

        sig_row = np.zeros(SHARD, np.float64)
        sh_row = np.zeros(SHARD, np.float64)
        for t in range(NT):
            cols = slice(t * 128, (t + 1) * 128)
            nn = (t * N_CORES + c) * 128 + np.arange(128)
            sig_row[cols] = dinv_new[nn]
            sh_row[cols] = d2_new[nn] * dinv_new[nn]

        # pool panel [128, NT*G_PER]: 1.0 at (d, t*G_PER + local_graph)
        gpan = np.zeros((128, NT * G_PER), np.float64)
        for t in range(NT):
            nn = (t * N_CORES + c) * 128 + np.arange(128)
            gb = graph_local[batch_new[nn]]
            ok = valid_new[nn]
            gpan[np.arange(128)[ok], t * G_PER + gb[ok]] = 1.0

        cg = meta["core_graphs"][c]
        cores.append({
            "slotsrc": slotsrc,
            "pans": pans,
            "sig_row": sig_row,
            "sh_row": sh_row,
            "gpan": gpan.astype(NPBF16),
            "cntx": cntx[cg].astype(np.float32),     # [G_PER]
            "invc": invc[cg].astype(np.float32),     # [G_PER]
        })
    return cores


def _dup_layout(h_new, slotsrc, np_dt):
    """[NPAD(+1), F] new-indexed rows -> [128, NBLK*F] slot-stream layout."""
    rows = h_new[slotsrc]
    nblk = rows.shape[0] // 128
    F = rows.shape[1]
    return np.ascontiguousarray(
        rows.reshape(nblk, 128, F).transpose(1, 0, 2)
    ).reshape(128, nblk * F).astype(np_dt)


# ------------------------------------------------------------------ programs
def _build_stats_program(meta):
    """Per-core BN partial sums (Sum x, Sum x^2 over own nodes)."""
    F = 128
    NT = meta["NT"]
    nc = bacc.Bacc("TRN2", target_bir_lowering=False, debug=False,
                   num_devices=N_CORES)
    xs_d = nc.dram_tensor("x_sh", [128, NT * F], FP8,
                          kind="ExternalInput").ap()
    ident_d = nc.dram_tensor("ident", [128, 128], F32,
                             kind="ExternalInput").ap()
    out_d = nc.dram_tensor("stat_part", [128, 2], F32,
                           kind="ExternalOutput").ap()
    XS = [0, 12, 24, 36, 45, NT]
    with TileContext(nc) as tc:
        with tc.tile_pool(name="w", bufs=1) as wp, \
             tc.tile_pool(name="ps", bufs=1, space="PSUM") as pp:
            xs = wp.tile([128, NT * F], FP8, tag="xs")
            ident_s = wp.tile([128, 128], F32, tag="id")
            nc.sync.dma_start(out=xs[:, :XS[1] * F], in_=xs_d[:, :XS[1] * F])
            nc.scalar.dma_start(out=ident_s[:], in_=ident_d[:])
            for q in range(1, len(XS) - 1):
                nc.sync.dma_start(out=xs[:, XS[q] * F:XS[q + 1] * F],
                                  in_=xs_d[:, XS[q] * F:XS[q + 1] * F])
            ones_s = wp.tile([128, 1], FP8, tag="ones")
            nc.vector.memset(ones_s[:], 1.0)
            xtx_ps = pp.tile([128, 128], F32, tag="xtx")
            sx_ps = pp.tile([128, 1], F32, tag="sx")
            for t in range(NT):
                sl = xs[:, t * F:(t + 1) * F]
                nc.tensor.matmul(xtx_ps[:], sl, sl, start=(t == 0),
                                 stop=(t == NT - 1), skip_group_check=True)
                nc.tensor.matmul(sx_ps[:], sl, ones_s[:],
                                 start=(t == 0), stop=(t == NT - 1),
                                 skip_group_check=True)
            dg = wp.tile([128, 128], F32, tag="dg")
            nc.vector.tensor_tensor(dg[:], xtx_ps[:], ident_s[:],
                                    mybir.AluOpType.mult)
            o = wp.tile([128, 2], F32, tag="o")
            nc.vector.tensor_reduce(o[:, 1:2], dg[:], mybir.AxisListType.X,
                                    mybir.AluOpType.add)
            nc.vector.tensor_copy(o[:, 0:1], sx_ps[:])
            nc.scalar.dma_start(out=out_d[:], in_=o[:])
    nc.compile()
    return nc


def _build_layer_program(meta, lay):
    kt, blocks, pan_cols, wtot, nblk, tile_base = (
        meta["kt"], meta["blocks"], meta["pan_cols"], meta["wtot"],
        meta["nblk"], meta["tile_base"])
    NT, SHARD = meta["NT"], meta["SHARD"]
    F = 128 if lay < 2 else 64
    H = 128
    H2 = 64
    H4 = 32
    C = 2
    Ho = H if lay < 2 else H2
    N_true = meta["n_true"]
    dt_in = DUP_DT[lay]
    dt_out = OUT_DT[lay] if lay < 2 else None

    nc = bacc.Bacc("TRN2", target_bir_lowering=False, debug=False,
                   num_devices=N_CORES)

    def din(name, shape, dt):
        return nc.dram_tensor(name, list(shape), dt, kind="ExternalInput").ap()

    dup_d = din("dup", [128, nblk * F], dt_in)
    if lay == 2:
        PW_EXTRA = NT * G_PER       # gpan (0/1: fp8-exact)
    else:
        PW_EXTRA = 0               # W1 in f32pack; W2|W3 in wpack
    pan_d = din("pan", [128, wtot + PW_EXTRA], FP8)
    if lay == 1:
        wp_d = din("wpack", [128, H + H2], BF16)
    # packed bf16 row constants
    if lay == 0:
        RP = 2 * SHARD + H        # sig | sh | b1
    elif lay == 1:
        RP = SHARD + H            # sig | b2
    else:
        RP = 1                    # b3 as a column
    rp_d = din("rowpack", [1, RP] if lay < 2 else [128, 65], BF16)
    if lay == 0:
        # sxp | exp | gamma | beta | W1(fp32)
        fp_d = din("f32pack", [128, 18 + H], F32)
        if TPRE:
            agp_d = din("aggT_pre", [128, TPRE * 128], BF16)
    if lay == 2:
        # mlp pack: Wc1 | Wc2 | bc1row | cntx | bc2 | invc  (f32)
        mp_d = din("mpack", [64, 80], F32)
        out_d = nc.dram_tensor("out", [G_PER, C], F32,
                               kind="ExternalOutput").ap()
    else:
        OW = 128 if lay == 0 else 64
        h_out = nc.dram_tensor("h_out", [OW, NT * 128], dt_out,
                               kind="ExternalOutput").ap()

    # process tiles high->low: degree sorting puts fat tiles at high
    # indices, so the tail (last chunk + final write) covers thin tiles.
    T0 = TPRE if lay == 0 else 0
    fwd = _chunk_list(T0, NT, [2, 2, 4], 8, tail=(4, 2))
    chunk_tiles = []
    hi = NT
    for ch in fwd:
        chunk_tiles.append(list(range(hi - len(ch), hi)))
        hi -= len(ch)
    assert hi == T0
    PBASE = pan_cols[TPRE][0] if lay == 0 else 0

    with TileContext(nc) as tc:
        with contextlib.ExitStack() as ctx:
            cpool = ctx.enter_context(tc.tile_pool(name="const", bufs=1))
            dpool = ctx.enter_context(tc.tile_pool(name="dup", bufs=5))
            ppool = ctx.enter_context(tc.tile_pool(name="pan", bufs=2))

            def chunk_loads(tiles):
                ct0, ct1 = tiles[0], tiles[-1] + 1
                b0 = int(tile_base[ct0] // 128)
                b1 = int(tile_base[ct1] // 128)
                dup_sb = dpool.tile([128, (b1 - b0) * F], dt_in, tag="dup")
                nc.sync.dma_start(out=dup_sb[:], in_=dup_d[:, b0 * F:b1 * F])
                return dup_sb, b0

            pend = [chunk_loads(chunk_tiles[0])]
            pan_sb = ppool.tile([128, wtot - PBASE + PW_EXTRA], FP8,
                                tag="pan")
            if lay == 1:
                wpk_s = cpool.tile([128, H + H2], BF16, tag="c_wpk")
                nc.scalar.dma_start(out=wpk_s[:], in_=wp_d[:])
            fst = NT - 12
            PAN_OFF = PW_EXTRA
            PSPLIT = PAN_OFF + pan_cols[fst][0] - PBASE
            nc.sync.dma_start(out=pan_sb[:, PSPLIT:],
                              in_=pan_d[:, PBASE + PSPLIT:])
            if PW_EXTRA:
                nc.scalar.dma_start(out=pan_sb[:, :PW_EXTRA],
                                    in_=pan_d[:, PBASE:PBASE + PW_EXTRA])

            rp_s = cpool.tile([1, RP] if lay < 2 else [128, 65], BF16,
                              tag="c_rp")
            (nc.scalar if lay == 0 else nc.sync).dma_start(
                out=rp_s[:], in_=rp_d[:])
            if lay == 0:
                fp_s = cpool.tile([128, 18 + H], F32, tag="c_fp")
                nc.scalar.dma_start(out=fp_s[:], in_=fp_d[:])
                if TPRE:
                    agp_s = cpool.tile([128, TPRE * 128], BF16, tag="c_agp")
                    nc.scalar.dma_start(out=agp_s[:], in_=agp_d[:])
            if lay == 2:
                mp_s = cpool.tile([64, 80], F32, tag="c_mp")
                nc.scalar.dma_start(out=mp_s[:], in_=mp_d[:])
            nc.sync.dma_start(out=pan_sb[:, PAN_OFF:PSPLIT],
                              in_=pan_d[:, PBASE + PAN_OFF:PBASE + PSPLIT])
            if lay == 0:
                sig_s = rp_s[0:1, 0:SHARD]
                sh_s = rp_s[0:1, SHARD:2 * SHARD]
                b_s = rp_s[0:1, 2 * SHARD:2 * SHARD + H]
            elif lay == 1:
                sig_s = rp_s[0:1, 0:SHARD]
                b_s = rp_s[0:1, SHARD:SHARD + H]
            else:
                b_s = mp_s[0:H2, 78:79]   # [H2, 1] f32 column
            zr_s = cpool.tile([1, 512], BF16, tag="c_zr")
            nc.vector.memset(zr_s[:], 0.0)
            if lay == 0:
                w1f_s = fp_s[:, 18:18 + H]
                w_s = cpool.tile([128, H], BF16, tag="c_wt")
                rw_s = cpool.tile([1, H], BF16, tag="c_rw")
            elif lay == 1:
                w_s = wpk_s[:, 0:H]
                w3_s = wpk_s[:, H:H + H2]
            else:
                gpan_s = pan_sb[:, 0:NT * G_PER]
                id_s = rp_s[:, 0:64]
                wc1_s = mp_s[:, 0:H4]                  # [64, 32]
                wc2_s = mp_s[0:H4, H4:H4 + C]          # [32, 2]
                bc1_r = mp_s[0:1, 34:66]               # [1, 32]
                cntx_r = mp_s[0:1, 66:74]              # [1, 8]
                bc2_r = mp_s[0:1, 74:76]               # [1, 2]
                invc_c = mp_s[0:G_PER, 76:77]          # [8, 1]

            # ---- BN statistics (layer 0) -> W~1 and shift row rw
            if lay == 0:
                with tc.tile_pool(name="ps_st", bufs=1, space="PSUM") as pst, \
                     tc.tile_pool(name="st_w", bufs=2) as stw:
                    sxp_s = fp_s[:, 0:8]
                    exp_s = fp_s[:, 8:16]
                    gam_s = fp_s[:, 16:17]
                    bet_s = fp_s[:, 17:18]
                    ex2 = stw.tile([128, 1], F32, tag="v1")
                    nc.vector.tensor_reduce(ex2[:], exp_s,
                                            mybir.AxisListType.X,
                                            mybir.AluOpType.add)
                    sx = stw.tile([128, 1], F32, tag="v0")
                    nc.vector.tensor_reduce(sx[:], sxp_s,
                                            mybir.AxisListType.X,
                                            mybir.AluOpType.add)
                    mu = stw.tile([128, 1], F32, tag="v2")
                    nc.vector.tensor_scalar_mul(mu[:], sx[:], 1.0 / N_true)
                    var = stw.tile([128, 1], F32, tag="v3")
                    nc.vector.tensor_scalar_mul(var[:], ex2[:], 1.0 / N_true)
                    mu2 = stw.tile([128, 1], F32, tag="v4")
                    nc.vector.tensor_tensor(mu2[:], mu[:], mu[:],
                                            mybir.AluOpType.mult)
                    nc.vector.tensor_tensor(var[:], var[:], mu2[:],
                                            mybir.AluOpType.subtract)
                    nc.vector.tensor_scalar_add(var[:], var[:], BN_EPS)
                    rec = stw.tile([128, 1], F32, tag="v5")
                    nc.vector.reciprocal(rec[:], var[:])
                    isd = stw.tile([128, 1], F32, tag="v6")
                    nc.scalar.activation(isd[:], rec[:],
                                         mybir.ActivationFunctionType.Sqrt)
                    a_c = stw.tile([128, 1], F32, tag="v7")
                    nc.vector.tensor_tensor(a_c[:], gam_s, isd[:],
                                            mybir.AluOpType.mult)
                    a8 = stw.tile([128, 1], F32, tag="v9")
                    nc.vector.tensor_scalar_mul(a8[:], a_c[:], 0.125)
                    nc.vector.tensor_scalar_mul(w_s[:], w1f_s, a8[:])
                    ca = stw.tile([128, 1], F32, tag="v8")
                    nc.vector.tensor_tensor(ca[:], mu[:], a_c[:],
                                            mybir.AluOpType.mult)
                    nc.vector.tensor_tensor(ca[:], bet_s, ca[:],
                                            mybir.AluOpType.subtract)
                    rw_ps = pst.tile([1, H], F32, tag="rw")
                    nc.tensor.matmul(rw_ps[:], ca[:], w1f_s,
                                     start=True, stop=True)
                    nc.scalar.activation(rw_s[:], rw_ps[:],
                                         mybir.ActivationFunctionType.Copy)

            spool = ctx.enter_context(tc.tile_pool(name="stg", bufs=1))
            wpool = ctx.enter_context(tc.tile_pool(name="wk", bufs=4))
            ps_agg = ctx.enter_context(
                tc.tile_pool(name="ps_agg", bufs=3, space="PSUM"))
            if lay < 2:
                ps_out = ctx.enter_context(
                    tc.tile_pool(name="ps_out", bufs=3, space="PSUM"))
            if lay == 1:
                ps_t = ctx.enter_context(
                    tc.tile_pool(name="ps_t", bufs=2, space="PSUM"))
                tstage = spool.tile([64, NT * 128], dt_out, tag="tstg")
            if lay == 2:
                ps_tr = ctx.enter_context(
                    tc.tile_pool(name="ps_tr", bufs=2, space="PSUM"))
                ps_pl = ctx.enter_context(
                    tc.tile_pool(name="ps_pl", bufs=1, space="PSUM"))
                pool_ps = ps_pl.tile([H2, G_PER], F32, tag="pool")

            if lay == 0:
                stage = spool.tile([128, NT * 128], dt_out, tag="stg")
            elif lay == 1:
                stage = spool.tile([128, NT * 128], BF16, tag="stg")

            state = {"use_dve": False, "rot": 0}

            def flip():
                state["use_dve"] = not state["use_dve"]
                return state["use_dve"]

            def rot():
                state["rot"] = (state["rot"] + 1) % 3
                return state["rot"]

            def split_copy(dst, src_ps, w):
                """PSUM->SBUF copy split across Act | DVE halves."""
                h = (w // 2 + 63) & ~63 if w > 128 else w
                nc.scalar.activation(dst[:, 0:h], src_ps[:, 0:h],
                                     mybir.ActivationFunctionType.Copy)
                if h < w:
                    nc.vector.tensor_copy(dst[:, h:w], src_ps[:, h:w])

            def split_relu(dst, src_ps, w):
                h = (w // 2 + 63) & ~63 if w > 128 else w
                nc.scalar.activation(dst[:, 0:h], src_ps[:, 0:h],
                                     mybir.ActivationFunctionType.Relu)
                if h < w:
                    nc.vector.tensor_scalar_max(dst[:, h:w],
                                                src_ps[:, h:w], 0.0)

            def split_relu_bias(dst, src_ps, w, bias):
                h = (w // 2 + 63) & ~63 if w > 128 else w
                nc.scalar.activation(dst[:, 0:h], src_ps[:, 0:h],
                                     mybir.ActivationFunctionType.Relu,
                                     bias=bias)
                if h < w:
                    nc.vector.tensor_scalar(dst[:, h:w], src_ps[:, h:w],
                                            bias, 0.0,
                                            mybir.AluOpType.add,
                                            mybir.AluOpType.max)

            def phase1(pr, dup_sb, b0):
                """agg matmuls (+ L2: bias + relu straight from PSUM)."""
                if dup_sb is None:   # lay0 tiles pre-aggregated in stats
                    return pr, None, agp_s[:, pr[0] * 128:(pr[-1] + 1) * 128]
                pw = len(pr) * 128
                rows = H2 if lay == 2 else 128
                agg_ps = ps_agg.tile([rows, pw], F32, tag="agg")
                nc.tensor.matmul(agg_ps[:], zr_s[0:1, 0:rows],
                                 zr_s[0:1, 0:pw], start=True, stop=False,
                                 skip_group_check=True)
                nb_pair = sum(int(kt[t]) for t in pr)
                bi = 0
                for hi, t in enumerate(pr):
                    for b, (lo, w) in enumerate(blocks[t]):
                        gb = int(tile_base[t] // 128) + b
                        co = pan_cols[t][b]
                        bi += 1
                        nc.tensor.matmul(
                            agg_ps[:, hi * 128 + lo:hi * 128 + lo + w],
                            dup_sb[:, (gb - b0) * F:(gb - b0 + 1) * F],
                            pan_sb[:, PAN_OFF + co - PBASE:
                                   PAN_OFF + co - PBASE + w],
                            start=False, stop=(bi == nb_pair),
                            skip_group_check=True)
                if lay == 2:
                    hsT = wpool.tile([H2, pw], BF16, tag="hsT")
                    if flip():
                        nc.vector.tensor_scalar(
                            hsT[:], agg_ps[:], b_s, 0.0,
                            mybir.AluOpType.add, mybir.AluOpType.max)
                    else:
                        nc.scalar.activation(
                            hsT[:], agg_ps[:],
                            mybir.ActivationFunctionType.Relu,
                            bias=b_s)
                    return pr, agg_ps, hsT
                aggT = wpool.tile([128, pw], BF16, tag="aggT")
                if flip():
                    nc.vector.tensor_copy(aggT[:], agg_ps[:])
                else:
                    nc.scalar.activation(aggT[:], agg_ps[:],
                                         mybir.ActivationFunctionType.Copy)
                return pr, agg_ps, aggT

            def phase2(st1):
                pr, agg_ps, aggT = st1
                pw = len(pr) * 128
                if lay < 2:
                    h_ps = ps_out.tile([Ho, pw], F32, tag="hps")
                    c0 = pr[0] * 128
                    nc.tensor.matmul(h_ps[:], w_s[:] if lay == 0 else w_s,
                                     aggT[:], start=True, stop=False,
                                     skip_group_check=True)
                    nc.tensor.matmul(h_ps[:], b_s,
                                     sig_s[0:1, c0:c0 + pw],
                                     start=False, stop=(lay != 0),
                                     skip_group_check=True)
                    if lay == 0:
                        nc.tensor.matmul(h_ps[:], rw_s[:],
                                         sh_s[0:1, c0:c0 + pw],
                                         start=False, stop=True,
                                         skip_group_check=True)
                    so = pr[0] * 128
                    if flip():
                        nc.vector.tensor_scalar_max(
                            stage[:, so:so + pw], h_ps[:], 0.0)
                    else:
                        nc.scalar.activation(
                            stage[:, so:so + pw], h_ps[:],
                            mybir.ActivationFunctionType.Relu)
                    return st1
                # lay 2: transpose each tile's hsT: [64, 128] -> [128, 64]
                hsT = aggT
                tr_ps = ps_tr.tile([128, len(pr) * H2], BF16, tag="tr")
                for hi, t in enumerate(pr):
                    nc.tensor.transpose(tr_ps[:, hi * H2:(hi + 1) * H2],
                                        hsT[:, hi * 128:(hi + 1) * 128],
                                        id_s[0:64, 0:64])
                hs_sb = wpool.tile([128, len(pr) * H2], BF16, tag="hs")
                if flip():
                    nc.vector.tensor_copy(hs_sb[:], tr_ps[:])
                else:
                    nc.scalar.activation(
                        hs_sb[:], tr_ps[:],
                        mybir.ActivationFunctionType.Copy)
                return [(t, hs_sb, hi * H2) for hi, t in enumerate(pr)]

            def phase3(st2):
                if lay == 2:
                    flip()          # odd flips/group: engines alternate
                if lay == 1:
                    pr = st2[0]
                    pw = len(pr) * 128
                    so = pr[0] * 128
                    t_ps = ps_t.tile([H2, pw], F32, tag="tps")
                    nc.tensor.matmul(t_ps[:], w3_s, stage[:, so:so + pw],
                                     start=True, stop=True,
                                     skip_group_check=True)
                    if flip():
                        nc.vector.tensor_copy(tstage[:, so:so + pw], t_ps[:])
                    else:
                        nc.scalar.activation(
                            tstage[:, so:so + pw], t_ps[:],
                            mybir.ActivationFunctionType.Copy)
                elif lay == 2:
                    for t, hs_sb, off in st2:
                        state["npool"] = state.get("npool", 0) + 1
                        nc.tensor.matmul(pool_ps[:],
                                         hs_sb[:, off:off + H2],
                                         gpan_s[:, t * G_PER:(t + 1) * G_PER],
                                         start=(state["npool"] == 1),
                                         stop=(state["npool"] == NT),
                                         skip_group_check=True)

            all_pairs = []
            for ci, tiles in enumerate(chunk_tiles):
                dup_sb, b0 = pend.pop(0)
                if ci + 1 < len(chunk_tiles):
                    pend.append(chunk_loads(chunk_tiles[ci + 1]))
                GW = 4
                grps = [tiles[i:i + GW] for i in range(0, len(tiles), GW)]
                for g in reversed(grps):
                    all_pairs.append((g, dup_sb, b0))
            if lay == 0 and TPRE:
                # pre-aggregated thin tiles: compute-only, processed last
                pg = [list(range(i, min(i + 4, TPRE)))
                      for i in range(0, TPRE, 4)]
                for g in reversed(pg):
                    all_pairs.append((g, None, None))

            hastail = lay > 0
            q2, q3 = [], []
            out_stage = stage if lay == 0 else (tstage if lay == 1 else None)
            OW = 128 if lay == 0 else 64
            wb = [NT, 24, 8, 2, 0]
            WRITES = [(wb[i + 1], wb[i]) for i in range(len(wb) - 1)]

            def maybe_write(done_min):
                if lay == 2:
                    return
                while WRITES and done_min <= WRITES[0][0]:
                    wt0, wt1 = WRITES.pop(0)
                    q = nc.sync if wt0 == 0 else nc.gpsimd
                    q.dma_start(
                        out=h_out[:, wt0 * 128:wt1 * 128],
                        in_=out_stage[:, wt0 * 128:wt1 * 128])

            def run3():
                st3 = q3.pop(0)
                phase3(st3)
                done = st3[0][0] if lay == 1 else st3[0][0]
                maybe_write(done)

            def run2():
                st2 = phase2(q2.pop(0))
                if hastail:
                    q3.append(st2)
                else:
                    maybe_write(st2[0][0])

            LAG2 = 1
            LAG3 = 1
            for item in all_pairs:
                st1 = phase1(*item)
                if len(q3) >= LAG3:
                    run3()
                if len(q2) >= LAG2:
                    run2()
                q2.append(st1)
            while q2 or q3:
                if q3:
                    run3()
                if q2:
                    run2()

            # ---- classifier MLP on this core's G_PER graphs (lay 2)
            if lay == 2:
                p01 = wpool.tile([H2, G_PER], F32, tag="p01")
                nc.vector.tensor_copy(p01[:], pool_ps[:])
                y_ps = ps_pl.tile([H4, G_PER], F32, tag="yps")
                nc.tensor.matmul(y_ps[:], wc1_s, p01[:],
                                 start=True, stop=False,
                                 skip_group_check=True)
                nc.tensor.matmul(y_ps[:], bc1_r, cntx_r,
                                 start=False, stop=True,
                                 skip_group_check=True)
                y_s = wpool.tile([H4, G_PER], F32, tag="ys")
                nc.vector.tensor_scalar_max(y_s[:], y_ps[:], 0.0)
                o_ps = ps_pl.tile([G_PER, C], F32, tag="ops")
                nc.tensor.matmul(o_ps[:], y_s[:], wc2_s,
                                 start=True, stop=False,
                                 skip_group_check=True)
                nc.tensor.matmul(o_ps[:], cntx_r, bc2_r,
                                 start=False, stop=True,
                                 skip_group_check=True)
                o_s = wpool.tile([G_PER, C], F32, tag="os")
                nc.scalar.activation(o_s[:], o_ps[:],
                                     mybir.ActivationFunctionType.Copy,
                                     scale=invc_c)
                nc.sync.dma_start(out=out_d[:], in_=o_s[:])

    nc.compile()
    return nc


# ------------------------------------------------------------------ driver
_CACHE = {}


def _get_programs(meta):
    key = (tuple(meta["kt"]), meta["n_true"], meta["NT"])
    if key not in _CACHE:
        progs = [_build_stats_program(meta)]
        progs += [_build_layer_program(meta, lay) for lay in range(3)]
        _CACHE[key] = progs
    return _CACHE[key]


def run_gnn(runner=None, **inputs):
    F, H, H2, H4, C = 128, 128, 64, 32, 2
    x = np.asarray(inputs["x"], np.float32)
    n_true = x.shape[0]
    src = np.asarray(inputs["edge_index"][0], np.int64)
    dst = np.asarray(inputs["edge_index"][1], np.int64)
    batch = np.asarray(inputs["batch"], np.int64)

    meta = _plan(src, dst, batch, n_true)
    NT, SHARD, NPAD = meta["NT"], meta["SHARD"], meta["NPAD"]
    cores = _build_static(meta, src, dst, batch)
    order = meta["order"]
    progs = _get_programs(meta)

    def run(nc, in_maps):
        if runner is not None:
            return runner(nc, in_maps)
        return run_bass_kernel_spmd(
            nc, in_maps, core_ids=list(range(N_CORES))).results

    x_new = np.zeros((NPAD + 1, F), np.float32)
    x_new[:NPAD][order < n_true] = x[order[order < n_true]]

    # ---- stats launch (BN partials + L0 pre-agg of tiles 0..TPRE-1)
    xb = x_new[:NPAD].astype(NPFP8)
    l0_dups = [_dup_layout(x_new, cores[c]["slotsrc"], DUP_NP[0])
               for c in range(N_CORES)]
    stats_maps = []
    for c in range(N_CORES):
        idx = ((np.arange(NT) * N_CORES + c)[:, None] * 128
               + np.arange(128)[None, :])
        slab = xb[idx]
        slab = np.ascontiguousarray(slab.transpose(1, 0, 2)).reshape(
            128, NT * F)
        stats_maps.append({
            "x_sh": slab, "ident": np.eye(128, dtype=np.float32)})
    res = run(progs[0], stats_maps)
    parts = np.stack([np.asarray(res[c]["stat_part"])
                      for c in range(N_CORES)], axis=2)
    sx_parts = np.ascontiguousarray(parts[:, 0, :], dtype=np.float32)
    ex2_parts = np.ascontiguousarray(parts[:, 1, :], dtype=np.float32)

    W = [np.asarray(inputs["W1"], np.float32),
         np.asarray(inputs["W2"], np.float32),
         np.asarray(inputs["W3"], np.float32)]
    brows = [np.asarray(inputs["b1"], np.float32).reshape(1, H),
             np.asarray(inputs["b2"], np.float32).reshape(1, H),
             np.asarray(inputs["b3"], np.float32).reshape(1, H2)]

    h_new = x_new
    core_out = None
    for lay in range(3):
        maps = []
        for c in range(N_CORES):
            st = cores[c]
            if lay == 0:
                rp = np.concatenate([st["sig_row"], st["sh_row"],
                                     brows[0].ravel()])
            elif lay == 1:
                rp = np.concatenate([st["sig_row"], brows[1].ravel()])
            else:
                rp = np.zeros((128, 65), np.float64)
                rp[0:64, 0:64] = np.eye(64)
            if lay == 1:
                pan = st["pans"][1]
            elif lay == 2:
                pan = np.concatenate([st["gpan"].astype(NPFP8),
                                      st["pans"][2]], axis=1)
            else:
                pan = st["pans"][0]
            m = {"dup": l0_dups[c] if lay == 0 else
                 _dup_layout(h_new, st["slotsrc"], DUP_NP[lay]),
                 "pan": np.ascontiguousarray(pan),
                 "rowpack": (rp.astype(NPBF16).reshape(1, -1) if lay < 2
                             else np.ascontiguousarray(rp.astype(NPBF16)))}
            if lay == 1:
                m["wpack"] = np.ascontiguousarray(np.concatenate(
                    [(W[1] / 8.0).astype(NPBF16), W[2].astype(NPBF16)],
                    axis=1))
            if lay == 0:
                fp = np.zeros((128, 18 + H), np.float32)
                fp[:, 0:8] = sx_parts
                fp[:, 8:16] = ex2_parts
                fp[:, 16] = np.asarray(inputs["bn_gamma"], np.float32)
                fp[:, 17] = np.asarray(inputs["bn_beta"], np.float32)
                fp[:, 18:] = W[0]
                m["f32pack"] = fp
            if lay == 2:
                mp = np.zeros((64, 80), np.float32)
                mp[:, 0:H4] = np.asarray(inputs["Wc1"], np.float32)
                mp[0:H4, H4:H4 + C] = np.asarray(inputs["Wc2"], np.float32)
                mp[0, 34:66] = np.asarray(inputs["bc1"], np.float32)
                mp[0, 66:74] = st["cntx"] * 4.0
                mp[0, 74:76] = np.asarray(inputs["bc2"], np.float32)
                mp[0:G_PER, 76] = st["invc"] / 4.0
                mp[0:64, 78] = np.asarray(inputs["b3"], np.float32) * 4.0
                m["mpack"] = mp
            maps.append(m)
        res = run(progs[1 + lay], maps)
        if lay < 2:
            OW = 128 if lay == 0 else 64
            h_new = np.zeros((NPAD + 1, OW), np.float32)
            for c in range(N_CORES):
                ho = np.asarray(res[c]["h_out"])
                hoT = ho.reshape(OW, NT, 128).transpose(1, 2, 0)
                idx = ((np.arange(NT) * N_CORES + c)[:, None] * 128
                       + np.arange(128)[None, :])
                h_new[idx] = hoT
        else:
            core_out = [np.asarray(res[c]["out"]) for c in range(N_CORES)]

    out = np.zeros((G, C), np.float32)
    for c in range(N_CORES):
        for lg, g in enumerate(meta["core_graphs"][c]):
            out[g] = core_out[c][lg]
    return out


def kernel(**inputs):
    return run_gnn(**inputs)


# revision 37
# speedup vs baseline: 1.0103x; 1.0028x over previous
"""Trainium2 Bass kernel for AudioOnlyGNN (3-layer GCN + BatchNorm + mean-pool + MLP).

v3 — graph-partitioned static slot stream:

Nodes are assigned to cores by *graph* ownership (8 graphs per core,
balanced by node count), then degree-sorted within each core and laid out in
128-row tiles; tile t's slot budget k_t = max in-degree(+self) over that tile
across all cores, giving a static slot stream identical on every core.  For
each layer the host materialises the edge-source rows in slot order (a pure
gather) so the device reads large contiguous DMA blocks.

On device, a 128-slot block contributes to a [F, ncols] PSUM tile via one
matmul whose moving operand is a narrow "panel" (slot -> dst column weight
with the GCN normalisation baked in).  The aggregate is transformed
(W^T @ agg), bias/BN-shift added as rank-1 matmuls, ReLU'd, written back.
Layers 0/1 write h' = dinv*ReLU(...) so panels never depend on h.

Because every graph lives entirely on one core, the mean-pool and classifier
MLP complete locally inside the L2 launch (no cross-core reduction): launches
are [stats+pre-agg] [L0] [L1] [L2+pool+mlp].  Between launches the host only
reorders bytes (gather / transpose), never does arithmetic on activations.
"""

import sys

sys.path.insert(0, "/opt/trn_rl_repo")

import contextlib

import numpy as np
import ml_dtypes

import concourse.bacc as bacc
import concourse.bass as bass
import concourse.mybir as mybir
from concourse.tile import TileContext
from concourse.bass_utils import run_bass_kernel_spmd

BF16 = mybir.dt.bfloat16
F32 = mybir.dt.float32
FP8 = mybir.dt.float8e3  # e3m4

NPBF16 = ml_dtypes.bfloat16
NPFP8 = ml_dtypes.float8_e3m4

N_CORES = 8
BN_EPS = 1e-5
G = 64
G_PER = G // N_CORES   # graphs per core
TPRE = 0               # tiles of L0 pre-aggregated inside the stats launch

# dtype of the host-expanded per-slot source rows, per layer
DUP_DT = [FP8, FP8, FP8]
DUP_NP = [NPFP8, NPFP8, NPFP8]
OUT_DT = [FP8, FP8]
OUT_NP = [NPFP8, NPFP8]


def _chunk_list(n0, n1, lead, mid, tail=(4, 2, 1)):
    """Chunk [n0, n1) into sizes lead + [mid...] + tail (tapered ends)."""
    n = n1 - n0
    sizes = []
    for s in lead:
        if sum(sizes) + s > n:
            break
        sizes.append(s)
    tl = [s for s in tail if s < mid]
    while sum(sizes) + sum(tl) + mid <= n:
        sizes.append(mid)
    rem = n - sum(sizes) - sum(tl)
    while rem > 0:
        add = min(rem, mid)
        sizes.append(add)
        rem -= add
    sizes += tl
    sizes = [s for s in sizes if s > 0]
    # clip overflow
    while sum(sizes) > n:
        sizes[-1] -= sum(sizes) - n
        sizes = [s for s in sizes if s > 0]
    out = []
    t = n0
    for cs in sizes:
        out.append(list(range(t, t + cs)))
        t += cs
    assert t == n1, (sizes, n0, n1)
    return out


# ------------------------------------------------------------------ planning
def _plan(src, dst, batch, n_true):
    """Static (h-independent) structure: graph packing, renumbering, slots."""
    cnt_g = np.bincount(batch, minlength=G).astype(np.int64)
    g_order = np.argsort(-cnt_g, kind="stable")
    core_graphs = [[] for _ in range(N_CORES)]
    loads = np.zeros(N_CORES, np.int64)
    for g in g_order:
        cand = [i for i in range(N_CORES) if len(core_graphs[i]) < G_PER]
        i = min(cand, key=lambda i: loads[i])
        core_graphs[i].append(int(g))
        loads[i] += cnt_g[g]
    NT = max(49, int(-(-loads.max() // 128)))
    SHARD = NT * 128
    NPAD = N_CORES * SHARD

    graph_core = np.zeros(G, np.int64)
    graph_local = np.zeros(G, np.int64)
    for c in range(N_CORES):
        for lg, g in enumerate(core_graphs[c]):
            graph_core[g] = c
            graph_local[g] = lg

    degp_true = np.bincount(dst, minlength=n_true).astype(np.int64) + 1
    node_core = graph_core[batch]

    order = np.empty(NPAD, np.int64)
    virt = n_true
    for c in range(N_CORES):
        nodes_c = np.where(node_core == c)[0]
        nodes_c = nodes_c[np.argsort(degp_true[nodes_c], kind="stable")]
        npadc = SHARD - len(nodes_c)
        ids = np.concatenate([np.arange(virt, virt + npadc), nodes_c])
        virt += npadc
        idx = ((np.arange(NT) * N_CORES + c)[:, None] * 128
               + np.arange(128)[None, :])
        order[idx.ravel()] = ids
    assert virt == NPAD
    newpos = np.empty(NPAD, np.int64)
    newpos[order] = np.arange(NPAD)

    degp = np.zeros(NPAD, np.int64)
    degp[:n_true] = degp_true

    kt = np.zeros(NT, np.int64)
    for t in range(NT):
        kt[t] = degp[order[t * 1024:(t + 1) * 1024]].max()
    kt = np.maximum(kt, 1)

    blocks = []   # per tile: list of (lo, w)
    pan_cols = []  # per tile: list of panel col offsets
    wtot = 0
    for t in range(NT):
        k = int(kt[t])
        bl = []
        for b in range(k):
            lo = (128 * b) // k
            hi = (128 * (b + 1) - 1) // k
            bl.append((lo, hi - lo + 1))
        blocks.append(bl)
        offs = []
        for lo, w in bl:
            offs.append(wtot)
            wtot += w
        pan_cols.append(offs)

    nblk = int(kt.sum())
    tile_base = np.zeros(NT + 1, np.int64)
    tile_base[1:] = np.cumsum(128 * kt)
    meta = {"kt": kt, "blocks": blocks, "pan_cols": pan_cols,
            "wtot": wtot, "nblk": nblk, "order": order, "newpos": newpos,
            "n_true": n_true, "tile_base": tile_base,
            "total_slots": int(tile_base[-1]),
            "NT": NT, "SHARD": SHARD, "NPAD": NPAD,
            "core_graphs": core_graphs, "graph_core": graph_core,
            "graph_local": graph_local, "cnt_g": cnt_g}
    return meta


def _build_static(meta, src, dst, batch):
    """Per-core constant tables: slot->src map, per-layer panels, rows."""
    kt, blocks, pan_cols = meta["kt"], meta["blocks"], meta["pan_cols"]
    wtot, nblk, order, newpos = (meta["wtot"], meta["nblk"], meta["order"],
                                 meta["newpos"])
    n_true = meta["n_true"]
    NT, SHARD, NPAD = meta["NT"], meta["SHARD"], meta["NPAD"]
    graph_local, cnt_g = meta["graph_local"], meta["cnt_g"]

    deg = np.bincount(dst, minlength=NPAD).astype(np.float64) + 1.0
    dinv = (1.0 / np.sqrt(deg)).astype(np.float64)
    dinv_pad = dinv.copy()
    dinv_pad[n_true:] = 1.0

    dinv_new = dinv_pad[order]
    batch_pad = np.full(NPAD, 0, np.int64)
    batch_pad[:n_true] = batch
    batch_new = batch_pad[order]
    valid_new = (order < n_true)

    sneig = np.bincount(dst, weights=dinv[src], minlength=NPAD)
    d2 = dinv_pad * (sneig + dinv_pad)
    d2_new = d2[order]

    cntx = np.maximum(cnt_g.astype(np.float64), 1.0)   # [G]
    invc = 1.0 / cntx

    s_new = newpos[src]
    d_new = newpos[dst]
    g_tile = d_new // 128
    core_of = g_tile % N_CORES
    tloc = g_tile // N_CORES
    dloc = d_new % 128

    tile_base = meta["tile_base"]
    total_slots = meta["total_slots"]

    edge_w0 = dinv[src] * dinv_pad[dst] * dinv_pad[dst]

    cores = []
    for c in range(N_CORES):
        sel = core_of == c
        es, et, ed = s_new[sel], tloc[sel], dloc[sel]
        ew0 = edge_w0[sel]
        key = et * (128 * 64) + ed
        o = np.argsort(key, kind="stable")
        es, et, ed, ew0 = es[o], et[o], ed[o], ew0[o]
        k_of = kt[et]
        node_key = et * 128 + ed
        uniq, first_idx, counts = np.unique(node_key, return_index=True,
                                            return_counts=True)
        rank = np.arange(len(node_key)) - np.repeat(first_idx, counts)
        slot = tile_base[et] + ed * k_of + 1 + rank   # +1: self slot at 0

        tt = np.arange(NT).repeat(128)
        dd = np.tile(np.arange(128), NT)
        own_new = (tt * N_CORES + np.full(NT * 128, c)) * 128 + dd
        own_valid = valid_new[own_new]
        self_slot = tile_base[tt] + dd * kt[tt]

        slotsrc = np.full(total_slots, NPAD, np.int64)  # NPAD -> zero row
        slotsrc[slot] = es
        slotsrc[self_slot[own_valid]] = own_new[own_valid]

        dv_own = dinv_new[own_new]
        w_l0 = np.zeros(total_slots, np.float64)
        w_l0[slot] = ew0
        w_l0[self_slot[own_valid]] = (dv_own ** 3)[own_valid]
        col_dinv = np.repeat(dv_own, np.repeat(kt, 128))
        filled = np.zeros(total_slots, bool)
        filled[slot] = True
        filled[self_slot[own_valid]] = True
        w_l1 = np.where(filled, col_dinv ** 2, 0.0)
        w_l2 = np.where(filled, col_dinv, 0.0)

        pans = []
        for wv, psc in ((w_l0, 8.0), (w_l1, 8.0), (w_l2, 4.0)):
            pan = np.zeros((128, wtot), np.float64)
            for t in range(NT):
                k = int(kt[t])
                for b, (lo, w) in enumerate(blocks[t]):
                    co = pan_cols[t][b]
                    sl0 = tile_base[t] + b * 128
                    ss = np.arange(sl0, sl0 + 128)
                    cc = (ss - tile_base[t]) // k - lo
                    ok = (cc >= 0) & (cc < w)
                    pan[np.arange(128)[ok], co + cc[ok]] = wv[ss][ok]
            pans.append((pan * psc).astype(NPFP8))

        sig_row = np.zeros(SHARD, np.float64)
        sh_row = np.zeros(SHARD, np.float64)
        for t in range(NT):
            cols = slice(t * 128, (t + 1) * 128)
            nn = (t * N_CORES + c) * 128 + np.arange(128)
            sig_row[cols] = dinv_new[nn]
            sh_row[cols] = d2_new[nn] * dinv_new[nn]

        # pool panel [128, NT*G_PER]: 1.0 at (d, t*G_PER + local_graph)
        gpan = np.zeros((128, NT * G_PER), np.float64)
        for t in range(NT):
            nn = (t * N_CORES + c) * 128 + np.arange(128)
            gb = graph_local[batch_new[nn]]
            ok = valid_new[nn]
            gpan[np.arange(128)[ok], t * G_PER + gb[ok]] = 1.0

        cg = meta["core_graphs"][c]
        cores.append({
            "slotsrc": slotsrc,
            "pans": pans,
            "sig_row": sig_row,
            "sh_row": sh_row,
            "gpan": gpan.astype(NPBF16),
            "cntx": cntx[cg].astype(np.float32),     # [G_PER]
            "invc": invc[cg].astype(np.float32),     # [G_PER]
        })
    return cores


def _dup_layout(h_new, slotsrc, np_dt):
    """[NPAD(+1), F] new-indexed rows -> [128, NBLK*F] slot-stream layout."""
    rows = h_new[slotsrc]
    nblk = rows.shape[0] // 128
    F = rows.shape[1]
    return np.ascontiguousarray(
        rows.reshape(nblk, 128, F).transpose(1, 0, 2)
    ).reshape(128, nblk * F).astype(np_dt)


# ------------------------------------------------------------------ programs
def _build_stats_program(meta):
    """Per-core BN partial sums (Sum x, Sum x^2 over own nodes)."""
    F = 128
    NT = meta["NT"]
    nc = bacc.Bacc("TRN2", target_bir_lowering=False, debug=False,
                   num_devices=N_CORES)
    xs_d = nc.dram_tensor("x_sh", [128, NT * F], FP8,
                          kind="ExternalInput").ap()
    ident_d = nc.dram_tensor("ident", [128, 128], F32,
                             kind="ExternalInput").ap()
    out_d = nc.dram_tensor("stat_part", [128, 2], F32,
                           kind="ExternalOutput").ap()
    XS = [0, 12, 24, 36, 45, NT]
    with TileContext(nc) as tc:
        with tc.tile_pool(name="w", bufs=1) as wp, \
             tc.tile_pool(name="ps", bufs=1, space="PSUM") as pp:
            xs = wp.tile([128, NT * F], FP8, tag="xs")
            ident_s = wp.tile([128, 128], F32, tag="id")
            nc.sync.dma_start(out=xs[:, :XS[1] * F], in_=xs_d[:, :XS[1] * F])
            nc.scalar.dma_start(out=ident_s[:], in_=ident_d[:])
            for q in range(1, len(XS) - 1):
                nc.sync.dma_start(out=xs[:, XS[q] * F:XS[q + 1] * F],
                                  in_=xs_d[:, XS[q] * F:XS[q + 1] * F])
            ones_s = wp.tile([128, 1], FP8, tag="ones")
            nc.vector.memset(ones_s[:], 1.0)
            xtx_ps = pp.tile([128, 128], F32, tag="xtx")
            sx_ps = pp.tile([128, 1], F32, tag="sx")
            for t in range(NT):
                sl = xs[:, t * F:(t + 1) * F]
                nc.tensor.matmul(xtx_ps[:], sl, sl, start=(t == 0),
                                 stop=(t == NT - 1), skip_group_check=True)
                nc.tensor.matmul(sx_ps[:], sl, ones_s[:],
                                 start=(t == 0), stop=(t == NT - 1),
                                 skip_group_check=True)
            dg = wp.tile([128, 128], F32, tag="dg")
            nc.vector.tensor_tensor(dg[:], xtx_ps[:], ident_s[:],
                                    mybir.AluOpType.mult)
            o = wp.tile([128, 2], F32, tag="o")
            nc.vector.tensor_reduce(o[:, 1:2], dg[:], mybir.AxisListType.X,
                                    mybir.AluOpType.add)
            nc.vector.tensor_copy(o[:, 0:1], sx_ps[:])
            nc.scalar.dma_start(out=out_d[:], in_=o[:])
    nc.compile()
    return nc


def _build_layer_program(meta, lay):
    kt, blocks, pan_cols, wtot, nblk, tile_base = (
        meta["kt"], meta["blocks"], meta["pan_cols"], meta["wtot"],
        meta["nblk"], meta["tile_base"])
    NT, SHARD = meta["NT"], meta["SHARD"]
    F = 128 if lay < 2 else 64
    H = 128
    H2 = 64
    H4 = 32
    C = 2
    Ho = H if lay < 2 else H2
    N_true = meta["n_true"]
    dt_in = DUP_DT[lay]
    dt_out = OUT_DT[lay] if lay < 2 else None

    nc = bacc.Bacc("TRN2", target_bir_lowering=False, debug=False,
                   num_devices=N_CORES)

    def din(name, shape, dt):
        return nc.dram_tensor(name, list(shape), dt, kind="ExternalInput").ap()

    dup_d = din("dup", [128, nblk * F], dt_in)
    if lay == 2:
        PW_EXTRA = NT * G_PER       # gpan (0/1: fp8-exact)
    else:
        PW_EXTRA = 0               # W1 in f32pack; W2|W3 in wpack
    pan_d = din("pan", [128, wtot + PW_EXTRA], FP8)
    if lay == 1:
        wp_d = din("wpack", [128, H + H2], BF16)
    # packed bf16 row constants
    if lay == 0:
        RP = 2 * SHARD + H        # sig | sh | b1
    elif lay == 1:
        RP = SHARD + H            # sig | b2
    else:
        RP = 1                    # b3 as a column
    rp_d = din("rowpack", [1, RP] if lay < 2 else [128, 65], BF16)
    if lay == 0:
        # sxp | exp | gamma | beta | W1(fp32)
        fp_d = din("f32pack", [128, 18 + H], F32)
        if TPRE:
            agp_d = din("aggT_pre", [128, TPRE * 128], BF16)
    if lay == 2:
        # mlp pack: Wc1 | Wc2 | bc1row | cntx | bc2 | invc  (f32)
        mp_d = din("mpack", [64, 80], F32)
        out_d = nc.dram_tensor("out", [G_PER, C], F32,
                               kind="ExternalOutput").ap()
    else:
        OW = 128 if lay == 0 else 64
        h_out = nc.dram_tensor("h_out", [OW, NT * 128], dt_out,
                               kind="ExternalOutput").ap()

    # process tiles high->low: degree sorting puts fat tiles at high
    # indices, so the tail (last chunk + final write) covers thin tiles.
    T0 = TPRE if lay == 0 else 0
    fwd = _chunk_list(T0, NT, [2, 2, 4], 8, tail=(4, 2))
    chunk_tiles = []
    hi = NT
    for ch in fwd:
        chunk_tiles.append(list(range(hi - len(ch), hi)))
        hi -= len(ch)
    assert hi == T0
    PBASE = pan_cols[TPRE][0] if lay == 0 else 0

    with TileContext(nc) as tc:
        with contextlib.ExitStack() as ctx:
            cpool = ctx.enter_context(tc.tile_pool(name="const", bufs=1))
            dpool = ctx.enter_context(tc.tile_pool(name="dup", bufs=5))
            ppool = ctx.enter_context(tc.tile_pool(name="pan", bufs=2))

            def chunk_loads(tiles):
                ct0, ct1 = tiles[0], tiles[-1] + 1
                b0 = int(tile_base[ct0] // 128)
                b1 = int(tile_base[ct1] // 128)
                dup_sb = dpool.tile([128, (b1 - b0) * F], dt_in, tag="dup")
                nc.sync.dma_start(out=dup_sb[:], in_=dup_d[:, b0 * F:b1 * F])
                return dup_sb, b0

            pend = [chunk_loads(chunk_tiles[0])]
            pan_sb = ppool.tile([128, wtot - PBASE + PW_EXTRA], FP8,
                                tag="pan")
            if lay == 1:
                wpk_s = cpool.tile([128, H + H2], BF16, tag="c_wpk")
                nc.scalar.dma_start(out=wpk_s[:], in_=wp_d[:])
            fst = NT - 12
            PAN_OFF = PW_EXTRA
            PSPLIT = PAN_OFF + pan_cols[fst][0] - PBASE
            nc.sync.dma_start(out=pan_sb[:, PSPLIT:],
                              in_=pan_d[:, PBASE + PSPLIT:])
            if PW_EXTRA:
                nc.scalar.dma_start(out=pan_sb[:, :PW_EXTRA],
                                    in_=pan_d[:, PBASE:PBASE + PW_EXTRA])

            rp_s = cpool.tile([1, RP] if lay < 2 else [128, 65], BF16,
                              tag="c_rp")
            (nc.scalar if lay == 0 else nc.sync).dma_start(
                out=rp_s[:], in_=rp_d[:])
            if lay == 0:
                fp_s = cpool.tile([128, 18 + H], F32, tag="c_fp")
                nc.scalar.dma_start(out=fp_s[:], in_=fp_d[:])
                if TPRE:
                    agp_s = cpool.tile([128, TPRE * 128], BF16, tag="c_agp")
                    nc.scalar.dma_start(out=agp_s[:], in_=agp_d[:])
            if lay == 2:
                mp_s = cpool.tile([64, 80], F32, tag="c_mp")
                nc.scalar.dma_start(out=mp_s[:], in_=mp_d[:])
            nc.sync.dma_start(out=pan_sb[:, PAN_OFF:PSPLIT],
                              in_=pan_d[:, PBASE + PAN_OFF:PBASE + PSPLIT])
            if lay == 0:
                sig_s = rp_s[0:1, 0:SHARD]
                sh_s = rp_s[0:1, SHARD:2 * SHARD]
                b_s = rp_s[0:1, 2 * SHARD:2 * SHARD + H]
            elif lay == 1:
                sig_s = rp_s[0:1, 0:SHARD]
                b_s = rp_s[0:1, SHARD:SHARD + H]
            else:
                b_s = mp_s[0:H2, 78:79]   # [H2, 1] f32 column
            zr_s = cpool.tile([1, 512], BF16, tag="c_zr")
            nc.vector.memset(zr_s[:], 0.0)
            if lay == 0:
                w1f_s = fp_s[:, 18:18 + H]
                w_s = cpool.tile([128, H], BF16, tag="c_wt")
                rw_s = cpool.tile([1, H], BF16, tag="c_rw")
            elif lay == 1:
                w_s = wpk_s[:, 0:H]
                w3_s = wpk_s[:, H:H + H2]
            else:
                gpan_s = pan_sb[:, 0:NT * G_PER]
                id_s = rp_s[:, 0:64]
                wc1_s = mp_s[:, 0:H4]                  # [64, 32]
                wc2_s = mp_s[0:H4, H4:H4 + C]          # [32, 2]
                bc1_r = mp_s[0:1, 34:66]               # [1, 32]
                cntx_r = mp_s[0:1, 66:74]              # [1, 8]
                bc2_r = mp_s[0:1, 74:76]               # [1, 2]
                invc_c = mp_s[0:G_PER, 76:77]          # [8, 1]

            # ---- BN statistics (layer 0) -> W~1 and shift row rw
            if lay == 0:
                with tc.tile_pool(name="ps_st", bufs=1, space="PSUM") as pst, \
                     tc.tile_pool(name="st_w", bufs=2) as stw:
                    sxp_s = fp_s[:, 0:8]
                    exp_s = fp_s[:, 8:16]
                    gam_s = fp_s[:, 16:17]
                    bet_s = fp_s[:, 17:18]
                    ex2 = stw.tile([128, 1], F32, tag="v1")
                    nc.vector.tensor_reduce(ex2[:], exp_s,
                                            mybir.AxisListType.X,
                                            mybir.AluOpType.add)
                    sx = stw.tile([128, 1], F32, tag="v0")
                    nc.vector.tensor_reduce(sx[:], sxp_s,
                                            mybir.AxisListType.X,
                                            mybir.AluOpType.add)
                    mu = stw.tile([128, 1], F32, tag="v2")
                    nc.vector.tensor_scalar_mul(mu[:], sx[:], 1.0 / N_true)
                    var = stw.tile([128, 1], F32, tag="v3")
                    nc.vector.tensor_scalar_mul(var[:], ex2[:], 1.0 / N_true)
                    mu2 = stw.tile([128, 1], F32, tag="v4")
                    nc.vector.tensor_tensor(mu2[:], mu[:], mu[:],
                                            mybir.AluOpType.mult)
                    nc.vector.tensor_tensor(var[:], var[:], mu2[:],
                                            mybir.AluOpType.subtract)
                    nc.vector.tensor_scalar_add(var[:], var[:], BN_EPS)
                    rec = stw.tile([128, 1], F32, tag="v5")
                    nc.vector.reciprocal(rec[:], var[:])
                    isd = stw.tile([128, 1], F32, tag="v6")
                    nc.scalar.activation(isd[:], rec[:],
                                         mybir.ActivationFunctionType.Sqrt)
                    a_c = stw.tile([128, 1], F32, tag="v7")
                    nc.vector.tensor_tensor(a_c[:], gam_s, isd[:],
                                            mybir.AluOpType.mult)
                    a8 = stw.tile([128, 1], F32, tag="v9")
                    nc.vector.tensor_scalar_mul(a8[:], a_c[:], 0.125)
                    nc.vector.tensor_scalar_mul(w_s[:], w1f_s, a8[:])
                    ca = stw.tile([128, 1], F32, tag="v8")
                    nc.vector.tensor_tensor(ca[:], mu[:], a_c[:],
                                            mybir.AluOpType.mult)
                    nc.vector.tensor_tensor(ca[:], bet_s, ca[:],
                                            mybir.AluOpType.subtract)
                    rw_ps = pst.tile([1, H], F32, tag="rw")
                    nc.tensor.matmul(rw_ps[:], ca[:], w1f_s,
                                     start=True, stop=True)
                    nc.scalar.activation(rw_s[:], rw_ps[:],
                                         mybir.ActivationFunctionType.Copy)

            spool = ctx.enter_context(tc.tile_pool(name="stg", bufs=1))
            wpool = ctx.enter_context(tc.tile_pool(name="wk", bufs=4))
            ps_agg = ctx.enter_context(
                tc.tile_pool(name="ps_agg", bufs=3, space="PSUM"))
            if lay < 2:
                ps_out = ctx.enter_context(
                    tc.tile_pool(name="ps_out", bufs=3, space="PSUM"))
            if lay == 1:
                ps_t = ctx.enter_context(
                    tc.tile_pool(name="ps_t", bufs=2, space="PSUM"))
                tstage = spool.tile([64, NT * 128], dt_out, tag="tstg")
            if lay == 2:
                ps_tr = ctx.enter_context(
                    tc.tile_pool(name="ps_tr", bufs=2, space="PSUM"))
                ps_pl = ctx.enter_context(
                    tc.tile_pool(name="ps_pl", bufs=1, space="PSUM"))
                pool_ps = ps_pl.tile([H2, G_PER], F32, tag="pool")

            if lay == 0:
                stage = spool.tile([128, NT * 128], dt_out, tag="stg")
            elif lay == 1:
                stage = spool.tile([128, NT * 128], BF16, tag="stg")

            state = {"use_dve": False, "rot": 0}

            def flip():
                state["use_dve"] = not state["use_dve"]
                return state["use_dve"]

            def rot():
                state["rot"] = (state["rot"] + 1) % 3
                return state["rot"]

            def split_copy(dst, src_ps, w):
                """PSUM->SBUF copy split across Act | DVE halves."""
                h = (w // 2 + 63) & ~63 if w > 128 else w
                nc.scalar.activation(dst[:, 0:h], src_ps[:, 0:h],
                                     mybir.ActivationFunctionType.Copy)
                if h < w:
                    nc.vector.tensor_copy(dst[:, h:w], src_ps[:, h:w])

            def split_relu(dst, src_ps, w):
                h = (w // 2 + 63) & ~63 if w > 128 else w
                nc.scalar.activation(dst[:, 0:h], src_ps[:, 0:h],
                                     mybir.ActivationFunctionType.Relu)
                if h < w:
                    nc.vector.tensor_scalar_max(dst[:, h:w],
                                                src_ps[:, h:w], 0.0)

            def split_relu_bias(dst, src_ps, w, bias):
                h = (w // 2 + 63) & ~63 if w > 128 else w
                nc.scalar.activation(dst[:, 0:h], src_ps[:, 0:h],
                                     mybir.ActivationFunctionType.Relu,
                                     bias=bias)
                if h < w:
                    nc.vector.tensor_scalar(dst[:, h:w], src_ps[:, h:w],
                                            bias, 0.0,
                                            mybir.AluOpType.add,
                                            mybir.AluOpType.max)

            def phase1(pr, dup_sb, b0):
                """agg matmuls (+ L2: bias + relu straight from PSUM)."""
                if dup_sb is None:   # lay0 tiles pre-aggregated in stats
                    return pr, None, agp_s[:, pr[0] * 128:(pr[-1] + 1) * 128]
                pw = len(pr) * 128
                rows = H2 if lay == 2 else 128
                agg_ps = ps_agg.tile([rows, pw], F32, tag="agg")
                nc.tensor.matmul(agg_ps[:], zr_s[0:1, 0:rows],
                                 zr_s[0:1, 0:pw], start=True, stop=False,
                                 skip_group_check=True)
                nb_pair = sum(int(kt[t]) for t in pr)
                bi = 0
                for hi, t in enumerate(pr):
                    for b, (lo, w) in enumerate(blocks[t]):
                        gb = int(tile_base[t] // 128) + b
                        co = pan_cols[t][b]
                        bi += 1
                        nc.tensor.matmul(
                            agg_ps[:, hi * 128 + lo:hi * 128 + lo + w],
                            dup_sb[:, (gb - b0) * F:(gb - b0 + 1) * F],
                            pan_sb[:, PAN_OFF + co - PBASE:
                                   PAN_OFF + co - PBASE + w],
                            start=False, stop=(bi == nb_pair),
                            skip_group_check=True)
                if lay == 2:
                    hsT = wpool.tile([H2, pw], BF16, tag="hsT")
                    if flip():
                        nc.vector.tensor_scalar(
                            hsT[:], agg_ps[:], b_s, 0.0,
                            mybir.AluOpType.add, mybir.AluOpType.max)
                    else:
                        nc.scalar.activation(
                            hsT[:], agg_ps[:],
                            mybir.ActivationFunctionType.Relu,
                            bias=b_s)
                    return pr, agg_ps, hsT
                aggT = wpool.tile([128, pw], BF16, tag="aggT")
                if flip():
                    nc.vector.tensor_copy(aggT[:], agg_ps[:])
                else:
                    nc.scalar.activation(aggT[:], agg_ps[:],
                                         mybir.ActivationFunctionType.Copy)
                return pr, agg_ps, aggT

            def phase2(st1):
                pr, agg_ps, aggT = st1
                pw = len(pr) * 128
                if lay < 2:
                    h_ps = ps_out.tile([Ho, pw], F32, tag="hps")
                    c0 = pr[0] * 128
                    nc.tensor.matmul(h_ps[:], w_s[:] if lay == 0 else w_s,
                                     aggT[:], start=True, stop=False,
                                     skip_group_check=True)
                    nc.tensor.matmul(h_ps[:], b_s,
                                     sig_s[0:1, c0:c0 + pw],
                                     start=False, stop=(lay != 0),
                                     skip_group_check=True)
                    if lay == 0:
                        nc.tensor.matmul(h_ps[:], rw_s[:],
                                         sh_s[0:1, c0:c0 + pw],
                                         start=False, stop=True,
                                         skip_group_check=True)
                    so = pr[0] * 128
                    if flip():
                        nc.vector.tensor_scalar_max(
                            stage[:, so:so + pw], h_ps[:], 0.0)
                    else:
                        nc.scalar.activation(
                            stage[:, so:so + pw], h_ps[:],
                            mybir.ActivationFunctionType.Relu)
                    return st1
                # lay 2: transpose each tile's hsT: [64, 128] -> [128, 64]
                hsT = aggT
                tr_ps = ps_tr.tile([128, len(pr) * H2], BF16, tag="tr")
                for hi, t in enumerate(pr):
                    nc.tensor.transpose(tr_ps[:, hi * H2:(hi + 1) * H2],
                                        hsT[:, hi * 128:(hi + 1) * 128],
                                        id_s[0:64, 0:64])
                hs_sb = wpool.tile([128, len(pr) * H2], BF16, tag="hs")
                if flip():
                    nc.vector.tensor_copy(hs_sb[:], tr_ps[:])
                else:
                    nc.scalar.activation(
                        hs_sb[:], tr_ps[:],
                        mybir.ActivationFunctionType.Copy)
                return [(t, hs_sb, hi * H2) for hi, t in enumerate(pr)]

            def phase3(st2):
                if lay == 2:
                    flip()          # odd flips/group: engines alternate
                if lay == 1:
                    pr = st2[0]
                    pw = len(pr) * 128
                    so = pr[0] * 128
                    t_ps = ps_t.tile([H2, pw], F32, tag="tps")
                    nc.tensor.matmul(t_ps[:], w3_s, stage[:, so:so + pw],
                                     start=True, stop=True,
                                     skip_group_check=True)
                    if flip():
                        nc.vector.tensor_copy(tstage[:, so:so + pw], t_ps[:])
                    else:
                        nc.scalar.activation(
                            tstage[:, so:so + pw], t_ps[:],
                            mybir.ActivationFunctionType.Copy)
                elif lay == 2:
                    for t, hs_sb, off in st2:
                        state["npool"] = state.get("npool", 0) + 1
                        nc.tensor.matmul(pool_ps[:],
                                         hs_sb[:, off:off + H2],
                                         gpan_s[:, t * G_PER:(t + 1) * G_PER],
                                         start=(state["npool"] == 1),
                                         stop=(state["npool"] == NT),
                                         skip_group_check=True)

            all_pairs = []
            for ci, tiles in enumerate(chunk_tiles):
                dup_sb, b0 = pend.pop(0)
                if ci + 1 < len(chunk_tiles):
                    pend.append(chunk_loads(chunk_tiles[ci + 1]))
                GW = 4
                grps = [tiles[i:i + GW] for i in range(0, len(tiles), GW)]
                for g in reversed(grps):
                    all_pairs.append((g, dup_sb, b0))
            if lay == 0 and TPRE:
                # pre-aggregated thin tiles: compute-only, processed last
                pg = [list(range(i, min(i + 4, TPRE)))
                      for i in range(0, TPRE, 4)]
                for g in reversed(pg):
                    all_pairs.append((g, None, None))

            hastail = lay > 0
            q2, q3 = [], []
            out_stage = stage if lay == 0 else (tstage if lay == 1 else None)
            OW = 128 if lay == 0 else 64
            wb = [NT, 24, 8, 2, 0]
            WRITES = [(wb[i + 1], wb[i]) for i in range(len(wb) - 1)]

            def maybe_write(done_min):
                if lay == 2:
                    return
                while WRITES and done_min <= WRITES[0][0]:
                    wt0, wt1 = WRITES.pop(0)
                    q = nc.sync if wt0 == 0 else nc.gpsimd
                    q.dma_start(
                        out=h_out[:, wt0 * 128:wt1 * 128],
                        in_=out_stage[:, wt0 * 128:wt1 * 128])

            def run3():
                st3 = q3.pop(0)
                phase3(st3)
                done = st3[0][0] if lay == 1 else st3[0][0]
                maybe_write(done)

            def run2():
                st2 = phase2(q2.pop(0))
                if hastail:
                    q3.append(st2)
                else:
                    maybe_write(st2[0][0])

            LAG2 = 1 if lay >= 1 else 2
            LAG3 = 1
            for item in all_pairs:
                st1 = phase1(*item)
                if len(q3) >= LAG3:
                    run3()
                if len(q2) >= LAG2:
                    run2()
                q2.append(st1)
            while q2 or q3:
                if q3:
                    run3()
                if q2:
                    run2()

            # ---- classifier MLP on this core's G_PER graphs (lay 2)
            if lay == 2:
                p01 = wpool.tile([H2, G_PER], F32, tag="p01")
                nc.vector.tensor_copy(p01[:], pool_ps[:])
                y_ps = ps_pl.tile([H4, G_PER], F32, tag="yps")
                nc.tensor.matmul(y_ps[:], wc1_s, p01[:],
                                 start=True, stop=False,
                                 skip_group_check=True)
                nc.tensor.matmul(y_ps[:], bc1_r, cntx_r,
                                 start=False, stop=True,
                                 skip_group_check=True)
                y_s = wpool.tile([H4, G_PER], F32, tag="ys")
                nc.vector.tensor_scalar_max(y_s[:], y_ps[:], 0.0)
                o_ps = ps_pl.tile([G_PER, C], F32, tag="ops")
                nc.tensor.matmul(o_ps[:], y_s[:], wc2_s,
                                 start=True, stop=False,
                                 skip_group_check=True)
                nc.tensor.matmul(o_ps[:], cntx_r, bc2_r,
                                 start=False, stop=True,
                                 skip_group_check=True)
                o_s = wpool.tile([G_PER, C], F32, tag="os")
                nc.scalar.activation(o_s[:], o_ps[:],
                                     mybir.ActivationFunctionType.Copy,
                                     scale=invc_c)
                nc.sync.dma_start(out=out_d[:], in_=o_s[:])

    nc.compile()
    return nc


# ------------------------------------------------------------------ driver
_CACHE = {}


def _get_programs(meta):
    key = (tuple(meta["kt"]), meta["n_true"], meta["NT"])
    if key not in _CACHE:
        progs = [_build_stats_program(meta)]
        progs += [_build_layer_program(meta, lay) for lay in range(3)]
        _CACHE[key] = progs
    return _CACHE[key]


def run_gnn(runner=None, **inputs):
    F, H, H2, H4, C = 128, 128, 64, 32, 2
    x = np.asarray(inputs["x"], np.float32)
    n_true = x.shape[0]
    src = np.asarray(inputs["edge_index"][0], np.int64)
    dst = np.asarray(inputs["edge_index"][1], np.int64)
    batch = np.asarray(inputs["batch"], np.int64)

    meta = _plan(src, dst, batch, n_true)
    NT, SHARD, NPAD = meta["NT"], meta["SHARD"], meta["NPAD"]
    cores = _build_static(meta, src, dst, batch)
    order = meta["order"]
    progs = _get_programs(meta)

    def run(nc, in_maps):
        if runner is not None:
            return runner(nc, in_maps)
        return run_bass_kernel_spmd(
            nc, in_maps, core_ids=list(range(N_CORES))).results

    x_new = np.zeros((NPAD + 1, F), np.float32)
    x_new[:NPAD][order < n_true] = x[order[order < n_true]]

    # ---- stats launch (BN partials + L0 pre-agg of tiles 0..TPRE-1)
    xb = x_new[:NPAD].astype(NPFP8)
    l0_dups = [_dup_layout(x_new, cores[c]["slotsrc"], DUP_NP[0])
               for c in range(N_CORES)]
    stats_maps = []
    for c in range(N_CORES):
        idx = ((np.arange(NT) * N_CORES + c)[:, None] * 128
               + np.arange(128)[None, :])
        slab = xb[idx]
        slab = np.ascontiguousarray(slab.transpose(1, 0, 2)).reshape(
            128, NT * F)
        stats_maps.append({
            "x_sh": slab, "ident": np.eye(128, dtype=np.float32)})
    res = run(progs[0], stats_maps)
    parts = np.stack([np.asarray(res[c]["stat_part"])
                      for c in range(N_CORES)], axis=2)
    sx_parts = np.ascontiguousarray(parts[:, 0, :], dtype=np.float32)
    ex2_parts = np.ascontiguousarray(parts[:, 1, :], dtype=np.float32)

    W = [np.asarray(inputs["W1"], np.float32),
         np.asarray(inputs["W2"], np.float32),
         np.asarray(inputs["W3"], np.float32)]
    brows = [np.asarray(inputs["b1"], np.float32).reshape(1, H),
             np.asarray(inputs["b2"], np.float32).reshape(1, H),
             np.asarray(inputs["b3"], np.float32).reshape(1, H2)]

    h_new = x_new
    core_out = None
    for lay in range(3):
        maps = []
        for c in range(N_CORES):
            st = cores[c]
            if lay == 0:
                rp = np.concatenate([st["sig_row"], st["sh_row"],
                                     brows[0].ravel()])
            elif lay == 1:
                rp = np.concatenate([st["sig_row"], brows[1].ravel()])
            else:
                rp = np.zeros((128, 65), np.float64)
                rp[0:64, 0:64] = np.eye(64)
            if lay == 1:
                pan = st["pans"][1]
            elif lay == 2:
                pan = np.concatenate([st["gpan"].astype(NPFP8),
                                      st["pans"][2]], axis=1)
            else:
                pan = st["pans"][0]
            m = {"dup": l0_dups[c] if lay == 0 else
                 _dup_layout(h_new, st["slotsrc"], DUP_NP[lay]),
                 "pan": np.ascontiguousarray(pan),
                 "rowpack": (rp.astype(NPBF16).reshape(1, -1) if lay < 2
                             else np.ascontiguousarray(rp.astype(NPBF16)))}
            if lay == 1:
                m["wpack"] = np.ascontiguousarray(np.concatenate(
                    [(W[1] / 8.0).astype(NPBF16), W[2].astype(NPBF16)],
                    axis=1))
            if lay == 0:
                fp = np.zeros((128, 18 + H), np.float32)
                fp[:, 0:8] = sx_parts
                fp[:, 8:16] = ex2_parts
                fp[:, 16] = np.asarray(inputs["bn_gamma"], np.float32)
                fp[:, 17] = np.asarray(inputs["bn_beta"], np.float32)
                fp[:, 18:] = W[0]
                m["f32pack"] = fp
            if lay == 2:
                mp = np.zeros((64, 80), np.float32)
                mp[:, 0:H4] = np.asarray(inputs["Wc1"], np.float32)
                mp[0:H4, H4:H4 + C] = np.asarray(inputs["Wc2"], np.float32)
                mp[0, 34:66] = np.asarray(inputs["bc1"], np.float32)
                mp[0, 66:74] = st["cntx"] * 4.0
                mp[0, 74:76] = np.asarray(inputs["bc2"], np.float32)
                mp[0:G_PER, 76] = st["invc"] / 4.0
                mp[0:64, 78] = np.asarray(inputs["b3"], np.float32) * 4.0
                m["mpack"] = mp
            maps.append(m)
        res = run(progs[1 + lay], maps)
        if lay < 2:
            OW = 128 if lay == 0 else 64
            h_new = np.zeros((NPAD + 1, OW), np.float32)
            for c in range(N_CORES):
                ho = np.asarray(res[c]["h_out"])
                hoT = ho.reshape(OW, NT, 128).transpose(1, 2, 0)
                idx = ((np.arange(NT) * N_CORES + c)[:, None] * 128
                       + np.arange(128)[None, :])
                h_new[idx] = hoT
        else:
            core_out = [np.asarray(res[c]["out"]) for c in range(N_CORES)]

    out = np.zeros((G, C), np.float32)
    for c in range(N_CORES):
        for lg, g in enumerate(meta["core_graphs"][c]):
            out[g] = core_out[c][lg]
    return out


def kernel(**inputs):
    return run_gnn(**inputs)


# revision 39
# speedup vs baseline: 1.0107x; 1.0004x over previous
"""Trainium2 Bass kernel for AudioOnlyGNN (3-layer GCN + BatchNorm + mean-pool + MLP).

v3 — graph-partitioned static slot stream:

Nodes are assigned to cores by *graph* ownership (8 graphs per core,
balanced by node count), then degree-sorted within each core and laid out in
128-row tiles; tile t's slot budget k_t = max in-degree(+self) over that tile
across all cores, giving a static slot stream identical on every core.  For
each layer the host materialises the edge-source rows in slot order (a pure
gather) so the device reads large contiguous DMA blocks.

On device, a 128-slot block contributes to a [F, ncols] PSUM tile via one
matmul whose moving operand is a narrow "panel" (slot -> dst column weight
with the GCN normalisation baked in).  The aggregate is transformed
(W^T @ agg), bias/BN-shift added as rank-1 matmuls, ReLU'd, written back.
Layers 0/1 write h' = dinv*ReLU(...) so panels never depend on h.

Because every graph lives entirely on one core, the mean-pool and classifier
MLP complete locally inside the L2 launch (no cross-core reduction): launches
are [stats+pre-agg] [L0] [L1] [L2+pool+mlp].  Between launches the host only
reorders bytes (gather / transpose), never does arithmetic on activations.
"""

import sys

sys.path.insert(0, "/opt/trn_rl_repo")

import contextlib

import numpy as np
import ml_dtypes

import concourse.bacc as bacc
import concourse.bass as bass
import concourse.mybir as mybir
from concourse.tile import TileContext
from concourse.bass_utils import run_bass_kernel_spmd

BF16 = mybir.dt.bfloat16
F32 = mybir.dt.float32
FP8 = mybir.dt.float8e3  # e3m4

NPBF16 = ml_dtypes.bfloat16
NPFP8 = ml_dtypes.float8_e3m4

N_CORES = 8
BN_EPS = 1e-5
G = 64
G_PER = G // N_CORES   # graphs per core
TPRE = 0               # tiles of L0 pre-aggregated inside the stats launch

# dtype of the host-expanded per-slot source rows, per layer
DUP_DT = [FP8, FP8, FP8]
DUP_NP = [NPFP8, NPFP8, NPFP8]
OUT_DT = [FP8, FP8]
OUT_NP = [NPFP8, NPFP8]


def _chunk_list(n0, n1, lead, mid, tail=(4, 2, 1)):
    """Chunk [n0, n1) into sizes lead + [mid...] + tail (tapered ends)."""
    n = n1 - n0
    sizes = []
    for s in lead:
        if sum(sizes) + s > n:
            break
        sizes.append(s)
    tl = [s for s in tail if s < mid]
    while sum(sizes) + sum(tl) + mid <= n:
        sizes.append(mid)
    rem = n - sum(sizes) - sum(tl)
    while rem > 0:
        add = min(rem, mid)
        sizes.append(add)
        rem -= add
    sizes += tl
    sizes = [s for s in sizes if s > 0]
    # clip overflow
    while sum(sizes) > n:
        sizes[-1] -= sum(sizes) - n
        sizes = [s for s in sizes if s > 0]
    out = []
    t = n0
    for cs in sizes:
        out.append(list(range(t, t + cs)))
        t += cs
    assert t == n1, (sizes, n0, n1)
    return out


# ------------------------------------------------------------------ planning
def _plan(src, dst, batch, n_true):
    """Static (h-independent) structure: graph packing, renumbering, slots."""
    cnt_g = np.bincount(batch, minlength=G).astype(np.int64)
    g_order = np.argsort(-cnt_g, kind="stable")
    core_graphs = [[] for _ in range(N_CORES)]
    loads = np.zeros(N_CORES, np.int64)
    for g in g_order:
        cand = [i for i in range(N_CORES) if len(core_graphs[i]) < G_PER]
        i = min(cand, key=lambda i: loads[i])
        core_graphs[i].append(int(g))
        loads[i] += cnt_g[g]
    NT = max(49, int(-(-loads.max() // 128)))
    SHARD = NT * 128
    NPAD = N_CORES * SHARD

    graph_core = np.zeros(G, np.int64)
    graph_local = np.zeros(G, np.int64)
    for c in range(N_CORES):
        for lg, g in enumerate(core_graphs[c]):
            graph_core[g] = c
            graph_local[g] = lg

    degp_true = np.bincount(dst, minlength=n_true).astype(np.int64) + 1
    node_core = graph_core[batch]

    order = np.empty(NPAD, np.int64)
    virt = n_true
    for c in range(N_CORES):
        nodes_c = np.where(node_core == c)[0]
        nodes_c = nodes_c[np.argsort(degp_true[nodes_c], kind="stable")]
        npadc = SHARD - len(nodes_c)
        ids = np.concatenate([np.arange(virt, virt + npadc), nodes_c])
        virt += npadc
        idx = ((np.arange(NT) * N_CORES + c)[:, None] * 128
               + np.arange(128)[None, :])
        order[idx.ravel()] = ids
    assert virt == NPAD
    newpos = np.empty(NPAD, np.int64)
    newpos[order] = np.arange(NPAD)

    degp = np.zeros(NPAD, np.int64)
    degp[:n_true] = degp_true

    kt = np.zeros(NT, np.int64)
    for t in range(NT):
        kt[t] = degp[order[t * 1024:(t + 1) * 1024]].max()
    kt = np.maximum(kt, 1)

    blocks = []   # per tile: list of (lo, w)
    pan_cols = []  # per tile: list of panel col offsets
    wtot = 0
    for t in range(NT):
        k = int(kt[t])
        bl = []
        for b in range(k):
            lo = (128 * b) // k
            hi = (128 * (b + 1) - 1) // k
            bl.append((lo, hi - lo + 1))
        blocks.append(bl)
        offs = []
        for lo, w in bl:
            offs.append(wtot)
            wtot += w
        pan_cols.append(offs)

    nblk = int(kt.sum())
    tile_base = np.zeros(NT + 1, np.int64)
    tile_base[1:] = np.cumsum(128 * kt)
    meta = {"kt": kt, "blocks": blocks, "pan_cols": pan_cols,
            "wtot": wtot, "nblk": nblk, "order": order, "newpos": newpos,
            "n_true": n_true, "tile_base": tile_base,
            "total_slots": int(tile_base[-1]),
            "NT": NT, "SHARD": SHARD, "NPAD": NPAD,
            "core_graphs": core_graphs, "graph_core": graph_core,
            "graph_local": graph_local, "cnt_g": cnt_g}
    return meta


def _build_static(meta, src, dst, batch):
    """Per-core constant tables: slot->src map, per-layer panels, rows."""
    kt, blocks, pan_cols = meta["kt"], meta["blocks"], meta["pan_cols"]
    wtot, nblk, order, newpos = (meta["wtot"], meta["nblk"], meta["order"],
                                 meta["newpos"])
    n_true = meta["n_true"]
    NT, SHARD, NPAD = meta["NT"], meta["SHARD"], meta["NPAD"]
    graph_local, cnt_g = meta["graph_local"], meta["cnt_g"]

    deg = np.bincount(dst, minlength=NPAD).astype(np.float64) + 1.0
    dinv = (1.0 / np.sqrt(deg)).astype(np.float64)
    dinv_pad = dinv.copy()
    dinv_pad[n_true:] = 1.0

    dinv_new = dinv_pad[order]
    batch_pad = np.full(NPAD, 0, np.int64)
    batch_pad[:n_true] = batch
    batch_new = batch_pad[order]
    valid_new = (order < n_true)

    sneig = np.bincount(dst, weights=dinv[src], minlength=NPAD)
    d2 = dinv_pad * (sneig + dinv_pad)
    d2_new = d2[order]

    cntx = np.maximum(cnt_g.astype(np.float64), 1.0)   # [G]
    invc = 1.0 / cntx

    s_new = newpos[src]
    d_new = newpos[dst]
    g_tile = d_new // 128
    core_of = g_tile % N_CORES
    tloc = g_tile // N_CORES
    dloc = d_new % 128

    tile_base = meta["tile_base"]
    total_slots = meta["total_slots"]

    edge_w0 = dinv[src] * dinv_pad[dst] * dinv_pad[dst]

    cores = []
    for c in range(N_CORES):
        sel = core_of == c
        es, et, ed = s_new[sel], tloc[sel], dloc[sel]
        ew0 = edge_w0[sel]
        key = et * (128 * 64) + ed
        o = np.argsort(key, kind="stable")
        es, et, ed, ew0 = es[o], et[o], ed[o], ew0[o]
        k_of = kt[et]
        node_key = et * 128 + ed
        uniq, first_idx, counts = np.unique(node_key, return_index=True,
                                            return_counts=True)
        rank = np.arange(len(node_key)) - np.repeat(first_idx, counts)
        slot = tile_base[et] + ed * k_of + 1 + rank   # +1: self slot at 0

        tt = np.arange(NT).repeat(128)
        dd = np.tile(np.arange(128), NT)
        own_new = (tt * N_CORES + np.full(NT * 128, c)) * 128 + dd
        own_valid = valid_new[own_new]
        self_slot = tile_base[tt] + dd * kt[tt]

        slotsrc = np.full(total_slots, NPAD, np.int64)  # NPAD -> zero row
        slotsrc[slot] = es
        slotsrc[self_slot[own_valid]] = own_new[own_valid]

        dv_own = dinv_new[own_new]
        w_l0 = np.zeros(total_slots, np.float64)
        w_l0[slot] = ew0
        w_l0[self_slot[own_valid]] = (dv_own ** 3)[own_valid]
        col_dinv = np.repeat(dv_own, np.repeat(kt, 128))
        filled = np.zeros(total_slots, bool)
        filled[slot] = True
        filled[self_slot[own_valid]] = True
        w_l1 = np.where(filled, col_dinv ** 2, 0.0)
        w_l2 = np.where(filled, col_dinv, 0.0)

        pans = []
        for wv, psc in ((w_l0, 8.0), (w_l1, 8.0), (w_l2, 4.0)):
            pan = np.zeros((128, wtot), np.float64)
            for t in range(NT):
                k = int(kt[t])
                for b, (lo, w) in enumerate(blocks[t]):
                    co = pan_cols[t][b]
                    sl0 = tile_base[t] + b * 128
                    ss = np.arange(sl0, sl0 + 128)
                    cc = (ss - tile_base[t]) // k - lo
                    ok = (cc >= 0) & (cc < w)
                    pan[np.arange(128)[ok], co + cc[ok]] = wv[ss][ok]
            pans.append((pan * psc).astype(NPFP8))

        sig_row = np.zeros(SHARD, np.float64)
        sh_row = np.zeros(SHARD, np.float64)
        for t in range(NT):
            cols = slice(t * 128, (t + 1) * 128)
            nn = (t * N_CORES + c) * 128 + np.arange(128)
            sig_row[cols] = dinv_new[nn]
            sh_row[cols] = d2_new[nn] * dinv_new[nn]

        # pool panel [128, NT*G_PER]: 1.0 at (d, t*G_PER + local_graph)
        gpan = np.zeros((128, NT * G_PER), np.float64)
        for t in range(NT):
            nn = (t * N_CORES + c) * 128 + np.arange(128)
            gb = graph_local[batch_new[nn]]
            ok = valid_new[nn]
            gpan[np.arange(128)[ok], t * G_PER + gb[ok]] = 1.0

        cg = meta["core_graphs"][c]
        cores.append({
            "slotsrc": slotsrc,
            "pans": pans,
            "sig_row": sig_row,
            "sh_row": sh_row,
            "gpan": gpan.astype(NPBF16),
            "cntx": cntx[cg].astype(np.float32),     # [G_PER]
            "invc": invc[cg].astype(np.float32),     # [G_PER]
        })
    return cores


def _dup_layout(h_new, slotsrc, np_dt):
    """[NPAD(+1), F] new-indexed rows -> [128, NBLK*F] slot-stream layout."""
    rows = h_new[slotsrc]
    nblk = rows.shape[0] // 128
    F = rows.shape[1]
    return np.ascontiguousarray(
        rows.reshape(nblk, 128, F).transpose(1, 0, 2)
    ).reshape(128, nblk * F).astype(np_dt)


# ------------------------------------------------------------------ programs
def _build_stats_program(meta):
    """Per-core BN partial sums (Sum x, Sum x^2 over own nodes)."""
    F = 128
    NT = meta["NT"]
    nc = bacc.Bacc("TRN2", target_bir_lowering=False, debug=False,
                   num_devices=N_CORES)
    xs_d = nc.dram_tensor("x_sh", [128, NT * F], FP8,
                          kind="ExternalInput").ap()
    ident_d = nc.dram_tensor("ident", [128, 128], F32,
                             kind="ExternalInput").ap()
    out_d = nc.dram_tensor("stat_part", [128, 2], F32,
                           kind="ExternalOutput").ap()
    XS = [0, 12, 24, 36, 45, NT]
    with TileContext(nc) as tc:
        with tc.tile_pool(name="w", bufs=1) as wp, \
             tc.tile_pool(name="ps", bufs=1, space="PSUM") as pp:
            xs = wp.tile([128, NT * F], FP8, tag="xs")
            ident_s = wp.tile([128, 128], F32, tag="id")
            nc.sync.dma_start(out=xs[:, :XS[1] * F], in_=xs_d[:, :XS[1] * F])
            nc.scalar.dma_start(out=ident_s[:], in_=ident_d[:])
            for q in range(1, len(XS) - 1):
                nc.sync.dma_start(out=xs[:, XS[q] * F:XS[q + 1] * F],
                                  in_=xs_d[:, XS[q] * F:XS[q + 1] * F])
            ones_s = wp.tile([128, 1], FP8, tag="ones")
            nc.vector.memset(ones_s[:], 1.0)
            xtx_ps = pp.tile([128, 128], F32, tag="xtx")
            sx_ps = pp.tile([128, 1], F32, tag="sx")
            for t in range(NT):
                sl = xs[:, t * F:(t + 1) * F]
                nc.tensor.matmul(xtx_ps[:], sl, sl, start=(t == 0),
                                 stop=(t == NT - 1), skip_group_check=True)
                nc.tensor.matmul(sx_ps[:], sl, ones_s[:],
                                 start=(t == 0), stop=(t == NT - 1),
                                 skip_group_check=True)
            dg = wp.tile([128, 128], F32, tag="dg")
            nc.vector.tensor_tensor(dg[:], xtx_ps[:], ident_s[:],
                                    mybir.AluOpType.mult)
            o = wp.tile([128, 2], F32, tag="o")
            nc.vector.tensor_reduce(o[:, 1:2], dg[:], mybir.AxisListType.X,
                                    mybir.AluOpType.add)
            nc.vector.tensor_copy(o[:, 0:1], sx_ps[:])
            nc.scalar.dma_start(out=out_d[:], in_=o[:])
    nc.compile()
    return nc


def _build_layer_program(meta, lay):
    kt, blocks, pan_cols, wtot, nblk, tile_base = (
        meta["kt"], meta["blocks"], meta["pan_cols"], meta["wtot"],
        meta["nblk"], meta["tile_base"])
    NT, SHARD = meta["NT"], meta["SHARD"]
    F = 128 if lay < 2 else 64
    H = 128
    H2 = 64
    H4 = 32
    C = 2
    Ho = H if lay < 2 else H2
    N_true = meta["n_true"]
    dt_in = DUP_DT[lay]
    dt_out = OUT_DT[lay] if lay < 2 else None

    nc = bacc.Bacc("TRN2", target_bir_lowering=False, debug=False,
                   num_devices=N_CORES)

    def din(name, shape, dt):
        return nc.dram_tensor(name, list(shape), dt, kind="ExternalInput").ap()

    dup_d = din("dup", [128, nblk * F], dt_in)
    if lay == 2:
        PW_EXTRA = NT * G_PER       # gpan (0/1: fp8-exact)
    else:
        PW_EXTRA = 0               # W1 in f32pack; W2|W3 in wpack
    pan_d = din("pan", [128, wtot + PW_EXTRA], FP8)
    if lay == 1:
        wp_d = din("wpack", [128, H + H2], BF16)
    # packed bf16 row constants
    if lay == 0:
        RP = 2 * SHARD + H        # sig | sh | b1
    elif lay == 1:
        RP = SHARD + H            # sig | b2
    else:
        RP = 1                    # b3 as a column
    rp_d = din("rowpack", [1, RP] if lay < 2 else [128, 65], BF16)
    if lay == 0:
        # sxp | exp | gamma | beta | W1(fp32)
        fp_d = din("f32pack", [128, 18 + H], F32)
        if TPRE:
            agp_d = din("aggT_pre", [128, TPRE * 128], BF16)
    if lay == 2:
        # mlp pack: Wc1 | Wc2 | bc1row | cntx | bc2 | invc  (f32)
        mp_d = din("mpack", [64, 80], F32)
        out_d = nc.dram_tensor("out", [G_PER, C], F32,
                               kind="ExternalOutput").ap()
    else:
        OW = 128 if lay == 0 else 64
        h_out = nc.dram_tensor("h_out", [OW, NT * 128], dt_out,
                               kind="ExternalOutput").ap()

    # process tiles high->low: degree sorting puts fat tiles at high
    # indices, so the tail (last chunk + final write) covers thin tiles.
    T0 = TPRE if lay == 0 else 0
    fwd = _chunk_list(T0, NT, [2, 2, 4], 8,
                      tail=(4, 2) if lay != 1 else (4, 2, 1))
    chunk_tiles = []
    hi = NT
    for ch in fwd:
        chunk_tiles.append(list(range(hi - len(ch), hi)))
        hi -= len(ch)
    assert hi == T0
    PBASE = pan_cols[TPRE][0] if lay == 0 else 0

    with TileContext(nc) as tc:
        with contextlib.ExitStack() as ctx:
            cpool = ctx.enter_context(tc.tile_pool(name="const", bufs=1))
            dpool = ctx.enter_context(tc.tile_pool(name="dup", bufs=5))
            ppool = ctx.enter_context(tc.tile_pool(name="pan", bufs=2))

            def chunk_loads(tiles):
                ct0, ct1 = tiles[0], tiles[-1] + 1
                b0 = int(tile_base[ct0] // 128)
                b1 = int(tile_base[ct1] // 128)
                dup_sb = dpool.tile([128, (b1 - b0) * F], dt_in, tag="dup")
                nc.sync.dma_start(out=dup_sb[:], in_=dup_d[:, b0 * F:b1 * F])
                return dup_sb, b0

            pend = [chunk_loads(chunk_tiles[0])]
            pan_sb = ppool.tile([128, wtot - PBASE + PW_EXTRA], FP8,
                                tag="pan")
            if lay == 1:
                wpk_s = cpool.tile([128, H + H2], BF16, tag="c_wpk")
                nc.scalar.dma_start(out=wpk_s[:], in_=wp_d[:])
            fst = NT - 12
            PAN_OFF = PW_EXTRA
            PSPLIT = PAN_OFF + pan_cols[fst][0] - PBASE
            nc.sync.dma_start(out=pan_sb[:, PSPLIT:],
                              in_=pan_d[:, PBASE + PSPLIT:])
            if PW_EXTRA:
                nc.scalar.dma_start(out=pan_sb[:, :PW_EXTRA],
                                    in_=pan_d[:, PBASE:PBASE + PW_EXTRA])

            rp_s = cpool.tile([1, RP] if lay < 2 else [128, 65], BF16,
                              tag="c_rp")
            (nc.scalar if lay == 0 else nc.sync).dma_start(
                out=rp_s[:], in_=rp_d[:])
            if lay == 0:
                fp_s = cpool.tile([128, 18 + H], F32, tag="c_fp")
                nc.scalar.dma_start(out=fp_s[:], in_=fp_d[:])
                if TPRE:
                    agp_s = cpool.tile([128, TPRE * 128], BF16, tag="c_agp")
                    nc.scalar.dma_start(out=agp_s[:], in_=agp_d[:])
            if lay == 2:
                mp_s = cpool.tile([64, 80], F32, tag="c_mp")
                nc.scalar.dma_start(out=mp_s[:], in_=mp_d[:])
            nc.sync.dma_start(out=pan_sb[:, PAN_OFF:PSPLIT],
                              in_=pan_d[:, PBASE + PAN_OFF:PBASE + PSPLIT])
            if lay == 0:
                sig_s = rp_s[0:1, 0:SHARD]
                sh_s = rp_s[0:1, SHARD:2 * SHARD]
                b_s = rp_s[0:1, 2 * SHARD:2 * SHARD + H]
            elif lay == 1:
                sig_s = rp_s[0:1, 0:SHARD]
                b_s = rp_s[0:1, SHARD:SHARD + H]
            else:
                b_s = mp_s[0:H2, 78:79]   # [H2, 1] f32 column
            zr_s = cpool.tile([1, 512], BF16, tag="c_zr")
            nc.vector.memset(zr_s[:], 0.0)
            if lay == 0:
                w1f_s = fp_s[:, 18:18 + H]
                w_s = cpool.tile([128, H], BF16, tag="c_wt")
                rw_s = cpool.tile([1, H], BF16, tag="c_rw")
            elif lay == 1:
                w_s = wpk_s[:, 0:H]
                w3_s = wpk_s[:, H:H + H2]
            else:
                gpan_s = pan_sb[:, 0:NT * G_PER]
                id_s = rp_s[:, 0:64]
                wc1_s = mp_s[:, 0:H4]                  # [64, 32]
                wc2_s = mp_s[0:H4, H4:H4 + C]          # [32, 2]
                bc1_r = mp_s[0:1, 34:66]               # [1, 32]
                cntx_r = mp_s[0:1, 66:74]              # [1, 8]
                bc2_r = mp_s[0:1, 74:76]               # [1, 2]
                invc_c = mp_s[0:G_PER, 76:77]          # [8, 1]

            # ---- BN statistics (layer 0) -> W~1 and shift row rw
            if lay == 0:
                with tc.tile_pool(name="ps_st", bufs=1, space="PSUM") as pst, \
                     tc.tile_pool(name="st_w", bufs=2) as stw:
                    sxp_s = fp_s[:, 0:8]
                    exp_s = fp_s[:, 8:16]
                    gam_s = fp_s[:, 16:17]
                    bet_s = fp_s[:, 17:18]
                    ex2 = stw.tile([128, 1], F32, tag="v1")
                    nc.vector.tensor_reduce(ex2[:], exp_s,
                                            mybir.AxisListType.X,
                                            mybir.AluOpType.add)
                    sx = stw.tile([128, 1], F32, tag="v0")
                    nc.vector.tensor_reduce(sx[:], sxp_s,
                                            mybir.AxisListType.X,
                                            mybir.AluOpType.add)
                    mu = stw.tile([128, 1], F32, tag="v2")
                    nc.vector.tensor_scalar_mul(mu[:], sx[:], 1.0 / N_true)
                    var = stw.tile([128, 1], F32, tag="v3")
                    nc.vector.tensor_scalar_mul(var[:], ex2[:], 1.0 / N_true)
                    mu2 = stw.tile([128, 1], F32, tag="v4")
                    nc.vector.tensor_tensor(mu2[:], mu[:], mu[:],
                                            mybir.AluOpType.mult)
                    nc.vector.tensor_tensor(var[:], var[:], mu2[:],
                                            mybir.AluOpType.subtract)
                    nc.vector.tensor_scalar_add(var[:], var[:], BN_EPS)
                    rec = stw.tile([128, 1], F32, tag="v5")
                    nc.vector.reciprocal(rec[:], var[:])
                    isd = stw.tile([128, 1], F32, tag="v6")
                    nc.scalar.activation(isd[:], rec[:],
                                         mybir.ActivationFunctionType.Sqrt)
                    a_c = stw.tile([128, 1], F32, tag="v7")
                    nc.vector.tensor_tensor(a_c[:], gam_s, isd[:],
                                            mybir.AluOpType.mult)
                    a8 = stw.tile([128, 1], F32, tag="v9")
                    nc.vector.tensor_scalar_mul(a8[:], a_c[:], 0.125)
                    nc.vector.tensor_scalar_mul(w_s[:], w1f_s, a8[:])
                    ca = stw.tile([128, 1], F32, tag="v8")
                    nc.vector.tensor_tensor(ca[:], mu[:], a_c[:],
                                            mybir.AluOpType.mult)
                    nc.vector.tensor_tensor(ca[:], bet_s, ca[:],
                                            mybir.AluOpType.subtract)
                    rw_ps = pst.tile([1, H], F32, tag="rw")
                    nc.tensor.matmul(rw_ps[:], ca[:], w1f_s,
                                     start=True, stop=True)
                    nc.scalar.activation(rw_s[:], rw_ps[:],
                                         mybir.ActivationFunctionType.Copy)

            spool = ctx.enter_context(tc.tile_pool(name="stg", bufs=1))
            wpool = ctx.enter_context(tc.tile_pool(name="wk", bufs=4))
            ps_agg = ctx.enter_context(
                tc.tile_pool(name="ps_agg", bufs=3, space="PSUM"))
            if lay < 2:
                ps_out = ctx.enter_context(
                    tc.tile_pool(name="ps_out", bufs=3, space="PSUM"))
            if lay == 1:
                ps_t = ctx.enter_context(
                    tc.tile_pool(name="ps_t", bufs=2, space="PSUM"))
                tstage = spool.tile([64, NT * 128], dt_out, tag="tstg")
            if lay == 2:
                ps_tr = ctx.enter_context(
                    tc.tile_pool(name="ps_tr", bufs=2, space="PSUM"))
                ps_pl = ctx.enter_context(
                    tc.tile_pool(name="ps_pl", bufs=1, space="PSUM"))
                pool_ps = ps_pl.tile([H2, G_PER], F32, tag="pool")

            if lay == 0:
                stage = spool.tile([128, NT * 128], dt_out, tag="stg")
            elif lay == 1:
                stage = spool.tile([128, NT * 128], BF16, tag="stg")

            state = {"use_dve": False, "rot": 0}

            def flip():
                state["use_dve"] = not state["use_dve"]
                return state["use_dve"]

            def rot():
                state["rot"] = (state["rot"] + 1) % 3
                return state["rot"]

            def split_copy(dst, src_ps, w):
                """PSUM->SBUF copy split across Act | DVE halves."""
                h = (w // 2 + 63) & ~63 if w > 128 else w
                nc.scalar.activation(dst[:, 0:h], src_ps[:, 0:h],
                                     mybir.ActivationFunctionType.Copy)
                if h < w:
                    nc.vector.tensor_copy(dst[:, h:w], src_ps[:, h:w])

            def split_relu(dst, src_ps, w):
                h = (w // 2 + 63) & ~63 if w > 128 else w
                nc.scalar.activation(dst[:, 0:h], src_ps[:, 0:h],
                                     mybir.ActivationFunctionType.Relu)
                if h < w:
                    nc.vector.tensor_scalar_max(dst[:, h:w],
                                                src_ps[:, h:w], 0.0)

            def split_relu_bias(dst, src_ps, w, bias):
                h = (w // 2 + 63) & ~63 if w > 128 else w
                nc.scalar.activation(dst[:, 0:h], src_ps[:, 0:h],
                                     mybir.ActivationFunctionType.Relu,
                                     bias=bias)
                if h < w:
                    nc.vector.tensor_scalar(dst[:, h:w], src_ps[:, h:w],
                                            bias, 0.0,
                                            mybir.AluOpType.add,
                                            mybir.AluOpType.max)

            def phase1(pr, dup_sb, b0):
                """agg matmuls (+ L2: bias + relu straight from PSUM)."""
                if dup_sb is None:   # lay0 tiles pre-aggregated in stats
                    return pr, None, agp_s[:, pr[0] * 128:(pr[-1] + 1) * 128]
                pw = len(pr) * 128
                rows = H2 if lay == 2 else 128
                agg_ps = ps_agg.tile([rows, pw], F32, tag="agg")
                nc.tensor.matmul(agg_ps[:], zr_s[0:1, 0:rows],
                                 zr_s[0:1, 0:pw], start=True, stop=False,
                                 skip_group_check=True)
                nb_pair = sum(int(kt[t]) for t in pr)
                bi = 0
                for hi, t in enumerate(pr):
                    for b, (lo, w) in enumerate(blocks[t]):
                        gb = int(tile_base[t] // 128) + b
                        co = pan_cols[t][b]
                        bi += 1
                        nc.tensor.matmul(
                            agg_ps[:, hi * 128 + lo:hi * 128 + lo + w],
                            dup_sb[:, (gb - b0) * F:(gb - b0 + 1) * F],
                            pan_sb[:, PAN_OFF + co - PBASE:
                                   PAN_OFF + co - PBASE + w],
                            start=False, stop=(bi == nb_pair),
                            skip_group_check=True)
                if lay == 2:
                    hsT = wpool.tile([H2, pw], BF16, tag="hsT")
                    if flip():
                        nc.vector.tensor_scalar(
                            hsT[:], agg_ps[:], b_s, 0.0,
                            mybir.AluOpType.add, mybir.AluOpType.max)
                    else:
                        nc.scalar.activation(
                            hsT[:], agg_ps[:],
                            mybir.ActivationFunctionType.Relu,
                            bias=b_s)
                    return pr, agg_ps, hsT
                aggT = wpool.tile([128, pw], BF16, tag="aggT")
                if flip():
                    nc.vector.tensor_copy(aggT[:], agg_ps[:])
                else:
                    nc.scalar.activation(aggT[:], agg_ps[:],
                                         mybir.ActivationFunctionType.Copy)
                return pr, agg_ps, aggT

            def phase2(st1):
                pr, agg_ps, aggT = st1
                pw = len(pr) * 128
                if lay < 2:
                    h_ps = ps_out.tile([Ho, pw], F32, tag="hps")
                    c0 = pr[0] * 128
                    nc.tensor.matmul(h_ps[:], w_s[:] if lay == 0 else w_s,
                                     aggT[:], start=True, stop=False,
                                     skip_group_check=True)
                    nc.tensor.matmul(h_ps[:], b_s,
                                     sig_s[0:1, c0:c0 + pw],
                                     start=False, stop=(lay != 0),
                                     skip_group_check=True)
                    if lay == 0:
                        nc.tensor.matmul(h_ps[:], rw_s[:],
                                         sh_s[0:1, c0:c0 + pw],
                                         start=False, stop=True,
                                         skip_group_check=True)
                    so = pr[0] * 128
                    if flip():
                        nc.vector.tensor_scalar_max(
                            stage[:, so:so + pw], h_ps[:], 0.0)
                    else:
                        nc.scalar.activation(
                            stage[:, so:so + pw], h_ps[:],
                            mybir.ActivationFunctionType.Relu)
                    return st1
                # lay 2: transpose each tile's hsT: [64, 128] -> [128, 64]
                hsT = aggT
                tr_ps = ps_tr.tile([128, len(pr) * H2], BF16, tag="tr")
                for hi, t in enumerate(pr):
                    nc.tensor.transpose(tr_ps[:, hi * H2:(hi + 1) * H2],
                                        hsT[:, hi * 128:(hi + 1) * 128],
                                        id_s[0:64, 0:64])
                hs_sb = wpool.tile([128, len(pr) * H2], BF16, tag="hs")
                if flip():
                    nc.vector.tensor_copy(hs_sb[:], tr_ps[:])
                else:
                    nc.scalar.activation(
                        hs_sb[:], tr_ps[:],
                        mybir.ActivationFunctionType.Copy)
                return [(t, hs_sb, hi * H2) for hi, t in enumerate(pr)]

            def phase3(st2):
                if lay == 2:
                    flip()          # odd flips/group: engines alternate
                if lay == 1:
                    pr = st2[0]
                    pw = len(pr) * 128
                    so = pr[0] * 128
                    t_ps = ps_t.tile([H2, pw], F32, tag="tps")
                    nc.tensor.matmul(t_ps[:], w3_s, stage[:, so:so + pw],
                                     start=True, stop=True,
                                     skip_group_check=True)
                    if flip():
                        nc.vector.tensor_copy(tstage[:, so:so + pw], t_ps[:])
                    else:
                        nc.scalar.activation(
                            tstage[:, so:so + pw], t_ps[:],
                            mybir.ActivationFunctionType.Copy)
                elif lay == 2:
                    for t, hs_sb, off in st2:
                        state["npool"] = state.get("npool", 0) + 1
                        nc.tensor.matmul(pool_ps[:],
                                         hs_sb[:, off:off + H2],
                                         gpan_s[:, t * G_PER:(t + 1) * G_PER],
                                         start=(state["npool"] == 1),
                                         stop=(state["npool"] == NT),
                                         skip_group_check=True)

            all_pairs = []
            for ci, tiles in enumerate(chunk_tiles):
                dup_sb, b0 = pend.pop(0)
                if ci + 1 < len(chunk_tiles):
                    pend.append(chunk_loads(chunk_tiles[ci + 1]))
                GW = 4
                grps = [tiles[i:i + GW] for i in range(0, len(tiles), GW)]
                for g in reversed(grps):
                    all_pairs.append((g, dup_sb, b0))
            if lay == 0 and TPRE:
                # pre-aggregated thin tiles: compute-only, processed last
                pg = [list(range(i, min(i + 4, TPRE)))
                      for i in range(0, TPRE, 4)]
                for g in reversed(pg):
                    all_pairs.append((g, None, None))

            hastail = lay > 0
            q2, q3 = [], []
            out_stage = stage if lay == 0 else (tstage if lay == 1 else None)
            OW = 128 if lay == 0 else 64
            wb = [NT, 24, 8, 2, 0]
            WRITES = [(wb[i + 1], wb[i]) for i in range(len(wb) - 1)]

            def maybe_write(done_min):
                if lay == 2:
                    return
                while WRITES and done_min <= WRITES[0][0]:
                    wt0, wt1 = WRITES.pop(0)
                    q = nc.sync if wt0 == 0 else nc.gpsimd
                    q.dma_start(
                        out=h_out[:, wt0 * 128:wt1 * 128],
                        in_=out_stage[:, wt0 * 128:wt1 * 128])

            def run3():
                st3 = q3.pop(0)
                phase3(st3)
                done = st3[0][0] if lay == 1 else st3[0][0]
                maybe_write(done)

            def run2():
                st2 = phase2(q2.pop(0))
                if hastail:
                    q3.append(st2)
                else:
                    maybe_write(st2[0][0])

            LAG2 = 1 if lay >= 1 else 2
            LAG3 = 1
            for item in all_pairs:
                st1 = phase1(*item)
                if len(q3) >= LAG3:
                    run3()
                if len(q2) >= LAG2:
                    run2()
                q2.append(st1)
            while q2 or q3:
                if q3:
                    run3()
                if q2:
                    run2()

            # ---- classifier MLP on this core's G_PER graphs (lay 2)
            if lay == 2:
                p01 = wpool.tile([H2, G_PER], F32, tag="p01")
                nc.vector.tensor_copy(p01[:], pool_ps[:])
                y_ps = ps_pl.tile([H4, G_PER], F32, tag="yps")
                nc.tensor.matmul(y_ps[:], wc1_s, p01[:],
                                 start=True, stop=False,
                                 skip_group_check=True)
                nc.tensor.matmul(y_ps[:], bc1_r, cntx_r,
                                 start=False, stop=True,
                                 skip_group_check=True)
                y_s = wpool.tile([H4, G_PER], F32, tag="ys")
                nc.vector.tensor_scalar_max(y_s[:], y_ps[:], 0.0)
                o_ps = ps_pl.tile([G_PER, C], F32, tag="ops")
                nc.tensor.matmul(o_ps[:], y_s[:], wc2_s,
                                 start=True, stop=False,
                                 skip_group_check=True)
                nc.tensor.matmul(o_ps[:], cntx_r, bc2_r,
                                 start=False, stop=True,
                                 skip_group_check=True)
                o_s = wpool.tile([G_PER, C], F32, tag="os")
                nc.scalar.activation(o_s[:], o_ps[:],
                                     mybir.ActivationFunctionType.Copy,
                                     scale=invc_c)
                nc.sync.dma_start(out=out_d[:], in_=o_s[:])

    nc.compile()
    return nc


# ------------------------------------------------------------------ driver
_CACHE = {}


def _get_programs(meta):
    key = (tuple(meta["kt"]), meta["n_true"], meta["NT"])
    if key not in _CACHE:
        progs = [_build_stats_program(meta)]
        progs += [_build_layer_program(meta, lay) for lay in range(3)]
        _CACHE[key] = progs
    return _CACHE[key]


def run_gnn(runner=None, **inputs):
    F, H, H2, H4, C = 128, 128, 64, 32, 2
    x = np.asarray(inputs["x"], np.float32)
    n_true = x.shape[0]
    src = np.asarray(inputs["edge_index"][0], np.int64)
    dst = np.asarray(inputs["edge_index"][1], np.int64)
    batch = np.asarray(inputs["batch"], np.int64)

    meta = _plan(src, dst, batch, n_true)
    NT, SHARD, NPAD = meta["NT"], meta["SHARD"], meta["NPAD"]
    cores = _build_static(meta, src, dst, batch)
    order = meta["order"]
    progs = _get_programs(meta)

    def run(nc, in_maps):
        if runner is not None:
            return runner(nc, in_maps)
        return run_bass_kernel_spmd(
            nc, in_maps, core_ids=list(range(N_CORES))).results

    x_new = np.zeros((NPAD + 1, F), np.float32)
    x_new[:NPAD][order < n_true] = x[order[order < n_true]]

    # ---- stats launch (BN partials + L0 pre-agg of tiles 0..TPRE-1)
    xb = x_new[:NPAD].astype(NPFP8)
    l0_dups = [_dup_layout(x_new, cores[c]["slotsrc"], DUP_NP[0])
               for c in range(N_CORES)]
    stats_maps = []
    for c in range(N_CORES):
        idx = ((np.arange(NT) * N_CORES + c)[:, None] * 128
               + np.arange(128)[None, :])
        slab = xb[idx]
        slab = np.ascontiguousarray(slab.transpose(1, 0, 2)).reshape(
            128, NT * F)
        stats_maps.append({
            "x_sh": slab, "ident": np.eye(128, dtype=np.float32)})
    res = run(progs[0], stats_maps)
    parts = np.stack([np.asarray(res[c]["stat_part"])
                      for c in range(N_CORES)], axis=2)
    sx_parts = np.ascontiguousarray(parts[:, 0, :], dtype=np.float32)
    ex2_parts = np.ascontiguousarray(parts[:, 1, :], dtype=np.float32)

    W = [np.asarray(inputs["W1"], np.float32),
         np.asarray(inputs["W2"], np.float32),
         np.asarray(inputs["W3"], np.float32)]
    brows = [np.asarray(inputs["b1"], np.float32).reshape(1, H),
             np.asarray(inputs["b2"], np.float32).reshape(1, H),
             np.asarray(inputs["b3"], np.float32).reshape(1, H2)]

    h_new = x_new
    core_out = None
    for lay in range(3):
        maps = []
        for c in range(N_CORES):
            st = cores[c]
            if lay == 0:
                rp = np.concatenate([st["sig_row"], st["sh_row"],
                                     brows[0].ravel()])
            elif lay == 1:
                rp = np.concatenate([st["sig_row"], brows[1].ravel()])
            else:
                rp = np.zeros((128, 65), np.float64)
                rp[0:64, 0:64] = np.eye(64)
            if lay == 1:
                pan = st["pans"][1]
            elif lay == 2:
                pan = np.concatenate([st["gpan"].astype(NPFP8),
                                      st["pans"][2]], axis=1)
            else:
                pan = st["pans"][0]
            m = {"dup": l0_dups[c] if lay == 0 else
                 _dup_layout(h_new, st["slotsrc"], DUP_NP[lay]),
                 "pan": np.ascontiguousarray(pan),
                 "rowpack": (rp.astype(NPBF16).reshape(1, -1) if lay < 2
                             else np.ascontiguousarray(rp.astype(NPBF16)))}
            if lay == 1:
                m["wpack"] = np.ascontiguousarray(np.concatenate(
                    [(W[1] / 8.0).astype(NPBF16), W[2].astype(NPBF16)],
                    axis=1))
            if lay == 0:
                fp = np.zeros((128, 18 + H), np.float32)
                fp[:, 0:8] = sx_parts
                fp[:, 8:16] = ex2_parts
                fp[:, 16] = np.asarray(inputs["bn_gamma"], np.float32)
                fp[:, 17] = np.asarray(inputs["bn_beta"], np.float32)
                fp[:, 18:] = W[0]
                m["f32pack"] = fp
            if lay == 2:
                mp = np.zeros((64, 80), np.float32)
                mp[:, 0:H4] = np.asarray(inputs["Wc1"], np.float32)
                mp[0:H4, H4:H4 + C] = np.asarray(inputs["Wc2"], np.float32)
                mp[0, 34:66] = np.asarray(inputs["bc1"], np.float32)
                mp[0, 66:74] = st["cntx"] * 4.0
                mp[0, 74:76] = np.asarray(inputs["bc2"], np.float32)
                mp[0:G_PER, 76] = st["invc"] / 4.0
                mp[0:64, 78] = np.asarray(inputs["b3"], np.float32) * 4.0
                m["mpack"] = mp
            maps.append(m)
        res = run(progs[1 + lay], maps)
        if lay < 2:
            OW = 128 if lay == 0 else 64
            h_new = np.zeros((NPAD + 1, OW), np.float32)
            for c in range(N_CORES):
                ho = np.asarray(res[c]["h_out"])
                hoT = ho.reshape(OW, NT, 128).transpose(1, 2, 0)
                idx = ((np.arange(NT) * N_CORES + c)[:, None] * 128
                       + np.arange(128)[None, :])
                h_new[idx] = hoT
        else:
            core_out = [np.asarray(res[c]["out"]) for c in range(N_CORES)]

    out = np.zeros((G, C), np.float32)
    for c in range(N_CORES):
        for lg, g in enumerate(meta["core_graphs"][c]):
            out[g] = core_out[c][lg]
    return out


def kernel(**inputs):
    return run_gnn(**inputs)


# revision 40
# speedup vs baseline: 1.0122x; 1.0015x over previous
"""Trainium2 Bass kernel for AudioOnlyGNN (3-layer GCN + BatchNorm + mean-pool + MLP).

v3 — graph-partitioned static slot stream:

Nodes are assigned to cores by *graph* ownership (8 graphs per core,
balanced by node count), then degree-sorted within each core and laid out in
128-row tiles; tile t's slot budget k_t = max in-degree(+self) over that tile
across all cores, giving a static slot stream identical on every core.  For
each layer the host materialises the edge-source rows in slot order (a pure
gather) so the device reads large contiguous DMA blocks.

On device, a 128-slot block contributes to a [F, ncols] PSUM tile via one
matmul whose moving operand is a narrow "panel" (slot -> dst column weight
with the GCN normalisation baked in).  The aggregate is transformed
(W^T @ agg), bias/BN-shift added as rank-1 matmuls, ReLU'd, written back.
Layers 0/1 write h' = dinv*ReLU(...) so panels never depend on h.

Because every graph lives entirely on one core, the mean-pool and classifier
MLP complete locally inside the L2 launch (no cross-core reduction): launches
are [stats+pre-agg] [L0] [L1] [L2+pool+mlp].  Between launches the host only
reorders bytes (gather / transpose), never does arithmetic on activations.
"""

import sys

sys.path.insert(0, "/opt/trn_rl_repo")

import contextlib

import numpy as np
import ml_dtypes

import concourse.bacc as bacc
import concourse.bass as bass
import concourse.mybir as mybir
from concourse.tile import TileContext
from concourse.bass_utils import run_bass_kernel_spmd

BF16 = mybir.dt.bfloat16
F32 = mybir.dt.float32
FP8 = mybir.dt.float8e3  # e3m4

NPBF16 = ml_dtypes.bfloat16
NPFP8 = ml_dtypes.float8_e3m4

N_CORES = 8
BN_EPS = 1e-5
G = 64
G_PER = G // N_CORES   # graphs per core
TPRE = 0               # tiles of L0 pre-aggregated inside the stats launch

# dtype of the host-expanded per-slot source rows, per layer
DUP_DT = [FP8, FP8, FP8]
DUP_NP = [NPFP8, NPFP8, NPFP8]
OUT_DT = [FP8, FP8]
OUT_NP = [NPFP8, NPFP8]


def _chunk_list(n0, n1, lead, mid, tail=(4, 2, 1)):
    """Chunk [n0, n1) into sizes lead + [mid...] + tail (tapered ends)."""
    n = n1 - n0
    sizes = []
    for s in lead:
        if sum(sizes) + s > n:
            break
        sizes.append(s)
    tl = [s for s in tail if s < mid]
    while sum(sizes) + sum(tl) + mid <= n:
        sizes.append(mid)
    rem = n - sum(sizes) - sum(tl)
    while rem > 0:
        add = min(rem, mid)
        sizes.append(add)
        rem -= add
    sizes += tl
    sizes = [s for s in sizes if s > 0]
    # clip overflow
    while sum(sizes) > n:
        sizes[-1] -= sum(sizes) - n
        sizes = [s for s in sizes if s > 0]
    out = []
    t = n0
    for cs in sizes:
        out.append(list(range(t, t + cs)))
        t += cs
    assert t == n1, (sizes, n0, n1)
    return out


# ------------------------------------------------------------------ planning
def _plan(src, dst, batch, n_true):
    """Static (h-independent) structure: graph packing, renumbering, slots."""
    cnt_g = np.bincount(batch, minlength=G).astype(np.int64)
    g_order = np.argsort(-cnt_g, kind="stable")
    core_graphs = [[] for _ in range(N_CORES)]
    loads = np.zeros(N_CORES, np.int64)
    for g in g_order:
        cand = [i for i in range(N_CORES) if len(core_graphs[i]) < G_PER]
        i = min(cand, key=lambda i: loads[i])
        core_graphs[i].append(int(g))
        loads[i] += cnt_g[g]
    NT = max(49, int(-(-loads.max() // 128)))
    SHARD = NT * 128
    NPAD = N_CORES * SHARD

    graph_core = np.zeros(G, np.int64)
    graph_local = np.zeros(G, np.int64)
    for c in range(N_CORES):
        for lg, g in enumerate(core_graphs[c]):
            graph_core[g] = c
            graph_local[g] = lg

    degp_true = np.bincount(dst, minlength=n_true).astype(np.int64) + 1
    node_core = graph_core[batch]

    order = np.empty(NPAD, np.int64)
    virt = n_true
    for c in range(N_CORES):
        nodes_c = np.where(node_core == c)[0]
        nodes_c = nodes_c[np.argsort(degp_true[nodes_c], kind="stable")]
        npadc = SHARD - len(nodes_c)
        ids = np.concatenate([np.arange(virt, virt + npadc), nodes_c])
        virt += npadc
        idx = ((np.arange(NT) * N_CORES + c)[:, None] * 128
               + np.arange(128)[None, :])
        order[idx.ravel()] = ids
    assert virt == NPAD
    newpos = np.empty(NPAD, np.int64)
    newpos[order] = np.arange(NPAD)

    degp = np.zeros(NPAD, np.int64)
    degp[:n_true] = degp_true

    kt = np.zeros(NT, np.int64)
    for t in range(NT):
        kt[t] = degp[order[t * 1024:(t + 1) * 1024]].max()
    kt = np.maximum(kt, 1)

    blocks = []   # per tile: list of (lo, w)
    pan_cols = []  # per tile: list of panel col offsets
    wtot = 0
    for t in range(NT):
        k = int(kt[t])
        bl = []
        for b in range(k):
            lo = (128 * b) // k
            hi = (128 * (b + 1) - 1) // k
            bl.append((lo, hi - lo + 1))
        blocks.append(bl)
        offs = []
        for lo, w in bl:
            offs.append(wtot)
            wtot += w
        pan_cols.append(offs)

    nblk = int(kt.sum())
    tile_base = np.zeros(NT + 1, np.int64)
    tile_base[1:] = np.cumsum(128 * kt)
    meta = {"kt": kt, "blocks": blocks, "pan_cols": pan_cols,
            "wtot": wtot, "nblk": nblk, "order": order, "newpos": newpos,
            "n_true": n_true, "tile_base": tile_base,
            "total_slots": int(tile_base[-1]),
            "NT": NT, "SHARD": SHARD, "NPAD": NPAD,
            "core_graphs": core_graphs, "graph_core": graph_core,
            "graph_local": graph_local, "cnt_g": cnt_g}
    return meta


def _build_static(meta, src, dst, batch):
    """Per-core constant tables: slot->src map, per-layer panels, rows."""
    kt, blocks, pan_cols = meta["kt"], meta["blocks"], meta["pan_cols"]
    wtot, nblk, order, newpos = (meta["wtot"], meta["nblk"], meta["order"],
                                 meta["newpos"])
    n_true = meta["n_true"]
    NT, SHARD, NPAD = meta["NT"], meta["SHARD"], meta["NPAD"]
    graph_local, cnt_g = meta["graph_local"], meta["cnt_g"]

    deg = np.bincount(dst, minlength=NPAD).astype(np.float64) + 1.0
    dinv = (1.0 / np.sqrt(deg)).astype(np.float64)
    dinv_pad = dinv.copy()
    dinv_pad[n_true:] = 1.0

    dinv_new = dinv_pad[order]
    batch_pad = np.full(NPAD, 0, np.int64)
    batch_pad[:n_true] = batch
    batch_new = batch_pad[order]
    valid_new = (order < n_true)

    sneig = np.bincount(dst, weights=dinv[src], minlength=NPAD)
    d2 = dinv_pad * (sneig + dinv_pad)
    d2_new = d2[order]

    cntx = np.maximum(cnt_g.astype(np.float64), 1.0)   # [G]
    invc = 1.0 / cntx

    s_new = newpos[src]
    d_new = newpos[dst]
    g_tile = d_new // 128
    core_of = g_tile % N_CORES
    tloc = g_tile // N_CORES
    dloc = d_new % 128

    tile_base = meta["tile_base"]
    total_slots = meta["total_slots"]

    edge_w0 = dinv[src] * dinv_pad[dst] * dinv_pad[dst]

    cores = []
    for c in range(N_CORES):
        sel = core_of == c
        es, et, ed = s_new[sel], tloc[sel], dloc[sel]
        ew0 = edge_w0[sel]
        key = et * (128 * 64) + ed
        o = np.argsort(key, kind="stable")
        es, et, ed, ew0 = es[o], et[o], ed[o], ew0[o]
        k_of = kt[et]
        node_key = et * 128 + ed
        uniq, first_idx, counts = np.unique(node_key, return_index=True,
                                            return_counts=True)
        rank = np.arange(len(node_key)) - np.repeat(first_idx, counts)
        slot = tile_base[et] + ed * k_of + 1 + rank   # +1: self slot at 0

        tt = np.arange(NT).repeat(128)
        dd = np.tile(np.arange(128), NT)
        own_new = (tt * N_CORES + np.full(NT * 128, c)) * 128 + dd
        own_valid = valid_new[own_new]
        self_slot = tile_base[tt] + dd * kt[tt]

        slotsrc = np.full(total_slots, NPAD, np.int64)  # NPAD -> zero row
        slotsrc[slot] = es
        slotsrc[self_slot[own_valid]] = own_new[own_valid]

        dv_own = dinv_new[own_new]
        w_l0 = np.zeros(total_slots, np.float64)
        w_l0[slot] = ew0
        w_l0[self_slot[own_valid]] = (dv_own ** 3)[own_valid]
        col_dinv = np.repeat(dv_own, np.repeat(kt, 128))
        filled = np.zeros(total_slots, bool)
        filled[slot] = True
        filled[self_slot[own_valid]] = True
        w_l1 = np.where(filled, col_dinv ** 2, 0.0)
        w_l2 = np.where(filled, col_dinv, 0.0)

        pans = []
        for wv, psc in ((w_l0, 8.0), (w_l1, 8.0), (w_l2, 4.0)):
            pan = np.zeros((128, wtot), np.float64)
            for t in range(NT):
                k = int(kt[t])
                for b, (lo, w) in enumerate(blocks[t]):
                    co = pan_cols[t][b]
                    sl0 = tile_base[t] + b * 128
                    ss = np.arange(sl0, sl0 + 128)
                    cc = (ss - tile_base[t]) // k - lo
                    ok = (cc >= 0) & (cc < w)
                    pan[np.arange(128)[ok], co + cc[ok]] = wv[ss][ok]
            pans.append((pan * psc).astype(NPFP8))

        sig_row = np.zeros(SHARD, np.float64)
        sh_row = np.zeros(SHARD, np.float64)
        for t in range(NT):
            cols = slice(t * 128, (t + 1) * 128)
            nn = (t * N_CORES + c) * 128 + np.arange(128)
            sig_row[cols] = dinv_new[nn]
            sh_row[cols] = d2_new[nn] * dinv_new[nn]

        # pool panel [128, NT*G_PER]: 1.0 at (d, t*G_PER + local_graph)
        gpan = np.zeros((128, NT * G_PER), np.float64)
        for t in range(NT):
            nn = (t * N_CORES + c) * 128 + np.arange(128)
            gb = graph_local[batch_new[nn]]
            ok = valid_new[nn]
            gpan[np.arange(128)[ok], t * G_PER + gb[ok]] = 1.0

        cg = meta["core_graphs"][c]
        cores.append({
            "slotsrc": slotsrc,
            "pans": pans,
            "sig_row": sig_row,
            "sh_row": sh_row,
            "gpan": gpan.astype(NPBF16),
            "cntx": cntx[cg].astype(np.float32),     # [G_PER]
            "invc": invc[cg].astype(np.float32),     # [G_PER]
        })
    return cores


def _dup_layout(h_new, slotsrc, np_dt):
    """[NPAD(+1), F] new-indexed rows -> [128, NBLK*F] slot-stream layout."""
    rows = h_new[slotsrc]
    nblk = rows.shape[0] // 128
    F = rows.shape[1]
    return np.ascontiguousarray(
        rows.reshape(nblk, 128, F).transpose(1, 0, 2)
    ).reshape(128, nblk * F).astype(np_dt)


# ------------------------------------------------------------------ programs
def _build_stats_program(meta):
    """Per-core BN partial sums (Sum x, Sum x^2 over own nodes)."""
    F = 128
    NT = meta["NT"]
    nc = bacc.Bacc("TRN2", target_bir_lowering=False, debug=False,
                   num_devices=N_CORES)
    xs_d = nc.dram_tensor("x_sh", [128, NT * F], FP8,
                          kind="ExternalInput").ap()
    ident_d = nc.dram_tensor("ident", [128, 128], F32,
                             kind="ExternalInput").ap()
    out_d = nc.dram_tensor("stat_part", [128, 2], F32,
                           kind="ExternalOutput").ap()
    XS = [0, 12, 24, 36, 45, NT]
    with TileContext(nc) as tc:
        with tc.tile_pool(name="w", bufs=1) as wp, \
             tc.tile_pool(name="ps", bufs=1, space="PSUM") as pp:
            xs = wp.tile([128, NT * F], FP8, tag="xs")
            ident_s = wp.tile([128, 128], F32, tag="id")
            nc.sync.dma_start(out=xs[:, :XS[1] * F], in_=xs_d[:, :XS[1] * F])
            nc.scalar.dma_start(out=ident_s[:], in_=ident_d[:])
            for q in range(1, len(XS) - 1):
                nc.sync.dma_start(out=xs[:, XS[q] * F:XS[q + 1] * F],
                                  in_=xs_d[:, XS[q] * F:XS[q + 1] * F])
            ones_s = wp.tile([128, 1], FP8, tag="ones")
            nc.vector.memset(ones_s[:], 1.0)
            xtx_ps = pp.tile([128, 128], F32, tag="xtx")
            sx_ps = pp.tile([128, 1], F32, tag="sx")
            for t in range(NT):
                sl = xs[:, t * F:(t + 1) * F]
                nc.tensor.matmul(xtx_ps[:], sl, sl, start=(t == 0),
                                 stop=(t == NT - 1), skip_group_check=True)
                nc.tensor.matmul(sx_ps[:], sl, ones_s[:],
                                 start=(t == 0), stop=(t == NT - 1),
                                 skip_group_check=True)
            dg = wp.tile([128, 128], F32, tag="dg")
            nc.vector.tensor_tensor(dg[:], xtx_ps[:], ident_s[:],
                                    mybir.AluOpType.mult)
            o = wp.tile([128, 2], F32, tag="o")
            nc.vector.tensor_reduce(o[:, 1:2], dg[:], mybir.AxisListType.X,
                                    mybir.AluOpType.add)
            nc.vector.tensor_copy(o[:, 0:1], sx_ps[:])
            nc.scalar.dma_start(out=out_d[:], in_=o[:])
    nc.compile()
    return nc


def _build_layer_program(meta, lay):
    kt, blocks, pan_cols, wtot, nblk, tile_base = (
        meta["kt"], meta["blocks"], meta["pan_cols"], meta["wtot"],
        meta["nblk"], meta["tile_base"])
    NT, SHARD = meta["NT"], meta["SHARD"]
    F = 128 if lay < 2 else 64
    H = 128
    H2 = 64
    H4 = 32
    C = 2
    Ho = H if lay < 2 else H2
    N_true = meta["n_true"]
    dt_in = DUP_DT[lay]
    dt_out = OUT_DT[lay] if lay < 2 else None

    nc = bacc.Bacc("TRN2", target_bir_lowering=False, debug=False,
                   num_devices=N_CORES)

    def din(name, shape, dt):
        return nc.dram_tensor(name, list(shape), dt, kind="ExternalInput").ap()

    dup_d = din("dup", [128, nblk * F], dt_in)
    if lay == 2:
        PW_EXTRA = NT * G_PER       # gpan (0/1: fp8-exact)
    else:
        PW_EXTRA = 0               # W1 in f32pack; W2|W3 in wpack
    pan_d = din("pan", [128, wtot + PW_EXTRA], FP8)
    if lay == 1:
        wp_d = din("wpack", [128, H + H2], BF16)
    # packed bf16 row constants
    if lay == 0:
        RP = 2 * SHARD + H        # sig | sh | b1
    elif lay == 1:
        RP = SHARD + H            # sig | b2
    else:
        RP = 1                    # b3 as a column
    rp_d = din("rowpack", [1, RP] if lay < 2 else [128, 65], BF16)
    if lay == 0:
        # sxp | exp | gamma | beta | W1(fp32)
        fp_d = din("f32pack", [128, 18 + H], F32)
        if TPRE:
            agp_d = din("aggT_pre", [128, TPRE * 128], BF16)
    if lay == 2:
        # mlp pack: Wc1 | Wc2 | bc1row | cntx | bc2 | invc  (f32)
        mp_d = din("mpack", [64, 80], F32)
        out_d = nc.dram_tensor("out", [G_PER, C], F32,
                               kind="ExternalOutput").ap()
    else:
        OW = 128 if lay == 0 else 64
        h_out = nc.dram_tensor("h_out", [OW, NT * 128], dt_out,
                               kind="ExternalOutput").ap()

    # process tiles high->low: degree sorting puts fat tiles at high
    # indices, so the tail (last chunk + final write) covers thin tiles.
    T0 = TPRE if lay == 0 else 0
    fwd = _chunk_list(T0, NT, [2, 2, 4], 8, tail=(4, 2, 1))
    chunk_tiles = []
    hi = NT
    for ch in fwd:
        chunk_tiles.append(list(range(hi - len(ch), hi)))
        hi -= len(ch)
    assert hi == T0
    PBASE = pan_cols[TPRE][0] if lay == 0 else 0

    with TileContext(nc) as tc:
        with contextlib.ExitStack() as ctx:
            cpool = ctx.enter_context(tc.tile_pool(name="const", bufs=1))
            dpool = ctx.enter_context(tc.tile_pool(name="dup", bufs=5))
            ppool = ctx.enter_context(tc.tile_pool(name="pan", bufs=2))

            def chunk_loads(tiles):
                ct0, ct1 = tiles[0], tiles[-1] + 1
                b0 = int(tile_base[ct0] // 128)
                b1 = int(tile_base[ct1] // 128)
                dup_sb = dpool.tile([128, (b1 - b0) * F], dt_in, tag="dup")
                nc.sync.dma_start(out=dup_sb[:], in_=dup_d[:, b0 * F:b1 * F])
                return dup_sb, b0

            pend = [chunk_loads(chunk_tiles[0])]
            pan_sb = ppool.tile([128, wtot - PBASE + PW_EXTRA], FP8,
                                tag="pan")
            if lay == 1:
                wpk_s = cpool.tile([128, H + H2], BF16, tag="c_wpk")
                nc.scalar.dma_start(out=wpk_s[:], in_=wp_d[:])
            fst = NT - 12
            PAN_OFF = PW_EXTRA
            PSPLIT = PAN_OFF + pan_cols[fst][0] - PBASE
            nc.sync.dma_start(out=pan_sb[:, PSPLIT:],
                              in_=pan_d[:, PBASE + PSPLIT:])
            if PW_EXTRA:
                nc.scalar.dma_start(out=pan_sb[:, :PW_EXTRA],
                                    in_=pan_d[:, PBASE:PBASE + PW_EXTRA])

            rp_s = cpool.tile([1, RP] if lay < 2 else [128, 65], BF16,
                              tag="c_rp")
            (nc.scalar if lay == 0 else nc.sync).dma_start(
                out=rp_s[:], in_=rp_d[:])
            if lay == 0:
                fp_s = cpool.tile([128, 18 + H], F32, tag="c_fp")
                nc.scalar.dma_start(out=fp_s[:], in_=fp_d[:])
                if TPRE:
                    agp_s = cpool.tile([128, TPRE * 128], BF16, tag="c_agp")
                    nc.scalar.dma_start(out=agp_s[:], in_=agp_d[:])
            if lay == 2:
                mp_s = cpool.tile([64, 80], F32, tag="c_mp")
                nc.scalar.dma_start(out=mp_s[:], in_=mp_d[:])
            nc.sync.dma_start(out=pan_sb[:, PAN_OFF:PSPLIT],
                              in_=pan_d[:, PBASE + PAN_OFF:PBASE + PSPLIT])
            if lay == 0:
                sig_s = rp_s[0:1, 0:SHARD]
                sh_s = rp_s[0:1, SHARD:2 * SHARD]
                b_s = rp_s[0:1, 2 * SHARD:2 * SHARD + H]
            elif lay == 1:
                sig_s = rp_s[0:1, 0:SHARD]
                b_s = rp_s[0:1, SHARD:SHARD + H]
            else:
                b_s = mp_s[0:H2, 78:79]   # [H2, 1] f32 column
            zr_s = cpool.tile([1, 512], BF16, tag="c_zr")
            nc.vector.memset(zr_s[:], 0.0)
            if lay == 0:
                w1f_s = fp_s[:, 18:18 + H]
                w_s = cpool.tile([128, H], BF16, tag="c_wt")
                rw_s = cpool.tile([1, H], BF16, tag="c_rw")
            elif lay == 1:
                w_s = wpk_s[:, 0:H]
                w3_s = wpk_s[:, H:H + H2]
            else:
                gpan_s = pan_sb[:, 0:NT * G_PER]
                id_s = rp_s[:, 0:64]
                wc1_s = mp_s[:, 0:H4]                  # [64, 32]
                wc2_s = mp_s[0:H4, H4:H4 + C]          # [32, 2]
                bc1_r = mp_s[0:1, 34:66]               # [1, 32]
                cntx_r = mp_s[0:1, 66:74]              # [1, 8]
                bc2_r = mp_s[0:1, 74:76]               # [1, 2]
                invc_c = mp_s[0:G_PER, 76:77]          # [8, 1]

            # ---- BN statistics (layer 0) -> W~1 and shift row rw
            if lay == 0:
                with tc.tile_pool(name="ps_st", bufs=1, space="PSUM") as pst, \
                     tc.tile_pool(name="st_w", bufs=2) as stw:
                    sxp_s = fp_s[:, 0:8]
                    exp_s = fp_s[:, 8:16]
                    gam_s = fp_s[:, 16:17]
                    bet_s = fp_s[:, 17:18]
                    ex2 = stw.tile([128, 1], F32, tag="v1")
                    nc.vector.tensor_reduce(ex2[:], exp_s,
                                            mybir.AxisListType.X,
                                            mybir.AluOpType.add)
                    sx = stw.tile([128, 1], F32, tag="v0")
                    nc.vector.tensor_reduce(sx[:], sxp_s,
                                            mybir.AxisListType.X,
                                            mybir.AluOpType.add)
                    mu = stw.tile([128, 1], F32, tag="v2")
                    nc.vector.tensor_scalar_mul(mu[:], sx[:], 1.0 / N_true)
                    var = stw.tile([128, 1], F32, tag="v3")
                    nc.vector.tensor_scalar_mul(var[:], ex2[:], 1.0 / N_true)
                    mu2 = stw.tile([128, 1], F32, tag="v4")
                    nc.vector.tensor_tensor(mu2[:], mu[:], mu[:],
                                            mybir.AluOpType.mult)
                    nc.vector.tensor_tensor(var[:], var[:], mu2[:],
                                            mybir.AluOpType.subtract)
                    nc.vector.tensor_scalar_add(var[:], var[:], BN_EPS)
                    rec = stw.tile([128, 1], F32, tag="v5")
                    nc.vector.reciprocal(rec[:], var[:])
                    isd = stw.tile([128, 1], F32, tag="v6")
                    nc.scalar.activation(isd[:], rec[:],
                                         mybir.ActivationFunctionType.Sqrt)
                    a_c = stw.tile([128, 1], F32, tag="v7")
                    nc.vector.tensor_tensor(a_c[:], gam_s, isd[:],
                                            mybir.AluOpType.mult)
                    a8 = stw.tile([128, 1], F32, tag="v9")
                    nc.vector.tensor_scalar_mul(a8[:], a_c[:], 0.125)
                    nc.vector.tensor_scalar_mul(w_s[:], w1f_s, a8[:])
                    ca = stw.tile([128, 1], F32, tag="v8")
                    nc.vector.tensor_tensor(ca[:], mu[:], a_c[:],
                                            mybir.AluOpType.mult)
                    nc.vector.tensor_tensor(ca[:], bet_s, ca[:],
                                            mybir.AluOpType.subtract)
                    rw_ps = pst.tile([1, H], F32, tag="rw")
                    nc.tensor.matmul(rw_ps[:], ca[:], w1f_s,
                                     start=True, stop=True)
                    nc.scalar.activation(rw_s[:], rw_ps[:],
                                         mybir.ActivationFunctionType.Copy)

            spool = ctx.enter_context(tc.tile_pool(name="stg", bufs=1))
            wpool = ctx.enter_context(tc.tile_pool(name="wk", bufs=4))
            ps_agg = ctx.enter_context(
                tc.tile_pool(name="ps_agg", bufs=3, space="PSUM"))
            if lay < 2:
                ps_out = ctx.enter_context(
                    tc.tile_pool(name="ps_out", bufs=3, space="PSUM"))
            if lay == 1:
                ps_t = ctx.enter_context(
                    tc.tile_pool(name="ps_t", bufs=2, space="PSUM"))
                tstage = spool.tile([64, NT * 128], dt_out, tag="tstg")
            if lay == 2:
                ps_tr = ctx.enter_context(
                    tc.tile_pool(name="ps_tr", bufs=2, space="PSUM"))
                ps_pl = ctx.enter_context(
                    tc.tile_pool(name="ps_pl", bufs=1, space="PSUM"))
                pool_ps = ps_pl.tile([H2, G_PER], F32, tag="pool")

            if lay == 0:
                stage = spool.tile([128, NT * 128], dt_out, tag="stg")
            elif lay == 1:
                stage = spool.tile([128, NT * 128], BF16, tag="stg")

            state = {"use_dve": False, "rot": 0}

            def flip():
                state["use_dve"] = not state["use_dve"]
                return state["use_dve"]

            def rot():
                state["rot"] = (state["rot"] + 1) % 3
                return state["rot"]

            def split_copy(dst, src_ps, w):
                """PSUM->SBUF copy split across Act | DVE halves."""
                h = (w // 2 + 63) & ~63 if w > 128 else w
                nc.scalar.activation(dst[:, 0:h], src_ps[:, 0:h],
                                     mybir.ActivationFunctionType.Copy)
                if h < w:
                    nc.vector.tensor_copy(dst[:, h:w], src_ps[:, h:w])

            def split_relu(dst, src_ps, w):
                h = (w // 2 + 63) & ~63 if w > 128 else w
                nc.scalar.activation(dst[:, 0:h], src_ps[:, 0:h],
                                     mybir.ActivationFunctionType.Relu)
                if h < w:
                    nc.vector.tensor_scalar_max(dst[:, h:w],
                                                src_ps[:, h:w], 0.0)

            def split_relu_bias(dst, src_ps, w, bias):
                h = (w // 2 + 63) & ~63 if w > 128 else w
                nc.scalar.activation(dst[:, 0:h], src_ps[:, 0:h],
                                     mybir.ActivationFunctionType.Relu,
                                     bias=bias)
                if h < w:
                    nc.vector.tensor_scalar(dst[:, h:w], src_ps[:, h:w],
                                            bias, 0.0,
                                            mybir.AluOpType.add,
                                            mybir.AluOpType.max)

            def phase1(pr, dup_sb, b0):
                """agg matmuls (+ L2: bias + relu straight from PSUM)."""
                if dup_sb is None:   # lay0 tiles pre-aggregated in stats
                    return pr, None, agp_s[:, pr[0] * 128:(pr[-1] + 1) * 128]
                pw = len(pr) * 128
                rows = H2 if lay == 2 else 128
                agg_ps = ps_agg.tile([rows, pw], F32, tag="agg")
                nc.tensor.matmul(agg_ps[:], zr_s[0:1, 0:rows],
                                 zr_s[0:1, 0:pw], start=True, stop=False,
                                 skip_group_check=True)
                nb_pair = sum(int(kt[t]) for t in pr)
                bi = 0
                for hi, t in enumerate(pr):
                    for b, (lo, w) in enumerate(blocks[t]):
                        gb = int(tile_base[t] // 128) + b
                        co = pan_cols[t][b]
                        bi += 1
                        nc.tensor.matmul(
                            agg_ps[:, hi * 128 + lo:hi * 128 + lo + w],
                            dup_sb[:, (gb - b0) * F:(gb - b0 + 1) * F],
                            pan_sb[:, PAN_OFF + co - PBASE:
                                   PAN_OFF + co - PBASE + w],
                            start=False, stop=(bi == nb_pair),
                            skip_group_check=True)
                if lay == 2:
                    hsT = wpool.tile([H2, pw], BF16, tag="hsT")
                    if flip():
                        nc.vector.tensor_scalar(
                            hsT[:], agg_ps[:], b_s, 0.0,
                            mybir.AluOpType.add, mybir.AluOpType.max)
                    else:
                        nc.scalar.activation(
                            hsT[:], agg_ps[:],
                            mybir.ActivationFunctionType.Relu,
                            bias=b_s)
                    return pr, agg_ps, hsT
                aggT = wpool.tile([128, pw], BF16, tag="aggT")
                if flip():
                    nc.vector.tensor_copy(aggT[:], agg_ps[:])
                else:
                    nc.scalar.activation(aggT[:], agg_ps[:],
                                         mybir.ActivationFunctionType.Copy)
                return pr, agg_ps, aggT

            def phase2(st1):
                pr, agg_ps, aggT = st1
                pw = len(pr) * 128
                if lay < 2:
                    h_ps = ps_out.tile([Ho, pw], F32, tag="hps")
                    c0 = pr[0] * 128
                    nc.tensor.matmul(h_ps[:], w_s[:] if lay == 0 else w_s,
                                     aggT[:], start=True, stop=False,
                                     skip_group_check=True)
                    nc.tensor.matmul(h_ps[:], b_s,
                                     sig_s[0:1, c0:c0 + pw],
                                     start=False, stop=(lay != 0),
                                     skip_group_check=True)
                    if lay == 0:
                        nc.tensor.matmul(h_ps[:], rw_s[:],
                                         sh_s[0:1, c0:c0 + pw],
                                         start=False, stop=True,
                                         skip_group_check=True)
                    so = pr[0] * 128
                    if flip():
                        nc.vector.tensor_scalar_max(
                            stage[:, so:so + pw], h_ps[:], 0.0)
                    else:
                        nc.scalar.activation(
                            stage[:, so:so + pw], h_ps[:],
                            mybir.ActivationFunctionType.Relu)
                    return st1
                # lay 2: transpose each tile's hsT: [64, 128] -> [128, 64]
                hsT = aggT
                tr_ps = ps_tr.tile([128, len(pr) * H2], BF16, tag="tr")
                for hi, t in enumerate(pr):
                    nc.tensor.transpose(tr_ps[:, hi * H2:(hi + 1) * H2],
                                        hsT[:, hi * 128:(hi + 1) * 128],
                                        id_s[0:64, 0:64])
                hs_sb = wpool.tile([128, len(pr) * H2], BF16, tag="hs")
                if flip():
                    nc.vector.tensor_copy(hs_sb[:], tr_ps[:])
                else:
                    nc.scalar.activation(
                        hs_sb[:], tr_ps[:],
                        mybir.ActivationFunctionType.Copy)
                return [(t, hs_sb, hi * H2) for hi, t in enumerate(pr)]

            def phase3(st2):
                if lay == 2:
                    flip()          # odd flips/group: engines alternate
                if lay == 1:
                    pr = st2[0]
                    pw = len(pr) * 128
                    so = pr[0] * 128
                    t_ps = ps_t.tile([H2, pw], F32, tag="tps")
                    nc.tensor.matmul(t_ps[:], w3_s, stage[:, so:so + pw],
                                     start=True, stop=True,
                                     skip_group_check=True)
                    if flip():
                        nc.vector.tensor_copy(tstage[:, so:so + pw], t_ps[:])
                    else:
                        nc.scalar.activation(
                            tstage[:, so:so + pw], t_ps[:],
                            mybir.ActivationFunctionType.Copy)
                elif lay == 2:
                    for t, hs_sb, off in st2:
                        state["npool"] = state.get("npool", 0) + 1
                        nc.tensor.matmul(pool_ps[:],
                                         hs_sb[:, off:off + H2],
                                         gpan_s[:, t * G_PER:(t + 1) * G_PER],
                                         start=(state["npool"] == 1),
                                         stop=(state["npool"] == NT),
                                         skip_group_check=True)

            all_pairs = []
            for ci, tiles in enumerate(chunk_tiles):
                dup_sb, b0 = pend.pop(0)
                if ci + 1 < len(chunk_tiles):
                    pend.append(chunk_loads(chunk_tiles[ci + 1]))
                GW = 4
                grps = [tiles[i:i + GW] for i in range(0, len(tiles), GW)]
                for g in reversed(grps):
                    all_pairs.append((g, dup_sb, b0))
            if lay == 0 and TPRE:
                # pre-aggregated thin tiles: compute-only, processed last
                pg = [list(range(i, min(i + 4, TPRE)))
                      for i in range(0, TPRE, 4)]
                for g in reversed(pg):
                    all_pairs.append((g, None, None))

            hastail = lay > 0
            q2, q3 = [], []
            out_stage = stage if lay == 0 else (tstage if lay == 1 else None)
            OW = 128 if lay == 0 else 64
            wb = [NT, 24, 8, 2, 0]
            WRITES = [(wb[i + 1], wb[i]) for i in range(len(wb) - 1)]

            def maybe_write(done_min):
                if lay == 2:
                    return
                while WRITES and done_min <= WRITES[0][0]:
                    wt0, wt1 = WRITES.pop(0)
                    q = nc.sync if wt0 == 0 else nc.gpsimd
                    q.dma_start(
                        out=h_out[:, wt0 * 128:wt1 * 128],
                        in_=out_stage[:, wt0 * 128:wt1 * 128])

            def run3():
                st3 = q3.pop(0)
                phase3(st3)
                done = st3[0][0] if lay == 1 else st3[0][0]
                maybe_write(done)

            def run2():
                st2 = phase2(q2.pop(0))
                if hastail:
                    q3.append(st2)
                else:
                    maybe_write(st2[0][0])

            LAG2 = 1 if lay >= 1 else 2
            LAG3 = 1
            for item in all_pairs:
                st1 = phase1(*item)
                if len(q3) >= LAG3:
                    run3()
                if len(q2) >= LAG2:
                    run2()
                q2.append(st1)
            while q2 or q3:
                if q3:
                    run3()
                if q2:
                    run2()

            # ---- classifier MLP on this core's G_PER graphs (lay 2)
            if lay == 2:
                p01 = wpool.tile([H2, G_PER], F32, tag="p01")
                nc.vector.tensor_copy(p01[:], pool_ps[:])
                y_ps = ps_pl.tile([H4, G_PER], F32, tag="yps")
                nc.tensor.matmul(y_ps[:], wc1_s, p01[:],
                                 start=True, stop=False,
                                 skip_group_check=True)
                nc.tensor.matmul(y_ps[:], bc1_r, cntx_r,
                                 start=False, stop=True,
                                 skip_group_check=True)
                y_s = wpool.tile([H4, G_PER], F32, tag="ys")
                nc.vector.tensor_scalar_max(y_s[:], y_ps[:], 0.0)
                o_ps = ps_pl.tile([G_PER, C], F32, tag="ops")
                nc.tensor.matmul(o_ps[:], y_s[:], wc2_s,
                                 start=True, stop=False,
                                 skip_group_check=True)
                nc.tensor.matmul(o_ps[:], cntx_r, bc2_r,
                                 start=False, stop=True,
                                 skip_group_check=True)
                o_s = wpool.tile([G_PER, C], F32, tag="os")
                nc.scalar.activation(o_s[:], o_ps[:],
                                     mybir.ActivationFunctionType.Copy,
                                     scale=invc_c)
                nc.sync.dma_start(out=out_d[:], in_=o_s[:])

    nc.compile()
    return nc


# ------------------------------------------------------------------ driver
_CACHE = {}


def _get_programs(meta):
    key = (tuple(meta["kt"]), meta["n_true"], meta["NT"])
    if key not in _CACHE:
        progs = [_build_stats_program(meta)]
        progs += [_build_layer_program(meta, lay) for lay in range(3)]
        _CACHE[key] = progs
    return _CACHE[key]


def run_gnn(runner=None, **inputs):
    F, H, H2, H4, C = 128, 128, 64, 32, 2
    x = np.asarray(inputs["x"], np.float32)
    n_true = x.shape[0]
    src = np.asarray(inputs["edge_index"][0], np.int64)
    dst = np.asarray(inputs["edge_index"][1], np.int64)
    batch = np.asarray(inputs["batch"], np.int64)

    meta = _plan(src, dst, batch, n_true)
    NT, SHARD, NPAD = meta["NT"], meta["SHARD"], meta["NPAD"]
    cores = _build_static(meta, src, dst, batch)
    order = meta["order"]
    progs = _get_programs(meta)

    def run(nc, in_maps):
        if runner is not None:
            return runner(nc, in_maps)
        return run_bass_kernel_spmd(
            nc, in_maps, core_ids=list(range(N_CORES))).results

    x_new = np.zeros((NPAD + 1, F), np.float32)
    x_new[:NPAD][order < n_true] = x[order[order < n_true]]

    # ---- stats launch (BN partials + L0 pre-agg of tiles 0..TPRE-1)
    xb = x_new[:NPAD].astype(NPFP8)
    l0_dups = [_dup_layout(x_new, cores[c]["slotsrc"], DUP_NP[0])
               for c in range(N_CORES)]
    stats_maps = []
    for c in range(N_CORES):
        idx = ((np.arange(NT) * N_CORES + c)[:, None] * 128
               + np.arange(128)[None, :])
        slab = xb[idx]
        slab = np.ascontiguousarray(slab.transpose(1, 0, 2)).reshape(
            128, NT * F)
        stats_maps.append({
            "x_sh": slab, "ident": np.eye(128, dtype=np.float32)})
    res = run(progs[0], stats_maps)
    parts = np.stack([np.asarray(res[c]["stat_part"])
                      for c in range(N_CORES)], axis=2)
    sx_parts = np.ascontiguousarray(parts[:, 0, :], dtype=np.float32)
    ex2_parts = np.ascontiguousarray(parts[:, 1, :], dtype=np.float32)

    W = [np.asarray(inputs["W1"], np.float32),
         np.asarray(inputs["W2"], np.float32),
         np.asarray(inputs["W3"], np.float32)]
    brows = [np.asarray(inputs["b1"], np.float32).reshape(1, H),
             np.asarray(inputs["b2"], np.float32).reshape(1, H),
             np.asarray(inputs["b3"], np.float32).reshape(1, H2)]

    h_new = x_new
    core_out = None
    for lay in range(3):
        maps = []
        for c in range(N_CORES):
            st = cores[c]
            if lay == 0:
                rp = np.concatenate([st["sig_row"], st["sh_row"],
                                     brows[0].ravel()])
            elif lay == 1:
                rp = np.concatenate([st["sig_row"], brows[1].ravel()])
            else:
                rp = np.zeros((128, 65), np.float64)
                rp[0:64, 0:64] = np.eye(64)
            if lay == 1:
                pan = st["pans"][1]
            elif lay == 2:
                pan = np.concatenate([st["gpan"].astype(NPFP8),
                                      st["pans"][2]], axis=1)
            else:
                pan = st["pans"][0]
            m = {"dup": l0_dups[c] if lay == 0 else
                 _dup_layout(h_new, st["slotsrc"], DUP_NP[lay]),
                 "pan": np.ascontiguousarray(pan),
                 "rowpack": (rp.astype(NPBF16).reshape(1, -1) if lay < 2
                             else np.ascontiguousarray(rp.astype(NPBF16)))}
            if lay == 1:
                m["wpack"] = np.ascontiguousarray(np.concatenate(
                    [(W[1] / 8.0).astype(NPBF16), W[2].astype(NPBF16)],
                    axis=1))
            if lay == 0:
                fp = np.zeros((128, 18 + H), np.float32)
                fp[:, 0:8] = sx_parts
                fp[:, 8:16] = ex2_parts
                fp[:, 16] = np.asarray(inputs["bn_gamma"], np.float32)
                fp[:, 17] = np.asarray(inputs["bn_beta"], np.float32)
                fp[:, 18:] = W[0]
                m["f32pack"] = fp
            if lay == 2:
                mp = np.zeros((64, 80), np.float32)
                mp[:, 0:H4] = np.asarray(inputs["Wc1"], np.float32)
                mp[0:H4, H4:H4 + C] = np.asarray(inputs["Wc2"], np.float32)
                mp[0, 34:66] = np.asarray(inputs["bc1"], np.float32)
                mp[0, 66:74] = st["cntx"] * 4.0
                mp[0, 74:76] = np.asarray(inputs["bc2"], np.float32)
                mp[0:G_PER, 76] = st["invc"] / 4.0
                mp[0:64, 78] = np.asarray(inputs["b3"], np.float32) * 4.0
                m["mpack"] = mp
            maps.append(m)
        res = run(progs[1 + lay], maps)
        if lay < 2:
            OW = 128 if lay == 0 else 64
            h_new = np.zeros((NPAD + 1, OW), np.float32)
            for c in range(N_CORES):
                ho = np.asarray(res[c]["h_out"])
                hoT = ho.reshape(OW, NT, 128).transpose(1, 2, 0)
                idx = ((np.arange(NT) * N_CORES + c)[:, None] * 128
                       + np.arange(128)[None, :])
                h_new[idx] = hoT
        else:
            core_out = [np.asarray(res[c]["out"]) for c in range(N_CORES)]

    out = np.zeros((G, C), np.float32)
    for c in range(N_CORES):
        for lg, g in enumerate(meta["core_graphs"][c]):
            out[g] = core_out[c][lg]
    return out


def kernel(**inputs):
    return run_gnn(**inputs)
